# revision 1
# baseline (speedup 1.0000x reference)
"""Trainium2 Bass kernel for nn_BDH_1726576853700 (sparse_attention).

3-layer sparse-attention net: B=1, T=1024, D=256, NH=4, N=8192, VOCAB=256.

Sharding over 8 NeuronCores: device d -> (head h=d//2, half=d%2) — each device
owns a 4096-wide slice of one head's sparse latent dim.  Within the slice the
latent index is permuted evens-first so the RoPE pair partner sits exactly 2048
rows away (tile i <-> tile i+16), turning the pair rotation into whole-tile
elementwise ops.  Per layer:
  - x_sparse^T = relu(enc_w^T @ x^T)   (local)
  - qr = rope(x_sparse)                (local, host-precomputed cos/sin tables)
  - S_partial = qr^T qr (local n contraction), strictly-causal masked
  - ykv_partial = S_masked^T @ x ; pair AllReduce (the two halves of one head)
  - ykv_ln = layernorm(ykv); y_sparse^T = relu(encv_w^T @ ykv_ln^T) (local)
  - ymlp^T_partial = dec^T-contracted with (x_sparse * y_sparse)    (local)
  - 8-way AllReduce(ymlp); x = ln(x + ln(ymlp)) (replicated)
Collectives run in fp16 (halves wire bytes); matmuls run in fp16 with fp32
PSUM accumulation; the residual stream x is kept in fp32 on-chip.

PSUM discipline: every accumulation group owns its bank(s) exclusively —
`start=True` clears has_written bits for the WHOLE bank, so two interleaved
groups must never share a bank.
"""

import math
import sys

for _p in ("/opt/trn_rl_repo",):
    if _p not in sys.path:
        sys.path.insert(0, _p)

import numpy as np

import concourse.bass as bass
import concourse.mybir as mybir
import concourse.tile as tile
from concourse import bacc, bass_utils
from concourse.masks import make_identity

# ---- problem constants (hardcoded per contract) ----
B, T, D, NH, N = 1, 1024, 256, 4, 8192
VOCAB = 256
N_LAYER = 3
EPS = 1e-5
TWO_PI = 2.0 * math.pi
N_CORES = 8
NLOC = N // 2          # latent columns per device: 4096
P = 128
NT = T // P            # 8 t-tiles
KD = D // P            # 2 d-tiles
NM = NLOC // P         # 32 n-tiles per device
NPAIR = NM // 2        # 16 rope pairs
HDT = mybir.dt.float16     # on-chip activation dtype
F32 = mybir.dt.float32
YKV_SCALE = 1.0 / 256.0    # keeps ykv in fp16 range; LN downstream is
                           # scale-invariant so the result is unchanged

_CACHE = {}


def _build_program(dbg=False, use_collectives=True, rope_gpsimd=False, skip_scores=False, skip_proj=False, n_layers=N_LAYER):
    def emit_allreduce(nc, groups, ins, outs):
        if use_collectives:
            nc.gpsimd.collective_compute(
                "AllReduce", mybir.AluOpType.add, replica_groups=groups,
                ins=ins, outs=outs)
        else:
            # timing/sim variant: replace the collective with a plain copy
            nc.sync.dma_start(outs[0], ins[0])
    nc = bacc.Bacc("TRN2", target_bir_lowering=False, debug=False,
                   num_devices=N_CORES)
    dbg_tensors = {}
    if dbg:
        for nm, shape, dt in [
            ("dbg_x0ln", [T, D], F32),
            ("dbg_xsp", [NM * P, T], HDT),
            ("dbg_qr", [NM * P, T], HDT),
            ("dbg_ykvpre", [T, D], HDT),
            ("dbg_ykvpost", [T, D], HDT),
            ("dbg_ykvT", [D, T], HDT),
            ("dbg_ymlppre", [D, T], HDT),
            ("dbg_ymlppost", [D, T], HDT),
            ("dbg_x1", [T, D], F32),
        ]:
            dbg_tensors[nm] = nc.dram_tensor(nm, shape, dt,
                                             kind="ExternalOutput")

    x0_d = nc.dram_tensor("x0", [T, D], F32, kind="ExternalInput")
    encw_d = nc.dram_tensor("encw", [D, NLOC], HDT, kind="ExternalInput")
    encvw_d = nc.dram_tensor("encvw", [D, NLOC], HDT, kind="ExternalInput")
    decw_d = nc.dram_tensor("decw", [NLOC, D], HDT, kind="ExternalInput")
    ct_d = nc.dram_tensor("ct", [NLOC // 2, T], HDT, kind="ExternalInput")
    st_d = nc.dram_tensor("st", [NLOC // 2, T], HDT, kind="ExternalInput")
    lmh_d = nc.dram_tensor("lmh", [D, VOCAB], HDT, kind="ExternalInput")
    umask_d = nc.dram_tensor("umask", [P, P], F32, kind="ExternalInput")
    logits_d = nc.dram_tensor("logits", [T, VOCAB], F32, kind="ExternalOutput")

    PAIR_GROUPS = [[0, 1], [2, 3], [4, 5], [6, 7]]
    ALL_GROUP = [list(range(N_CORES))]

    with tile.TileContext(nc) as tc:
        persist = tc.alloc_tile_pool(name="persist", bufs=1)
        dram = tc.alloc_tile_pool(name="dram", bufs=1, space="DRAM")

        # persistent SBUF state
        x_sp = persist.tile([P, NM, T], HDT)        # x_sparse^T tiles
        qr = persist.tile([P, NM, T], HDT)          # roped x_sparse^T
        x_f32 = persist.tile([P, NT, D], F32)       # residual stream (natural)
        x_h = persist.tile([P, NT, D], HDT)         # x natural fp16
        xT_h = persist.tile([P, KD, T], HDT)        # x^T fp16
        ykvT_h = persist.tile([P, KD, T], HDT)      # ykv_ln^T fp16
        lmh_sb = persist.tile([P, KD, VOCAB], HDT)
        umask_sb = persist.tile([P, P], F32)
        ident = persist.tile([P, P], HDT)

        eps_sb = persist.tile([P, 1], F32)
        nc.vector.memset(eps_sb[:], float(EPS))
        nc.sync.dma_start(umask_sb[:], umask_d.ap())
        make_identity(nc, ident[:])
        for k in range(KD):
            nc.sync.dma_start(lmh_sb[:, k, :], lmh_d.ap()[k * P:(k + 1) * P, :])

        # streaming / working pools (live across the whole kernel)
        wenc = tc.alloc_tile_pool(name="wenc", bufs=3)
        wdec = tc.alloc_tile_pool(name="wdec", bufs=4)
        csp = tc.alloc_tile_pool(name="csp", bufs=2)
        ropep = tc.alloc_tile_pool(name="ropep", bufs=2)
        schp = tc.alloc_tile_pool(name="schp", bufs=2)
        sdp = tc.alloc_tile_pool(name="sdp", bufs=2)
        yxp = tc.alloc_tile_pool(name="yxp", bufs=2)
        arp = tc.alloc_tile_pool(name="arp", bufs=1)
        lnp = tc.alloc_tile_pool(name="lnp", bufs=2)
        statp = tc.alloc_tile_pool(name="statp", bufs=4)

        def layer_norm(src_ap, out_ap):
            """LayerNorm over the free dim (size D) of a [P, D] tile."""
            stats = statp.tile([P, 6], F32, name="ln_stats")
            mv = statp.tile([P, 2], F32, name="ln_mv")
            rstd = statp.tile([P, 1], F32, name="ln_rstd")
            nc.vector.bn_stats(out=stats[:], in_=src_ap)
            nc.vector.bn_aggr(out=mv[:], in_=stats[:])
            nc.scalar.activation(out=rstd[:], in_=mv[:, 1:2],
                                 func=mybir.ActivationFunctionType.Sqrt,
                                 bias=eps_sb[:])
            nc.vector.reciprocal(out=rstd[:], in_=rstd[:])
            nc.vector.tensor_scalar(out=out_ap, in0=src_ap,
                                    scalar1=mv[:, 0:1], scalar2=rstd[:],
                                    op0=mybir.AluOpType.subtract,
                                    op1=mybir.AluOpType.mult)

        def transpose_into(dst_ap, src_ap, pst_pool):
            """PE-transpose a [P, P] fp16 SBUF block into dst (via PSUM)."""
            pst = pst_pool.tile([P, P], HDT, name="pst")
            nc.tensor.transpose(pst[:], src_ap, ident[:])
            nc.vector.tensor_copy(out=dst_ap, in_=pst[:])

        def set_x_from(j, src_f32_ap, pst_pool):
            """Write x_f32/x_h/xT_h for t-tile j from a normalized f32 tile."""
            if src_f32_ap is not x_f32:
                nc.vector.tensor_copy(out=x_f32[:, j, :], in_=src_f32_ap)
            nc.scalar.copy(out=x_h[:, j, :], in_=x_f32[:, j, :])
            for k in range(KD):
                transpose_into(xT_h[:, k, j * P:(j + 1) * P],
                               x_h[:, j, k * P:(k + 1) * P], pst_pool)

        # ---- initial x = ln(embed[idx]) (gather done on host into x0) ----
        with tc.tile_pool(name="ps_init", bufs=2, space="PSUM") as ps_init:
            for j in range(NT):
                x0t = lnp.tile([P, D], F32, name="x0t")
                nc.sync.dma_start(x0t[:], x0_d.ap()[j * P:(j + 1) * P, :])
                layer_norm(x0t[:], x_f32[:, j, :])
                set_x_from(j, x_f32, ps_init)
        if dbg:
            nc.sync.dma_start(
                dbg_tensors["dbg_x0ln"].ap().rearrange("(j p) d -> p j d", p=P),
                x_f32[:])

        # ---- layers ----
        for layer in range(n_layers):
            # Phase A: x_sparse^T = relu(enc^T x^T), then rope -> qr
            with tc.tile_pool(name=f"psA_{layer}", bufs=2,
                              space="PSUM") as psA:
                for m in range(NM):
                    ps = psA.tile([P, T], F32, name="psA")
                    et = wenc.tile([P, KD, P], HDT, name="enc_t")
                    nc.sync.dma_start(
                        et[:],
                        encw_d.ap().rearrange("(k p) n -> p k n", p=P)[
                            :, :, m * P:(m + 1) * P])
                    for c in range(2):
                        for k in range(1 if skip_proj else KD):
                            nc.tensor.matmul(
                                ps[:, c * 512:(c + 1) * 512],
                                lhsT=et[:, k, :],
                                rhs=xT_h[:, k, c * 512:(c + 1) * 512],
                                start=(k == 0),
                                stop=(k == (0 if skip_proj else KD - 1)))
                    nc.scalar.activation(
                        out=x_sp[:, m, :], in_=ps[:],
                        func=mybir.ActivationFunctionType.Relu)

                for i in range(NPAIR):
                    ctt = csp.tile([P, T], HDT, name="ctt")
                    stt = csp.tile([P, T], HDT, name="stt")
                    nc.sync.dma_start(ctt[:], ct_d.ap()[i * P:(i + 1) * P, :])
                    nc.sync.dma_start(stt[:], st_d.ap()[i * P:(i + 1) * P, :])
                    xe = x_sp[:, i, :]
                    xo = x_sp[:, i + NPAIR, :]
                    t1 = ropep.tile([P, T], HDT, name="rope_t1")
                    t2 = ropep.tile([P, T], HDT, name="rope_t2")
                    eng2 = nc.gpsimd if rope_gpsimd else nc.vector
                    nc.vector.tensor_mul(t1[:], xe, ctt[:])
                    eng2.tensor_mul(t2[:], xo, stt[:])
                    nc.vector.tensor_sub(qr[:, i, :], t1[:], t2[:])
                    t3 = ropep.tile([P, T], HDT, name="rope_t1")
                    t4 = ropep.tile([P, T], HDT, name="rope_t2")
                    nc.vector.tensor_mul(t3[:], xo, ctt[:])
                    eng2.tensor_mul(t4[:], xe, stt[:])
                    nc.vector.tensor_add(qr[:, i + NPAIR, :], t3[:], t4[:])

            if dbg and layer == 0:
                nc.sync.dma_start(
                    dbg_tensors["dbg_xsp"].ap().rearrange(
                        "(m p) t -> p m t", p=P), x_sp[:])
                nc.sync.dma_start(
                    dbg_tensors["dbg_qr"].ap().rearrange(
                        "(m p) t -> p m t", p=P), qr[:])

            # Phase B: S partial + causal mask + ykv partial accumulation.
            # c-major passes so the 4 live ykv accumulators each own a full
            # PSUM bank (plus 2 rotating banks for S chunks).
            ykv_pre = arp.tile([P, NT, D], HDT, name="ykv_pre")
            for c in range(2):
                with tc.tile_pool(name=f"psS_{layer}_{c}", bufs=3,
                                  space="PSUM") as psS, \
                     tc.tile_pool(name=f"psY_{layer}_{c}", bufs=1,
                                  space="PSUM") as psY:
                    ykv_ps = [psY.tile([P, D], F32, name=f"ykv_ps{j}",
                                       tag=f"ykv_ps{j}")
                              for j in range(4 * c, 4 * c + 4)]
                    for i in range(4 * c + 4):
                        # causal tiling: only columns t >= i*P are needed
                        base = max(c * 512, i * P)
                        width = (c + 1) * 512 - base
                        ps = psS.tile([P, 512], F32, name="psS")
                        for k in range(1 if skip_scores else NM):
                            nc.tensor.matmul(
                                ps[:, :width],
                                lhsT=qr[:, k, i * P:(i + 1) * P],
                                rhs=qr[:, k, base:base + width],
                                start=(k == 0),
                                stop=(k == (0 if skip_scores else NM - 1)))
                        sc = schp.tile([P, 512], HDT, name="schunk")
                        if i % 2 == 0:
                            nc.scalar.copy(out=sc[:, :width],
                                           in_=ps[:, :width])
                        else:
                            nc.vector.tensor_copy(out=sc[:, :width],
                                                  in_=ps[:, :width])
                        sd = None
                        if c == i // 4:
                            dcol = i * P - base
                            sd = sdp.tile([P, P], HDT, name="sdiag")
                            nc.vector.tensor_mul(sd[:],
                                                 ps[:, dcol:dcol + P],
                                                 umask_sb[:])
                        for j in range(max(4 * c, i), 4 * c + 4):
                            lhsT = sd[:] if j == i else \
                                sc[:, j * P - base:(j + 1) * P - base]
                            nc.tensor.matmul(
                                ykv_ps[j - 4 * c][:], lhsT=lhsT,
                                rhs=x_h[:, i, :],
                                start=(i == 0), stop=(i == j))
                    for j in range(4 * c, 4 * c + 4):
                        nc.scalar.mul(out=ykv_pre[:, j, :],
                                      in_=ykv_ps[j - 4 * c][:],
                                      mul=YKV_SCALE)

            if dbg and layer == 0:
                nc.sync.dma_start(
                    dbg_tensors["dbg_ykvpre"].ap().rearrange(
                        "(j p) d -> p j d", p=P), ykv_pre[:])

            # Phase C: pair AllReduce of ykv, layernorm, transpose
            ar_in = dram.tile([T, D], HDT, name=f"arin_{layer}",
                              tag=f"arin_{layer}")
            ar_out = dram.tile([T, D], HDT, name=f"arout_{layer}",
                               tag=f"arout_{layer}")
            nc.sync.dma_start(
                ar_in.rearrange("(j p) d -> p j d", p=P), ykv_pre[:])
            emit_allreduce(nc, PAIR_GROUPS, [ar_in.opt()], [ar_out.opt()])
            ykv_post = arp.tile([P, NT, D], HDT, name="ykv_post")
            nc.sync.dma_start(
                ykv_post[:], ar_out.rearrange("(j p) d -> p j d", p=P))
            with tc.tile_pool(name=f"psT_{layer}", bufs=2,
                              space="PSUM") as psT:
                for j in range(NT):
                    yl = lnp.tile([P, D], HDT, name="ykv_ln")
                    layer_norm(ykv_post[:, j, :], yl[:])
                    for k in range(KD):
                        transpose_into(ykvT_h[:, k, j * P:(j + 1) * P],
                                       yl[:, k * P:(k + 1) * P], psT)

            if dbg and layer == 0:
                nc.sync.dma_start(
                    dbg_tensors["dbg_ykvpost"].ap().rearrange(
                        "(j p) d -> p j d", p=P), ykv_post[:])
                nc.sync.dma_start(
                    dbg_tensors["dbg_ykvT"].ap().rearrange(
                        "(k p) t -> p k t", p=P), ykvT_h[:])

            # Phase D: y_sparse^T = relu(encv^T ykv_ln^T); xy = x_sp * y_sp;
            # ymlp^T accumulated transposed: lhsT = decoder tile, rhs = xy.
            # ymlp^T psum tiles span 2 banks each with exactly one
            # accumulation group per bank.
            ymlpT_pre = arp.tile([P, KD, T], HDT, name="ymlpT_pre")
            with tc.tile_pool(name=f"psD_{layer}", bufs=2,
                              space="PSUM") as psD, \
                 tc.tile_pool(name=f"psM_{layer}", bufs=1,
                              space="PSUM") as psM:
                ymlpT_ps = [psM.tile([P, T], F32, name=f"ymlpT_ps{k}",
                                     tag=f"ymlpT_ps{k}") for k in range(KD)]
                for m in range(NM):
                    ps = psD.tile([P, T], F32, name="psD")
                    et = wenc.tile([P, KD, P], HDT, name="encv_t")
                    nc.sync.dma_start(
                        et[:],
                        encvw_d.ap().rearrange("(k p) n -> p k n", p=P)[
                            :, :, m * P:(m + 1) * P])
                    for c in range(2):
                        for k in range(KD):
                            nc.tensor.matmul(
                                ps[:, c * 512:(c + 1) * 512],
                                lhsT=et[:, k, :],
                                rhs=ykvT_h[:, k, c * 512:(c + 1) * 512],
                                start=(k == 0), stop=(k == KD - 1))
                    ysp = yxp.tile([P, T], HDT, name="ysp")
                    nc.scalar.activation(
                        out=ysp[:], in_=ps[:],
                        func=mybir.ActivationFunctionType.Relu)
                    xy = yxp.tile([P, T], HDT, name="xy")
                    nc.vector.tensor_mul(xy[:], x_sp[:, m, :], ysp[:])
                    dm = wdec.tile([P, D], HDT, name="dec_t")
                    nc.sync.dma_start(dm[:],
                                      decw_d.ap()[m * P:(m + 1) * P, :])
                    for k in range(KD):
                        for c in range(2):
                            nc.tensor.matmul(
                                ymlpT_ps[k][:, c * 512:(c + 1) * 512],
                                lhsT=dm[:, k * P:(k + 1) * P],
                                rhs=xy[:, c * 512:(c + 1) * 512],
                                start=(m == 0), stop=(m == NM - 1))
                for k in range(KD):
                    nc.scalar.copy(out=ymlpT_pre[:, k, :],
                                   in_=ymlpT_ps[k][:])

            if dbg and layer == 0:
                nc.sync.dma_start(
                    dbg_tensors["dbg_ymlppre"].ap().rearrange(
                        "(k p) t -> p k t", p=P), ymlpT_pre[:])

            # Phase E: 8-way AllReduce of ymlp^T; x = ln(x + ln(ymlp))
            ar2_in = dram.tile([D, T], HDT, name=f"ar2in_{layer}",
                               tag=f"ar2in_{layer}")
            ar2_out = dram.tile([D, T], HDT, name=f"ar2out_{layer}",
                                tag=f"ar2out_{layer}", addr_space="Shared")
            nc.sync.dma_start(
                ar2_in.rearrange("(k p) t -> p k t", p=P), ymlpT_pre[:])
            emit_allreduce(nc, ALL_GROUP, [ar2_in.opt()], [ar2_out.opt()])
            ymlpT_post = arp.tile([P, KD, T], HDT, name="ymlpT_post")
            nc.sync.dma_start(
                ymlpT_post[:], ar2_out.rearrange("(k p) t -> p k t", p=P))
            if dbg and layer == 0:
                nc.sync.dma_start(
                    dbg_tensors["dbg_ymlppost"].ap().rearrange(
                        "(k p) t -> p k t", p=P), ymlpT_post[:])
            with tc.tile_pool(name=f"psE_{layer}", bufs=2,
                              space="PSUM") as psE:
                for j in range(NT):
                    ymt = lnp.tile([P, D], HDT, name="ymt")
                    for k in range(KD):
                        transpose_into(ymt[:, k * P:(k + 1) * P],
                                       ymlpT_post[:, k, j * P:(j + 1) * P],
                                       psE)
                    u = lnp.tile([P, D], F32, name="u_ln")
                    layer_norm(ymt[:], u[:])
                    xn = lnp.tile([P, D], F32, name="xn")
                    nc.vector.tensor_add(xn[:], x_f32[:, j, :], u[:])
                    layer_norm(xn[:], x_f32[:, j, :])
                    set_x_from(j, x_f32, psE)
            if dbg and layer == 0:
                nc.sync.dma_start(
                    dbg_tensors["dbg_x1"].ap().rearrange(
                        "(j p) d -> p j d", p=P), x_f32[:])

        # ---- logits = x @ lm_head ----
        with tc.tile_pool(name="psL", bufs=2, space="PSUM") as psL:
            for j in range(NT):
                ps = psL.tile([P, VOCAB], F32, name="psLt")
                for k in range(KD):
                    nc.tensor.matmul(ps[:],
                                     lhsT=xT_h[:, k, j * P:(j + 1) * P],
                                     rhs=lmh_sb[:, k, :],
                                     start=(k == 0), stop=(k == KD - 1))
                lg = lnp.tile([P, VOCAB], F32, name="lgt")
                nc.scalar.copy(out=lg[:], in_=ps[:])
                nc.sync.dma_start(logits_d.ap()[j * P:(j + 1) * P, :], lg[:])

        for _pool in (statp, lnp, arp, yxp, sdp, schp, ropep, csp,
                      wdec, wenc, dram, persist):
            _pool.release()

    nc.compile()
    return nc


def _host_inputs(idx, embed, encoder, encoder_v, decoder, lm_head):
    """Build the 8 per-core input maps (host-side sharding)."""
    f16 = np.float16
    idx = np.asarray(idx).reshape(-1).astype(np.int64)
    embed = np.asarray(embed, np.float32)
    enc = np.asarray(encoder, np.float32)
    encv = np.asarray(encoder_v, np.float32)
    dec = np.asarray(decoder, np.float32)
    lmh = np.asarray(lm_head, np.float32)

    x0 = embed[idx]  # [T, D] gather on host (pure indexing)

    # freqs exactly as the reference computes them (fp32)
    t = np.arange(0, N, dtype=np.float32)
    q = np.floor(t / 2.0) * 2.0
    freqs = (1.0 / ((2.0 ** 16) ** (q / N)) / TWO_PI).astype(np.float32)
    tvec = np.arange(T, dtype=np.float32)

    umask = (np.arange(P)[:, None] < np.arange(P)[None, :]).astype(np.float32)

    in_maps = []
    for d in range(N_CORES):
        h, half = d // 2, d % 2
        perm = np.concatenate([np.arange(0, NLOC, 2),
                               np.arange(1, NLOC, 2)]) + half * NLOC
        f_loc = freqs[perm[:NLOC // 2]]
        ph = (tvec[None, :] * f_loc[:, None]).astype(np.float32) % 1.0
        in_maps.append({
            "x0": np.ascontiguousarray(x0, np.float32),
            "encw": np.ascontiguousarray(enc[h][:, perm], f16),
            "encvw": np.ascontiguousarray(encv[h][:, perm], f16),
            "decw": np.ascontiguousarray(dec[h * N + perm, :], f16),
            "ct": np.ascontiguousarray(np.cos(TWO_PI * ph), f16),
            "st": np.ascontiguousarray(np.sin(TWO_PI * ph), f16),
            "lmh": np.ascontiguousarray(lmh, f16),
            "umask": umask,
        })
    return in_maps


def kernel(idx, embed, encoder, encoder_v, decoder, lm_head,
           _trace=False, _tmpdir=None):
    if "nc" not in _CACHE:
        _CACHE["nc"] = _build_program()
    nc = _CACHE["nc"]
    in_maps = _host_inputs(idx, embed, encoder, encoder_v, decoder, lm_head)
    res = bass_utils.run_bass_kernel_spmd(
        nc, in_maps, core_ids=list(range(N_CORES)),
        trace=_trace, tmpdir=_tmpdir)
    _CACHE["last_results"] = res
    logits = res.results[0]["logits"].astype(np.float32).reshape(B, T, VOCAB)
    return logits



# revision 2
# speedup vs baseline: 1.3081x; 1.3081x over previous
"""Trainium2 Bass kernel for nn_BDH_1726576853700 (sparse_attention), v3.

3-layer sparse-attention net: B=1, T=1024, D=256, NH=4, N=8192, VOCAB=256.

Sharding over 8 NeuronCores: device d -> (head h=d//2, half=d%2) — each device
owns a 4096-wide slice of one head's sparse latent dim, permuted evens-first so
the RoPE pair partner is tile i+16.

v3 design:
  - S = qr^T qr runs in fp8e4 DoubleRow (2 k-tiles per matmul, 0.5 cycles/row).
    qr is quantized to fp8 at scale 32 (folded into the x_sparse relu evac);
    the scale washes out in the downstream LayerNorm.
  - enc/encv/dec weights resident in SBUF (shared by all 3 layers, loaded once).
  - cos/sin tables stream as ONE combined DMA per rope pair ([P, 2T] tile),
    alternating between the SP and Activation HWDGE queues.
  - Elementwise spread across Act/DVE/Pool with tunable splits; LayerNorm
    normalization on Act (Identity with per-partition scale/bias).
  - Engine streams are in-order: Phase C/E LayerNorm pipelines are emitted
    op-major (loop fission) so independent tiles don't serialize behind
    cross-engine latency chains.
  - Phase A emits m-tiles in (0,16,1,17,...) order and Phase B contracts
    k-pairs in (0,8,1,9,...) order so S matmuls chase the rope pipeline.
  - Phase D runs c-outer (two T-halves) so encv matmuls start on the first
    ykvT tiles right after the ykv AllReduce readback begins.

PSUM discipline: every accumulation group owns its bank(s) exclusively
(zero-region = 2KB = one bank).
"""

import math
import sys

for _p in ("/opt/trn_rl_repo",):
    if _p not in sys.path:
        sys.path.insert(0, _p)

import numpy as np

import concourse.bass as bass
import concourse.mybir as mybir
import concourse.tile as tile
from concourse import bacc, bass_utils
from concourse.masks import make_identity

# ---- problem constants (hardcoded per contract) ----
B, T, D, NH, N = 1, 1024, 256, 4, 8192
VOCAB = 256
N_LAYER = 3
EPS = 1e-5
TWO_PI = 2.0 * math.pi
N_CORES = 8
NLOC = N // 2          # latent columns per device: 4096
P = 128
NT = T // P            # 8 t-tiles
KD = D // P            # 2 d-tiles
NM = NLOC // P         # 32 n-tiles per device
NPAIR = NM // 2        # 16 rope pairs
HDT = mybir.dt.float16
F8 = mybir.dt.float8e4
F32 = mybir.dt.float32
DR = mybir.MatmulPerfMode.DoubleRow

XSP_SCALE = 32.0       # x_sparse stored at 32x so qr lands in fp8 normal range
SC_SCALE = 1.0 / 16.0  # S psum -> sc fp16 evacuation scale
YKV_SCALE = 1.0 / 256.0

_CACHE = {}


def _build_program(dbg=False, use_collectives=True, n_layers=N_LAYER,
                   n_pool_rope=0, qr8_split=(10, 4, 18), xsp_dve=0,
                   ysp_dve=16, sc_pool=False, sd_pool=True, ps_s_bufs=4):
    def emit_allreduce(nc, groups, ins, outs):
        if use_collectives:
            nc.gpsimd.collective_compute(
                "AllReduce", mybir.AluOpType.add, replica_groups=groups,
                ins=ins, outs=outs)
        else:
            nc.sync.dma_start(outs[0], ins[0])

    nc = bacc.Bacc("TRN2", target_bir_lowering=False, debug=False,
                   num_devices=N_CORES)
    dbg_tensors = {}
    if dbg:
        for nm, shape, dt in [
            ("dbg_x0ln", [T, D], F32),
            ("dbg_xsp", [NM * P, T], HDT),
            ("dbg_qr", [NM * P, T], F32),
            ("dbg_ykvpre", [T, D], HDT),
            ("dbg_ykvpost", [T, D], HDT),
            ("dbg_ykvT", [D, T], HDT),
            ("dbg_ymlppre", [D, T], HDT),
            ("dbg_ymlppost", [D, T], HDT),
            ("dbg_x1", [T, D], F32),
        ]:
            dbg_tensors[nm] = nc.dram_tensor(nm, shape, dt,
                                             kind="ExternalOutput")

    x0_d = nc.dram_tensor("x0", [T, D], HDT, kind="ExternalInput")
    encw_d = nc.dram_tensor("encw", [D, NLOC], HDT, kind="ExternalInput")
    encvw_d = nc.dram_tensor("encvw", [D, NLOC], HDT, kind="ExternalInput")
    decw_d = nc.dram_tensor("decw", [NLOC, D], HDT, kind="ExternalInput")
    cs_d = nc.dram_tensor("cs", [NLOC // 2, 2 * T], HDT, kind="ExternalInput")
    lmh_d = nc.dram_tensor("lmh", [D, VOCAB], HDT, kind="ExternalInput")
    umask_d = nc.dram_tensor("umask", [P, P], HDT, kind="ExternalInput")
    logits_d = nc.dram_tensor("logits", [T, VOCAB], F32, kind="ExternalOutput")

    PAIR_GROUPS = [[0, 1], [2, 3], [4, 5], [6, 7]]
    ALL_GROUP = [list(range(N_CORES))]

    # interleaved m emission order: pair halves adjacent (0,16,1,17,...)
    M_ORDER = []
    for i in range(NPAIR):
        M_ORDER += [i, i + NPAIR]
    # S contraction k-pair order: (qe pair u) then (qo pair u): 0,8,1,9,...
    K_ORDER = []
    for u in range(NPAIR // 2):
        K_ORDER += [u, u + NPAIR // 2]

    with tile.TileContext(nc) as tc:
        persist = tc.alloc_tile_pool(name="persist", bufs=1)
        dram = tc.alloc_tile_pool(name="dram", bufs=1, space="DRAM")

        # persistent SBUF state
        encw_sb = persist.tile([P, KD, NLOC], HDT)   # 16KB/part
        encvw_sb = persist.tile([P, KD, NLOC], HDT)  # 16KB
        decw_sb = persist.tile([P, NM, D], HDT)      # 16KB
        x_sp = persist.tile([P, NM, T], HDT)         # 64KB, stored *XSP_SCALE
        qr8 = persist.tile([P, NM, T], F8)           # 32KB
        x_f32 = persist.tile([P, NT, D], F32)        # 8KB residual stream
        x_h = persist.tile([P, NT, D], HDT)          # 4KB
        xT_h = persist.tile([P, KD, T], HDT)         # 4KB
        ykvT_h = persist.tile([P, KD, T], HDT)       # 4KB
        lmh_sb = persist.tile([P, KD, VOCAB], HDT)
        umask_sb = persist.tile([P, P], HDT)
        ident = persist.tile([P, P], HDT)
        eps_sb = persist.tile([P, 1], F32)

        nc.vector.memset(eps_sb[:], float(EPS))
        nc.sync.dma_start(umask_sb[:], umask_d.ap())
        make_identity(nc, ident[:])

        # streaming / working pools
        csp = tc.alloc_tile_pool(name="csp", bufs=2)
        ropep = tc.alloc_tile_pool(name="ropep", bufs=2)
        schp = tc.alloc_tile_pool(name="schp", bufs=2)
        sdp = tc.alloc_tile_pool(name="sdp", bufs=2)
        yxp = tc.alloc_tile_pool(name="yxp", bufs=2)
        arp = tc.alloc_tile_pool(name="arp", bufs=1)
        lnp = tc.alloc_tile_pool(name="lnp", bufs=2)
        statp = tc.alloc_tile_pool(name="statp", bufs=8)

        def ln_stats(src_ap):
            """Emit stats chain ops; returns (nmur, rstd) [P,1] tiles."""
            stats = statp.tile([P, 6], F32, name="ln_stats")
            mv = statp.tile([P, 2], F32, name="ln_mv")
            rstd = statp.tile([P, 1], F32, name="ln_rstd")
            nmur = statp.tile([P, 1], F32, name="ln_nmur")
            nc.vector.bn_stats(out=stats[:], in_=src_ap)
            nc.vector.bn_aggr(out=mv[:], in_=stats[:])
            nc.scalar.activation(out=rstd[:], in_=mv[:, 1:2],
                                 func=mybir.ActivationFunctionType.Sqrt,
                                 bias=eps_sb[:])
            nc.vector.reciprocal(out=rstd[:], in_=rstd[:])
            nc.vector.tensor_scalar(out=nmur[:], in0=mv[:, 0:1],
                                    scalar1=rstd[:], scalar2=-1.0,
                                    op0=mybir.AluOpType.mult,
                                    op1=mybir.AluOpType.mult)
            return nmur, rstd

        def ln_apply(src_ap, out_ap, nr):
            nc.scalar.activation(out=out_ap, in_=src_ap,
                                 func=mybir.ActivationFunctionType.Identity,
                                 bias=nr[0][:], scale=nr[1][:])

        def layer_norm(src_ap, out_ap):
            ln_apply(src_ap, out_ap, ln_stats(src_ap))

        # Pool/GPSIMD cannot touch PSUM on real HW; PSUM evacuations must go
        # to DVE or Act.  Alternate between them for the transpose drains.
        _tr_rr = [0]

        def transpose_into(dst_ap, src_ap, pst_pool, eng=None):
            """PE-transpose a [P, P] fp16 SBUF block into dst (via PSUM)."""
            pst = pst_pool.tile([P, P], HDT, name="pst")
            nc.tensor.transpose(pst[:], src_ap, ident[:])
            if eng is None:
                _tr_rr[0] += 1
                if _tr_rr[0] % 2 == 0:
                    nc.vector.tensor_copy(out=dst_ap, in_=pst[:])
                else:
                    nc.scalar.copy(out=dst_ap, in_=pst[:])
            else:
                eng.tensor_copy(out=dst_ap, in_=pst[:])

        def set_x_from(j, pst_pool):
            """Write x_h/xT_h for t-tile j from x_f32."""
            nc.scalar.copy(out=x_h[:, j, :], in_=x_f32[:, j, :])
            for k in range(KD):
                transpose_into(xT_h[:, k, j * P:(j + 1) * P],
                               x_h[:, j, k * P:(k + 1) * P], pst_pool)

        # ---- initial x = ln(embed[idx]) (gather done on host into x0) ----
        with tc.tile_pool(name="ps_init", bufs=2, space="PSUM") as ps_init:
            for j in range(NT):
                x0t = lnp.tile([P, D], HDT, name="x0t", tag="ln_f16", bufs=4)
                nc.sync.dma_start(x0t[:], x0_d.ap()[j * P:(j + 1) * P, :])
                layer_norm(x0t[:], x_f32[:, j, :])
                set_x_from(j, ps_init)
        # weights load behind the init pipeline (enc is needed first, at A0)
        nc.sync.dma_start(
            encw_sb[:], encw_d.ap().rearrange("(k p) n -> p k n", p=P))
        nc.scalar.dma_start(
            encvw_sb[:], encvw_d.ap().rearrange("(k p) n -> p k n", p=P))
        nc.scalar.dma_start(
            decw_sb[:], decw_d.ap().rearrange("(m p) d -> p m d", p=P))
        for k in range(KD):
            nc.scalar.dma_start(lmh_sb[:, k, :],
                                lmh_d.ap()[k * P:(k + 1) * P, :])
        if dbg:
            nc.sync.dma_start(
                dbg_tensors["dbg_x0ln"].ap().rearrange("(j p) d -> p j d", p=P),
                x_f32[:])

        # ---- layers ----
        for layer in range(n_layers):
            # Phase A: x_sparse^T = relu(enc^T x^T)*XSP_SCALE, rope -> qr8.
            # m emitted interleaved so rope pair i fires after its 2 evacs.
            qr8_engs = ([nc.scalar] * qr8_split[0] + [nc.vector] * qr8_split[1]
                        + [nc.gpsimd] * qr8_split[2])
            qr8_engs = [qr8_engs[(7 * z) % len(qr8_engs)]
                        for z in range(len(qr8_engs))]
            with tc.tile_pool(name=f"psA_{layer}", bufs=2,
                              space="PSUM") as psA:
                for mi, m in enumerate(M_ORDER):
                    ps = psA.tile([P, T], F32, name="psA")
                    for c in range(2):
                        for k in range(KD):
                            nc.tensor.matmul(
                                ps[:, c * 512:(c + 1) * 512],
                                lhsT=encw_sb[:, k, m * P:(m + 1) * P],
                                rhs=xT_h[:, k, c * 512:(c + 1) * 512],
                                start=(k == 0), stop=(k == KD - 1))
                    if (mi * xsp_dve) % NM < xsp_dve:
                        # fused relu+scale on DVE: (ps max 0) * XSP_SCALE
                        nc.vector.tensor_scalar(
                            out=x_sp[:, m, :], in0=ps[:],
                            scalar1=0.0, scalar2=float(XSP_SCALE),
                            op0=mybir.AluOpType.max,
                            op1=mybir.AluOpType.mult)
                    else:
                        nc.scalar.activation(
                            out=x_sp[:, m, :], in_=ps[:],
                            func=mybir.ActivationFunctionType.Relu,
                            scale=float(XSP_SCALE))
                    if mi % 2 == 1:
                        i = m - NPAIR  # pair index just completed
                        cst = csp.tile([P, 2, T], HDT, name="cst")
                        dma_eng = nc.sync if i % 2 == 0 else nc.scalar
                        dma_eng.dma_start(
                            cst[:], cs_d.ap().rearrange(
                                "n (two t) -> n two t",
                                two=2)[i * P:(i + 1) * P, :, :])
                        xe = x_sp[:, i, :]
                        xo = x_sp[:, i + NPAIR, :]
                        ctt, stt = cst[:, 0, :], cst[:, 1, :]
                        engs = [nc.vector] * 6
                        for t in range(n_pool_rope):
                            engs[5 - t] = nc.gpsimd
                        t1 = ropep.tile([P, T], HDT, name="rope_t1",
                                        tag="rope_t", bufs=4)
                        t2 = ropep.tile([P, T], HDT, name="rope_t2",
                                        tag="rope_t", bufs=4)
                        qe = ropep.tile([P, T], HDT, name="rope_qe",
                                        tag="rope_q")
                        engs[0].tensor_mul(t1[:], xe, ctt)
                        engs[1].tensor_mul(t2[:], xo, stt)
                        engs[2].tensor_sub(qe[:], t1[:], t2[:])
                        t3 = ropep.tile([P, T], HDT, name="rope_t3",
                                        tag="rope_t", bufs=4)
                        t4 = ropep.tile([P, T], HDT, name="rope_t4",
                                        tag="rope_t", bufs=4)
                        qo = ropep.tile([P, T], HDT, name="rope_qo",
                                        tag="rope_q")
                        engs[3].tensor_mul(t3[:], xo, ctt)
                        engs[4].tensor_mul(t4[:], xe, stt)
                        engs[5].tensor_add(qo[:], t3[:], t4[:])
                        for src, dst_m, e in (
                                (qe, i, qr8_engs[2 * i]),
                                (qo, i + NPAIR, qr8_engs[2 * i + 1])):
                            if e is nc.scalar:
                                e.copy(out=qr8[:, dst_m, :], in_=src[:])
                            else:
                                e.tensor_copy(out=qr8[:, dst_m, :],
                                              in_=src[:])

            if dbg and layer == 0:
                nc.sync.dma_start(
                    dbg_tensors["dbg_xsp"].ap().rearrange(
                        "(m p) t -> p m t", p=P), x_sp[:])
                for m in range(NM):
                    qd = lnp.tile([P, T], F32, name="qr_dbg", tag="qr_dbg")
                    nc.vector.tensor_copy(out=qd[:], in_=qr8[:, m, :])
                    nc.sync.dma_start(
                        dbg_tensors["dbg_qr"].ap().rearrange(
                            "(m p) t -> p m t", p=P)[:, m, :], qd[:])

            # Phase B: S partial (fp8 DoubleRow) + causal mask + ykv partial.
            # One PSUM pool pair across both c passes: S chunks of the second
            # half start while the rope still streams (4 rotating S banks,
            # ykv banks handed from c=0 to c=1 by tag rotation).
            ykv_pre = arp.tile([P, NT, D], HDT, name="ykv_pre",
                               tag="ar_stage")
            with tc.tile_pool(name=f"psS_{layer}", bufs=ps_s_bufs,
                              space="PSUM") as psS, \
                 tc.tile_pool(name=f"psY_{layer}", bufs=1,
                              space="PSUM") as psY:
                for c in range(2):
                    ykv_ps = [psY.tile([P, D], F32, name=f"ykv_ps{j}",
                                       tag=f"ykv_ps{j % 4}")
                              for j in range(4 * c, 4 * c + 4)]
                    for i in range(4 * c + 4):
                        # causal tiling: only columns t >= i*P are needed
                        base = max(c * 512, i * P)
                        width = (c + 1) * 512 - base
                        ps = psS.tile([P, 512], F32, name="psS")
                        for ku, u in enumerate(K_ORDER):
                            nc.tensor.matmul(
                                ps[:, :width],
                                lhsT=qr8[:, 2 * u:2 * u + 2,
                                         i * P:(i + 1) * P],
                                rhs=qr8[:, 2 * u:2 * u + 2,
                                        base:base + width],
                                start=(ku == 0), stop=(ku == NPAIR - 1),
                                perf_mode=DR)
                        sc = schp.tile([P, 512], HDT, name="schunk")
                        if sc_pool:
                            nc.gpsimd.tensor_scalar_mul(
                                sc[:, :width], ps[:, :width], float(SC_SCALE))
                        else:
                            nc.scalar.mul(out=sc[:, :width],
                                          in_=ps[:, :width],
                                          mul=float(SC_SCALE))
                        sd = None
                        if c == i // 4:
                            dcol = i * P - base
                            sd = sdp.tile([P, P], HDT, name="sdiag")
                            (nc.gpsimd if sd_pool else nc.vector).tensor_mul(
                                sd[:], sc[:, dcol:dcol + P], umask_sb[:])
                        for j in range(max(4 * c, i), 4 * c + 4):
                            lhsT = sd[:] if j == i else \
                                sc[:, j * P - base:(j + 1) * P - base]
                            nc.tensor.matmul(
                                ykv_ps[j - 4 * c][:], lhsT=lhsT,
                                rhs=x_h[:, i, :],
                                start=(i == 0), stop=(i == j))
                    for j in range(4 * c, 4 * c + 4):
                        nc.scalar.mul(out=ykv_pre[:, j, :],
                                      in_=ykv_ps[j - 4 * c][:],
                                      mul=float(YKV_SCALE))

            if dbg and layer == 0:
                nc.sync.dma_start(
                    dbg_tensors["dbg_ykvpre"].ap().rearrange(
                        "(j p) d -> p j d", p=P), ykv_pre[:])

            # Phase C: pair AllReduce of ykv, layernorm, transpose.
            # Chunked staging DMAs + op-major (fissioned) LN pipeline.
            ar_in = dram.tile([T, D], HDT, name=f"arin_{layer}",
                              tag=f"arin_{layer}")
            ar_out = dram.tile([T, D], HDT, name=f"arout_{layer}",
                               tag=f"arout_{layer}")
            arin_p = ar_in.rearrange("(j p) d -> p j d", p=P)
            nc.sync.dma_start(arin_p[:, 0:4, :], ykv_pre[:, 0:4, :])
            nc.sync.dma_start(arin_p[:, 4:8, :], ykv_pre[:, 4:8, :])
            emit_allreduce(nc, PAIR_GROUPS, [ar_in.opt()], [ar_out.opt()])
            ykv_post = arp.tile([P, NT, D], HDT, name="ykv_post",
                                tag="ar_stage")
            arout_p = ar_out.rearrange("(j p) d -> p j d", p=P)
            for jc in range(4):
                nc.sync.dma_start(ykv_post[:, 2 * jc:2 * jc + 2, :],
                                  arout_p[:, 2 * jc:2 * jc + 2, :])
            with tc.tile_pool(name=f"psT_{layer}", bufs=4,
                              space="PSUM") as psT:
                for jh in range(2):
                    jr = list(range(4 * jh, 4 * jh + 4))
                    nrs = [ln_stats(ykv_post[:, j, :]) for j in jr]
                    yls = []
                    for idx, j in enumerate(jr):
                        yl = lnp.tile([P, D], HDT, name="ykv_ln",
                                      tag="ln_f16", bufs=4)
                        ln_apply(ykv_post[:, j, :], yl[:], nrs[idx])
                        yls.append(yl)
                    for idx, j in enumerate(jr):
                        for k in range(KD):
                            transpose_into(
                                ykvT_h[:, k, j * P:(j + 1) * P],
                                yls[idx][:, k * P:(k + 1) * P], psT)

            if dbg and layer == 0:
                nc.sync.dma_start(
                    dbg_tensors["dbg_ykvpost"].ap().rearrange(
                        "(j p) d -> p j d", p=P), ykv_post[:])
                nc.sync.dma_start(
                    dbg_tensors["dbg_ykvT"].ap().rearrange(
                        "(k p) t -> p k t", p=P), ykvT_h[:])

            # Phase D: y_sp = relu(encv^T ykv_ln^T); xy = x_sp*y_sp;
            # ymlp^T accumulated with dec tiles as lhsT.  c-outer so the
            # first T-half starts as soon as ykvT columns 0..511 exist.
            ymlpT_pre = arp.tile([P, KD, T], HDT, name="ymlpT_pre",
                                 tag="ar_stage")
            ar2_in = dram.tile([D, T], HDT, name=f"ar2in_{layer}",
                               tag=f"ar2in_{layer}")
            ar2_out = dram.tile([D, T], HDT, name=f"ar2out_{layer}",
                                tag=f"ar2out_{layer}", addr_space="Shared")
            ar2in_p = ar2_in.rearrange("(k p) t -> p k t", p=P)
            with tc.tile_pool(name=f"psD_{layer}", bufs=3,
                              space="PSUM") as psD, \
                 tc.tile_pool(name=f"psM_{layer}", bufs=1,
                              space="PSUM") as psM:
                ymlpT_ps = [psM.tile([P, T], F32, name=f"ymlpT_ps{k}",
                                     tag=f"ymlpT_ps{k}") for k in range(KD)]
                for c in range(2):
                    cs = slice(c * 512, (c + 1) * 512)
                    for m in range(NM):
                        ps = psD.tile([P, 512], F32, name="psD")
                        for k in range(KD):
                            nc.tensor.matmul(
                                ps[:],
                                lhsT=encvw_sb[:, k, m * P:(m + 1) * P],
                                rhs=ykvT_h[:, k, cs],
                                start=(k == 0), stop=(k == KD - 1))
                        ysp = yxp.tile([P, 512], HDT, name="ysp")
                        if (m * ysp_dve) % NM < ysp_dve:
                            nc.vector.tensor_scalar_max(ysp[:], ps[:], 0.0)
                        else:
                            nc.scalar.activation(
                                out=ysp[:], in_=ps[:],
                                func=mybir.ActivationFunctionType.Relu)
                        xy = yxp.tile([P, 512], HDT, name="xy")
                        nc.vector.tensor_mul(xy[:], x_sp[:, m, cs], ysp[:])
                        for k in range(KD):
                            nc.tensor.matmul(
                                ymlpT_ps[k][:, cs],
                                lhsT=decw_sb[:, m, k * P:(k + 1) * P],
                                rhs=xy[:],
                                start=(m == 0), stop=(m == NM - 1))
                    for k in range(KD):
                        nc.scalar.copy(out=ymlpT_pre[:, k, cs],
                                       in_=ymlpT_ps[k][:, cs])
                        # upload this quarter while the next half computes
                        nc.sync.dma_start(ar2in_p[:, k, cs],
                                          ymlpT_pre[:, k, cs])

            if dbg and layer == 0:
                nc.sync.dma_start(
                    dbg_tensors["dbg_ymlppre"].ap().rearrange(
                        "(k p) t -> p k t", p=P), ymlpT_pre[:])

            # Phase E: 8-way AllReduce of ymlp^T; x = ln(x + ln(ymlp)).
            # Fissioned: transposes first, then the two LN chains op-major.
            emit_allreduce(nc, ALL_GROUP, [ar2_in.opt()], [ar2_out.opt()])
            ymlpT_post = arp.tile([P, KD, T], HDT, name="ymlpT_post",
                                  tag="ar_stage")
            ar2out_p = ar2_out.rearrange("(k p) t -> p k t", p=P)
            for kc in range(KD):
                nc.sync.dma_start(ymlpT_post[:, kc, :], ar2out_p[:, kc, :])
            if dbg and layer == 0:
                nc.sync.dma_start(
                    dbg_tensors["dbg_ymlppost"].ap().rearrange(
                        "(k p) t -> p k t", p=P), ymlpT_post[:])
            with tc.tile_pool(name=f"psE_{layer}", bufs=6,
                              space="PSUM") as psE:
                for jh in range(2):
                    jr = list(range(4 * jh, 4 * jh + 4))
                    ymts = {}
                    for j in jr:
                        ymt = lnp.tile([P, D], HDT, name="ymt",
                                       tag="ln_f16", bufs=4)
                        for k in range(KD):
                            transpose_into(
                                ymt[:, k * P:(k + 1) * P],
                                ymlpT_post[:, k, j * P:(j + 1) * P], psE)
                        ymts[j] = ymt
                    nrs = {j: ln_stats(ymts[j][:]) for j in jr}
                    us = {}
                    for j in jr:
                        u = lnp.tile([P, D], F32, name="u_ln",
                                     tag="ln_f32", bufs=3)
                        ln_apply(ymts[j][:], u[:], nrs[j])
                        us[j] = u
                    xns = {}
                    for j in jr:
                        xn = lnp.tile([P, D], F32, name="xn",
                                      tag="ln_f32x", bufs=3)
                        nc.vector.tensor_add(xn[:], x_f32[:, j, :], us[j][:])
                        xns[j] = xn
                    nrs2 = {j: ln_stats(xns[j][:]) for j in jr}
                    for j in jr:
                        ln_apply(xns[j][:], x_f32[:, j, :], nrs2[j])
                    for j in jr:
                        set_x_from(j, psE)
            if dbg and layer == 0:
                nc.sync.dma_start(
                    dbg_tensors["dbg_x1"].ap().rearrange(
                        "(j p) d -> p j d", p=P), x_f32[:])

        # ---- logits = x @ lm_head ----
        with tc.tile_pool(name="psL", bufs=2, space="PSUM") as psL:
            for j in range(NT):
                ps = psL.tile([P, VOCAB], F32, name="psLt")
                for k in range(KD):
                    nc.tensor.matmul(ps[:],
                                     lhsT=xT_h[:, k, j * P:(j + 1) * P],
                                     rhs=lmh_sb[:, k, :],
                                     start=(k == 0), stop=(k == KD - 1))
                lg = lnp.tile([P, VOCAB], F32, name="lgt", tag="ln_f32",
                              bufs=3)
                nc.scalar.copy(out=lg[:], in_=ps[:])
                nc.sync.dma_start(logits_d.ap()[j * P:(j + 1) * P, :], lg[:])

        for _pool in (statp, lnp, arp, yxp, sdp, schp, ropep, csp,
                      dram, persist):
            _pool.release()

    nc.compile()
    return nc


def _host_inputs(idx, embed, encoder, encoder_v, decoder, lm_head):
    """Build the 8 per-core input maps (host-side sharding)."""
    f16 = np.float16
    idx = np.asarray(idx).reshape(-1).astype(np.int64)
    embed = np.asarray(embed, np.float32)
    enc = np.asarray(encoder, np.float32)
    encv = np.asarray(encoder_v, np.float32)
    dec = np.asarray(decoder, np.float32)
    lmh = np.asarray(lm_head, np.float32)

    x0 = embed[idx]  # [T, D] gather on host (pure indexing)

    # freqs exactly as the reference computes them (fp32)
    t = np.arange(0, N, dtype=np.float32)
    q = np.floor(t / 2.0) * 2.0
    freqs = (1.0 / ((2.0 ** 16) ** (q / N)) / TWO_PI).astype(np.float32)
    tvec = np.arange(T, dtype=np.float32)

    umask = (np.arange(P)[:, None] < np.arange(P)[None, :]).astype(f16)

    in_maps = []
    for d in range(N_CORES):
        h, half = d // 2, d % 2
        perm = np.concatenate([np.arange(0, NLOC, 2),
                               np.arange(1, NLOC, 2)]) + half * NLOC
        f_loc = freqs[perm[:NLOC // 2]]
        ph = (tvec[None, :] * f_loc[:, None]).astype(np.float32) % 1.0
        cs = np.concatenate([np.cos(TWO_PI * ph), np.sin(TWO_PI * ph)],
                            axis=1)  # [NLOC//2, 2T]
        in_maps.append({
            "x0": np.ascontiguousarray(x0, f16),
            "encw": np.ascontiguousarray(enc[h][:, perm], f16),
            "encvw": np.ascontiguousarray(encv[h][:, perm], f16),
            "decw": np.ascontiguousarray(dec[h * N + perm, :], f16),
            "cs": np.ascontiguousarray(cs, f16),
            "lmh": np.ascontiguousarray(lmh, f16),
            "umask": umask,
        })
    return in_maps


def kernel(idx, embed, encoder, encoder_v, decoder, lm_head,
           _trace=False, _tmpdir=None):
    if "nc" not in _CACHE:
        _CACHE["nc"] = _build_program()
    nc = _CACHE["nc"]
    in_maps = _host_inputs(idx, embed, encoder, encoder_v, decoder, lm_head)
    res = bass_utils.run_bass_kernel_spmd(
        nc, in_maps, core_ids=list(range(N_CORES)),
        trace=_trace, tmpdir=_tmpdir)
    _CACHE["last_results"] = res
    logits = res.results[0]["logits"].astype(np.float32).reshape(B, T, VOCAB)
    return logits


# revision 3
# speedup vs baseline: 1.3143x; 1.0047x over previous
"""Trainium2 Bass kernel for nn_BDH_1726576853700 (sparse_attention), v3.

3-layer sparse-attention net: B=1, T=1024, D=256, NH=4, N=8192, VOCAB=256.

Sharding over 8 NeuronCores: device d -> (head h=d//2, half=d%2) — each device
owns a 4096-wide slice of one head's sparse latent dim, permuted evens-first so
the RoPE pair partner is tile i+16.

v3 design:
  - S = qr^T qr runs in fp8e4 DoubleRow (2 k-tiles per matmul, 0.5 cycles/row).
    qr is quantized to fp8 at scale 32 (folded into the x_sparse relu evac);
    the scale washes out in the downstream LayerNorm.
  - encv projection runs as three fp8 DoubleRow terms at a common scale:
    vh^T yh + vh^T yl + vl^T yh, where vh=fp8(128 encv), vl=fp8(128 encv-vh)
    (host side) and yh=fp8(ykv_ln^T), yl=fp8(ykv_ln^T-yh) (device side,
    extracted during the transpose drain).  Residuals live in fp8 subnormals;
    they carry ~3%% of the magnitude so their quantization noise is ~0.1%%
    of the result.  Validated end-to-end at rel err 2.0e-3.
  - enc/dec stay fp16: every single-fp8 variant of the three projections
    measured over the 2e-2 gate in numpy rehearsal (enc8+x8: 3.8e-2,
    encv8+ykv8: 4.2e-2, dec8+xy8: 4.6e-2); hi/lo for dec needs an xy
    residual extraction that costs more DVE time than the PE it saves.
  - enc/dec weights resident in SBUF (shared by all 3 layers, loaded once).
  - cos/sin tables stream as ONE combined DMA per rope pair ([P, 2T] tile),
    alternating between the SP and Activation HWDGE queues.
  - Elementwise spread across Act/DVE/Pool with tunable splits; LayerNorm
    normalization on Act (Identity with per-partition scale/bias).
  - Engine streams are in-order: Phase C/E LayerNorm pipelines are emitted
    op-major (loop fission) so independent tiles don't serialize behind
    cross-engine latency chains.
  - Phase A emits m-tiles in (0,16,1,17,...) order and Phase B contracts
    k-pairs in (0,8,1,9,...) order so S matmuls chase the rope pipeline.
  - Phase D runs c-outer (two T-halves) so encv matmuls start on the first
    ykvT tiles right after the ykv AllReduce readback begins.

PSUM discipline: every accumulation group owns its bank(s) exclusively
(zero-region = 2KB = one bank).
"""

import math
import sys

for _p in ("/opt/trn_rl_repo",):
    if _p not in sys.path:
        sys.path.insert(0, _p)

import numpy as np

import concourse.bass as bass
import concourse.mybir as mybir
import concourse.tile as tile
from concourse import bacc, bass_utils
from concourse.masks import make_identity

# ---- problem constants (hardcoded per contract) ----
B, T, D, NH, N = 1, 1024, 256, 4, 8192
VOCAB = 256
N_LAYER = 3
EPS = 1e-5
TWO_PI = 2.0 * math.pi
N_CORES = 8
NLOC = N // 2          # latent columns per device: 4096
P = 128
NT = T // P            # 8 t-tiles
KD = D // P            # 2 d-tiles
NM = NLOC // P         # 32 n-tiles per device
NPAIR = NM // 2        # 16 rope pairs
HDT = mybir.dt.float16
F8 = mybir.dt.float8e4
F32 = mybir.dt.float32
DR = mybir.MatmulPerfMode.DoubleRow

XSP_SCALE = 32.0       # x_sparse stored at 32x so qr lands in fp8 normal range
SC_SCALE = 1.0 / 16.0  # S psum -> sc fp16 evacuation scale
YKV_SCALE = 1.0 / 256.0

_CACHE = {}


def _build_program(dbg=False, use_collectives=True, n_layers=N_LAYER,
                   n_pool_rope=0, qr8_split=(10, 4, 18), xsp_dve=0,
                   ysp_dve=8, sc_pool=False, sd_pool=True, ps_s_bufs=4):
    def emit_allreduce(nc, groups, ins, outs):
        if use_collectives:
            nc.gpsimd.collective_compute(
                "AllReduce", mybir.AluOpType.add, replica_groups=groups,
                ins=ins, outs=outs)
        else:
            nc.sync.dma_start(outs[0], ins[0])

    nc = bacc.Bacc("TRN2", target_bir_lowering=False, debug=False,
                   num_devices=N_CORES)
    dbg_tensors = {}
    if dbg:
        for nm, shape, dt in [
            ("dbg_x0ln", [T, D], F32),
            ("dbg_xsp", [NM * P, T], HDT),
            ("dbg_qr", [NM * P, T], F32),
            ("dbg_ykvpre", [T, D], HDT),
            ("dbg_ykvpost", [T, D], HDT),
            ("dbg_ykvT", [D, T], HDT),
            ("dbg_ymlppre", [D, T], HDT),
            ("dbg_ymlppost", [D, T], HDT),
            ("dbg_x1", [T, D], F32),
        ]:
            dbg_tensors[nm] = nc.dram_tensor(nm, shape, dt,
                                             kind="ExternalOutput")

    x0_d = nc.dram_tensor("x0", [T, D], HDT, kind="ExternalInput")
    encw_d = nc.dram_tensor("encw", [D, NLOC], HDT, kind="ExternalInput")
    encvh_d = nc.dram_tensor("encvh", [D, NLOC], F8, kind="ExternalInput")
    encvl_d = nc.dram_tensor("encvl", [D, NLOC], F8, kind="ExternalInput")
    decw_d = nc.dram_tensor("decw", [NLOC, D], HDT, kind="ExternalInput")
    cs_d = nc.dram_tensor("cs", [NLOC // 2, 2 * T], HDT, kind="ExternalInput")
    lmh_d = nc.dram_tensor("lmh", [D, VOCAB], HDT, kind="ExternalInput")
    umask_d = nc.dram_tensor("umask", [P, P], HDT, kind="ExternalInput")
    logits_d = nc.dram_tensor("logits", [T, VOCAB], F32, kind="ExternalOutput")

    PAIR_GROUPS = [[0, 1], [2, 3], [4, 5], [6, 7]]
    ALL_GROUP = [list(range(N_CORES))]

    # interleaved m emission order: pair halves adjacent (0,16,1,17,...)
    M_ORDER = []
    for i in range(NPAIR):
        M_ORDER += [i, i + NPAIR]
    # S contraction k-pair order: (qe pair u) then (qo pair u): 0,8,1,9,...
    K_ORDER = []
    for u in range(NPAIR // 2):
        K_ORDER += [u, u + NPAIR // 2]

    with tile.TileContext(nc) as tc:
        persist = tc.alloc_tile_pool(name="persist", bufs=1)
        dram = tc.alloc_tile_pool(name="dram", bufs=1, space="DRAM")

        # persistent SBUF state
        encw_sb = persist.tile([P, KD, NLOC], HDT)   # 16KB/part
        encvh_sb = persist.tile([P, KD, NLOC], F8)   # 8KB (128*encv hi)
        encvl_sb = persist.tile([P, KD, NLOC], F8)   # 8KB (residual)
        decw_sb = persist.tile([P, NM, D], HDT)      # 16KB
        x_sp = persist.tile([P, NM, T], HDT)         # 64KB, stored *XSP_SCALE
        qr8 = persist.tile([P, NM, T], F8)           # 32KB
        x_f32 = persist.tile([P, NT, D], F32)        # 8KB residual stream
        x_h = persist.tile([P, NT, D], HDT)          # 4KB
        xT_h = persist.tile([P, KD, T], HDT)         # 4KB
        yh8_sb = persist.tile([P, KD, T], F8)        # 2KB ykv_ln^T hi
        yl1_sb = persist.tile([P, KD, T], F8)        # 2KB ykv_ln^T residual
        lmh_sb = persist.tile([P, KD, VOCAB], HDT)
        umask_sb = persist.tile([P, P], HDT)
        ident = persist.tile([P, P], HDT)
        eps_sb = persist.tile([P, 1], F32)

        nc.vector.memset(eps_sb[:], float(EPS))
        nc.sync.dma_start(umask_sb[:], umask_d.ap())
        make_identity(nc, ident[:])

        # streaming / working pools
        csp = tc.alloc_tile_pool(name="csp", bufs=2)
        ropep = tc.alloc_tile_pool(name="ropep", bufs=2)
        schp = tc.alloc_tile_pool(name="schp", bufs=2)
        sdp = tc.alloc_tile_pool(name="sdp", bufs=2)
        yxp = tc.alloc_tile_pool(name="yxp", bufs=2)
        arp = tc.alloc_tile_pool(name="arp", bufs=1)
        lnp = tc.alloc_tile_pool(name="lnp", bufs=2)
        statp = tc.alloc_tile_pool(name="statp", bufs=8)

        def ln_stats(src_ap):
            """Emit stats chain ops; returns (nmur, rstd) [P,1] tiles."""
            stats = statp.tile([P, 6], F32, name="ln_stats")
            mv = statp.tile([P, 2], F32, name="ln_mv")
            rstd = statp.tile([P, 1], F32, name="ln_rstd")
            nmur = statp.tile([P, 1], F32, name="ln_nmur")
            nc.vector.bn_stats(out=stats[:], in_=src_ap)
            nc.vector.bn_aggr(out=mv[:], in_=stats[:])
            nc.scalar.activation(out=rstd[:], in_=mv[:, 1:2],
                                 func=mybir.ActivationFunctionType.Sqrt,
                                 bias=eps_sb[:])
            nc.vector.reciprocal(out=rstd[:], in_=rstd[:])
            nc.vector.tensor_scalar(out=nmur[:], in0=mv[:, 0:1],
                                    scalar1=rstd[:], scalar2=-1.0,
                                    op0=mybir.AluOpType.mult,
                                    op1=mybir.AluOpType.mult)
            return nmur, rstd

        def ln_apply(src_ap, out_ap, nr):
            nc.scalar.activation(out=out_ap, in_=src_ap,
                                 func=mybir.ActivationFunctionType.Identity,
                                 bias=nr[0][:], scale=nr[1][:])

        def layer_norm(src_ap, out_ap):
            ln_apply(src_ap, out_ap, ln_stats(src_ap))

        # Pool/GPSIMD cannot touch PSUM on real HW; PSUM evacuations must go
        # to DVE or Act.  Alternate between them for the transpose drains.
        _tr_rr = [0]

        def transpose_into(dst_ap, src_ap, pst_pool, eng=None):
            """PE-transpose a [P, P] fp16 SBUF block into dst (via PSUM)."""
            pst = pst_pool.tile([P, P], HDT, name="pst")
            nc.tensor.transpose(pst[:], src_ap, ident[:])
            if eng is None:
                _tr_rr[0] += 1
                if _tr_rr[0] % 2 == 0:
                    nc.vector.tensor_copy(out=dst_ap, in_=pst[:])
                else:
                    nc.scalar.copy(out=dst_ap, in_=pst[:])
            else:
                eng.tensor_copy(out=dst_ap, in_=pst[:])

        def set_x_from(j, pst_pool):
            """Write x_h/xT_h for t-tile j from x_f32."""
            nc.scalar.copy(out=x_h[:, j, :], in_=x_f32[:, j, :])
            for k in range(KD):
                transpose_into(xT_h[:, k, j * P:(j + 1) * P],
                               x_h[:, j, k * P:(k + 1) * P], pst_pool)

        # ---- initial x = ln(embed[idx]) (gather done on host into x0) ----
        with tc.tile_pool(name="ps_init", bufs=2, space="PSUM") as ps_init:
            for j in range(NT):
                x0t = lnp.tile([P, D], HDT, name="x0t", tag="ln_f16", bufs=4)
                nc.sync.dma_start(x0t[:], x0_d.ap()[j * P:(j + 1) * P, :])
                layer_norm(x0t[:], x_f32[:, j, :])
                set_x_from(j, ps_init)
        # weights load behind the init pipeline (enc is needed first, at A0)
        nc.sync.dma_start(
            encw_sb[:], encw_d.ap().rearrange("(k p) n -> p k n", p=P))
        nc.scalar.dma_start(
            encvh_sb[:], encvh_d.ap().rearrange("(k p) n -> p k n", p=P))
        nc.scalar.dma_start(
            encvl_sb[:], encvl_d.ap().rearrange("(k p) n -> p k n", p=P))
        nc.scalar.dma_start(
            decw_sb[:], decw_d.ap().rearrange("(m p) d -> p m d", p=P))
        for k in range(KD):
            nc.scalar.dma_start(lmh_sb[:, k, :],
                                lmh_d.ap()[k * P:(k + 1) * P, :])
        if dbg:
            nc.sync.dma_start(
                dbg_tensors["dbg_x0ln"].ap().rearrange("(j p) d -> p j d", p=P),
                x_f32[:])

        # ---- layers ----
        for layer in range(n_layers):
            # Phase A: x_sparse^T = relu(enc^T x^T)*XSP_SCALE, rope -> qr8.
            # m emitted interleaved so rope pair i fires after its 2 evacs.
            qr8_engs = ([nc.scalar] * qr8_split[0] + [nc.vector] * qr8_split[1]
                        + [nc.gpsimd] * qr8_split[2])
            qr8_engs = [qr8_engs[(7 * z) % len(qr8_engs)]
                        for z in range(len(qr8_engs))]
            with tc.tile_pool(name=f"psA_{layer}", bufs=2,
                              space="PSUM") as psA:
                for mi, m in enumerate(M_ORDER):
                    ps = psA.tile([P, T], F32, name="psA")
                    for c in range(2):
                        for k in range(KD):
                            nc.tensor.matmul(
                                ps[:, c * 512:(c + 1) * 512],
                                lhsT=encw_sb[:, k, m * P:(m + 1) * P],
                                rhs=xT_h[:, k, c * 512:(c + 1) * 512],
                                start=(k == 0), stop=(k == KD - 1))
                    if (mi * xsp_dve) % NM < xsp_dve:
                        # fused relu+scale on DVE: (ps max 0) * XSP_SCALE
                        nc.vector.tensor_scalar(
                            out=x_sp[:, m, :], in0=ps[:],
                            scalar1=0.0, scalar2=float(XSP_SCALE),
                            op0=mybir.AluOpType.max,
                            op1=mybir.AluOpType.mult)
                    else:
                        nc.scalar.activation(
                            out=x_sp[:, m, :], in_=ps[:],
                            func=mybir.ActivationFunctionType.Relu,
                            scale=float(XSP_SCALE))
                    if mi % 2 == 1:
                        i = m - NPAIR  # pair index just completed
                        cst = csp.tile([P, 2, T], HDT, name="cst")
                        dma_eng = nc.sync if i % 2 == 0 else nc.scalar
                        dma_eng.dma_start(
                            cst[:], cs_d.ap().rearrange(
                                "n (two t) -> n two t",
                                two=2)[i * P:(i + 1) * P, :, :])
                        xe = x_sp[:, i, :]
                        xo = x_sp[:, i + NPAIR, :]
                        ctt, stt = cst[:, 0, :], cst[:, 1, :]
                        engs = [nc.vector] * 6
                        for t in range(n_pool_rope):
                            engs[5 - t] = nc.gpsimd
                        t1 = ropep.tile([P, T], HDT, name="rope_t1",
                                        tag="rope_t", bufs=4)
                        t2 = ropep.tile([P, T], HDT, name="rope_t2",
                                        tag="rope_t", bufs=4)
                        qe = ropep.tile([P, T], HDT, name="rope_qe",
                                        tag="rope_q")
                        engs[0].tensor_mul(t1[:], xe, ctt)
                        engs[1].tensor_mul(t2[:], xo, stt)
                        engs[2].tensor_sub(qe[:], t1[:], t2[:])
                        t3 = ropep.tile([P, T], HDT, name="rope_t3",
                                        tag="rope_t", bufs=4)
                        t4 = ropep.tile([P, T], HDT, name="rope_t4",
                                        tag="rope_t", bufs=4)
                        qo = ropep.tile([P, T], HDT, name="rope_qo",
                                        tag="rope_q")
                        engs[3].tensor_mul(t3[:], xo, ctt)
                        engs[4].tensor_mul(t4[:], xe, stt)
                        engs[5].tensor_add(qo[:], t3[:], t4[:])
                        for src, dst_m, e in (
                                (qe, i, qr8_engs[2 * i]),
                                (qo, i + NPAIR, qr8_engs[2 * i + 1])):
                            if e is nc.scalar:
                                e.copy(out=qr8[:, dst_m, :], in_=src[:])
                            else:
                                e.tensor_copy(out=qr8[:, dst_m, :],
                                              in_=src[:])

            if dbg and layer == 0:
                nc.sync.dma_start(
                    dbg_tensors["dbg_xsp"].ap().rearrange(
                        "(m p) t -> p m t", p=P), x_sp[:])
                for m in range(NM):
                    qd = lnp.tile([P, T], F32, name="qr_dbg", tag="qr_dbg")
                    nc.vector.tensor_copy(out=qd[:], in_=qr8[:, m, :])
                    nc.sync.dma_start(
                        dbg_tensors["dbg_qr"].ap().rearrange(
                            "(m p) t -> p m t", p=P)[:, m, :], qd[:])

            # Phase B: S partial (fp8 DoubleRow) + causal mask + ykv partial.
            # One PSUM pool pair across both c passes: S chunks of the second
            # half start while the rope still streams (4 rotating S banks,
            # ykv banks handed from c=0 to c=1 by tag rotation).
            ykv_pre = arp.tile([P, NT, D], HDT, name="ykv_pre",
                               tag="ar_stage")
            with tc.tile_pool(name=f"psS_{layer}", bufs=ps_s_bufs,
                              space="PSUM") as psS, \
                 tc.tile_pool(name=f"psY_{layer}", bufs=1,
                              space="PSUM") as psY:
                for c in range(2):
                    ykv_ps = [psY.tile([P, D], F32, name=f"ykv_ps{j}",
                                       tag=f"ykv_ps{j % 4}")
                              for j in range(4 * c, 4 * c + 4)]
                    for i in range(4 * c + 4):
                        # causal tiling: only columns t >= i*P are needed
                        base = max(c * 512, i * P)
                        width = (c + 1) * 512 - base
                        ps = psS.tile([P, 512], F32, name="psS")
                        for ku, u in enumerate(K_ORDER):
                            nc.tensor.matmul(
                                ps[:, :width],
                                lhsT=qr8[:, 2 * u:2 * u + 2,
                                         i * P:(i + 1) * P],
                                rhs=qr8[:, 2 * u:2 * u + 2,
                                        base:base + width],
                                start=(ku == 0), stop=(ku == NPAIR - 1),
                                perf_mode=DR)
                        sc = schp.tile([P, 512], HDT, name="schunk")
                        if sc_pool:
                            nc.gpsimd.tensor_scalar_mul(
                                sc[:, :width], ps[:, :width], float(SC_SCALE))
                        else:
                            nc.scalar.mul(out=sc[:, :width],
                                          in_=ps[:, :width],
                                          mul=float(SC_SCALE))
                        sd = None
                        if c == i // 4:
                            dcol = i * P - base
                            sd = sdp.tile([P, P], HDT, name="sdiag")
                            (nc.gpsimd if sd_pool else nc.vector).tensor_mul(
                                sd[:], sc[:, dcol:dcol + P], umask_sb[:])
                        for j in range(max(4 * c, i), 4 * c + 4):
                            lhsT = sd[:] if j == i else \
                                sc[:, j * P - base:(j + 1) * P - base]
                            nc.tensor.matmul(
                                ykv_ps[j - 4 * c][:], lhsT=lhsT,
                                rhs=x_h[:, i, :],
                                start=(i == 0), stop=(i == j))
                    for j in range(4 * c, 4 * c + 4):
                        nc.scalar.mul(out=ykv_pre[:, j, :],
                                      in_=ykv_ps[j - 4 * c][:],
                                      mul=float(YKV_SCALE))

            if dbg and layer == 0:
                nc.sync.dma_start(
                    dbg_tensors["dbg_ykvpre"].ap().rearrange(
                        "(j p) d -> p j d", p=P), ykv_pre[:])

            # Phase C: pair AllReduce of ykv, layernorm, transpose.
            # Chunked staging DMAs + op-major (fissioned) LN pipeline.
            ar_in = dram.tile([T, D], HDT, name=f"arin_{layer}",
                              tag=f"arin_{layer}")
            ar_out = dram.tile([T, D], HDT, name=f"arout_{layer}",
                               tag=f"arout_{layer}")
            arin_p = ar_in.rearrange("(j p) d -> p j d", p=P)
            nc.sync.dma_start(arin_p[:, 0:4, :], ykv_pre[:, 0:4, :])
            nc.sync.dma_start(arin_p[:, 4:8, :], ykv_pre[:, 4:8, :])
            emit_allreduce(nc, PAIR_GROUPS, [ar_in.opt()], [ar_out.opt()])
            ykv_post = arp.tile([P, NT, D], HDT, name="ykv_post",
                                tag="ar_stage")
            arout_p = ar_out.rearrange("(j p) d -> p j d", p=P)
            for jc in range(4):
                nc.sync.dma_start(ykv_post[:, 2 * jc:2 * jc + 2, :],
                                  arout_p[:, 2 * jc:2 * jc + 2, :])
            with tc.tile_pool(name=f"psT_{layer}", bufs=4,
                              space="PSUM") as psT:
                for jh in range(2):
                    jr = list(range(4 * jh, 4 * jh + 4))
                    nrs = [ln_stats(ykv_post[:, j, :]) for j in jr]
                    yls = []
                    for idx, j in enumerate(jr):
                        yl = lnp.tile([P, D], HDT, name="ykv_ln",
                                      tag="ln_f16", bufs=4)
                        ln_apply(ykv_post[:, j, :], yl[:], nrs[idx])
                        yls.append(yl)
                    for idx, j in enumerate(jr):
                        for k in range(KD):
                            pst = psT.tile([P, P], HDT, name="pst")
                            nc.tensor.transpose(
                                pst[:], yls[idx][:, k * P:(k + 1) * P],
                                ident[:])
                            dst = slice(j * P, (j + 1) * P)
                            nc.scalar.copy(out=yh8_sb[:, k, dst],
                                           in_=pst[:])
                            nc.vector.tensor_sub(yl1_sb[:, k, dst],
                                                 pst[:], yh8_sb[:, k, dst])

            if dbg and layer == 0:
                nc.sync.dma_start(
                    dbg_tensors["dbg_ykvpost"].ap().rearrange(
                        "(j p) d -> p j d", p=P), ykv_post[:])


            # Phase D: y_sp = relu(encv^T ykv_ln^T); xy = x_sp*y_sp;
            # ymlp^T accumulated with dec tiles as lhsT.  c-outer so the
            # first T-half starts as soon as ykvT columns 0..511 exist.
            ymlpT_pre = arp.tile([P, KD, T], HDT, name="ymlpT_pre",
                                 tag="ar_stage")
            ar2_in = dram.tile([D, T], HDT, name=f"ar2in_{layer}",
                               tag=f"ar2in_{layer}")
            ar2_out = dram.tile([D, T], HDT, name=f"ar2out_{layer}",
                                tag=f"ar2out_{layer}", addr_space="Shared")
            ar2in_p = ar2_in.rearrange("(k p) t -> p k t", p=P)
            with tc.tile_pool(name=f"psD_{layer}", bufs=3,
                              space="PSUM") as psD, \
                 tc.tile_pool(name=f"psM_{layer}", bufs=1,
                              space="PSUM") as psM:
                ymlpT_ps = [psM.tile([P, T], F32, name=f"ymlpT_ps{k}",
                                     tag=f"ymlpT_ps{k}") for k in range(KD)]
                for c in range(2):
                    cs = slice(c * 512, (c + 1) * 512)
                    for m in range(NM):
                        ps = psD.tile([P, 512], F32, name="psD")
                        msl = slice(m * P, (m + 1) * P)
                        terms = ((encvh_sb, yh8_sb), (encvh_sb, yl1_sb),
                                 (encvl_sb, yh8_sb))
                        for ti, (wsb, ysb) in enumerate(terms):
                            nc.tensor.matmul(
                                ps[:], lhsT=wsb[:, 0:2, msl],
                                rhs=ysb[:, 0:2, cs],
                                start=(ti == 0), stop=(ti == 2),
                                perf_mode=DR)
                        ysp = yxp.tile([P, 512], HDT, name="ysp")
                        if (m * ysp_dve) % NM < ysp_dve:
                            # relu + 1/128 unscale fused on DVE
                            nc.vector.tensor_scalar(
                                out=ysp[:], in0=ps[:],
                                scalar1=0.0, scalar2=1.0 / 128.0,
                                op0=mybir.AluOpType.max,
                                op1=mybir.AluOpType.mult)
                        else:
                            nc.scalar.activation(
                                out=ysp[:], in_=ps[:],
                                func=mybir.ActivationFunctionType.Relu,
                                scale=1.0 / 128.0)
                        xy = yxp.tile([P, 512], HDT, name="xy")
                        nc.vector.tensor_mul(xy[:], x_sp[:, m, cs], ysp[:])
                        for k in range(KD):
                            nc.tensor.matmul(
                                ymlpT_ps[k][:, cs],
                                lhsT=decw_sb[:, m, k * P:(k + 1) * P],
                                rhs=xy[:],
                                start=(m == 0), stop=(m == NM - 1))
                    for k in range(KD):
                        nc.scalar.copy(out=ymlpT_pre[:, k, cs],
                                       in_=ymlpT_ps[k][:, cs])
                        # upload this quarter while the next half computes
                        nc.sync.dma_start(ar2in_p[:, k, cs],
                                          ymlpT_pre[:, k, cs])

            if dbg and layer == 0:
                nc.sync.dma_start(
                    dbg_tensors["dbg_ymlppre"].ap().rearrange(
                        "(k p) t -> p k t", p=P), ymlpT_pre[:])

            # Phase E: 8-way AllReduce of ymlp^T; x = ln(x + ln(ymlp)).
            # Fissioned: transposes first, then the two LN chains op-major.
            emit_allreduce(nc, ALL_GROUP, [ar2_in.opt()], [ar2_out.opt()])
            ymlpT_post = arp.tile([P, KD, T], HDT, name="ymlpT_post",
                                  tag="ar_stage")
            ar2out_p = ar2_out.rearrange("(k p) t -> p k t", p=P)
            for kc in range(KD):
                nc.sync.dma_start(ymlpT_post[:, kc, :], ar2out_p[:, kc, :])
            if dbg and layer == 0:
                nc.sync.dma_start(
                    dbg_tensors["dbg_ymlppost"].ap().rearrange(
                        "(k p) t -> p k t", p=P), ymlpT_post[:])
            with tc.tile_pool(name=f"psE_{layer}", bufs=6,
                              space="PSUM") as psE:
                for jh in range(2):
                    jr = list(range(4 * jh, 4 * jh + 4))
                    ymts = {}
                    for j in jr:
                        ymt = lnp.tile([P, D], HDT, name="ymt",
                                       tag="ln_f16", bufs=4)
                        for k in range(KD):
                            transpose_into(
                                ymt[:, k * P:(k + 1) * P],
                                ymlpT_post[:, k, j * P:(j + 1) * P], psE)
                        ymts[j] = ymt
                    nrs = {j: ln_stats(ymts[j][:]) for j in jr}
                    us = {}
                    for j in jr:
                        u = lnp.tile([P, D], F32, name="u_ln",
                                     tag="ln_f32", bufs=3)
                        ln_apply(ymts[j][:], u[:], nrs[j])
                        us[j] = u
                    xns = {}
                    for j in jr:
                        xn = lnp.tile([P, D], F32, name="xn",
                                      tag="ln_f32x", bufs=3)
                        nc.vector.tensor_add(xn[:], x_f32[:, j, :], us[j][:])
                        xns[j] = xn
                    nrs2 = {j: ln_stats(xns[j][:]) for j in jr}
                    for j in jr:
                        ln_apply(xns[j][:], x_f32[:, j, :], nrs2[j])
                    for j in jr:
                        set_x_from(j, psE)
            if dbg and layer == 0:
                nc.sync.dma_start(
                    dbg_tensors["dbg_x1"].ap().rearrange(
                        "(j p) d -> p j d", p=P), x_f32[:])

        # ---- logits = x @ lm_head ----
        with tc.tile_pool(name="psL", bufs=2, space="PSUM") as psL:
            for j in range(NT):
                ps = psL.tile([P, VOCAB], F32, name="psLt")
                for k in range(KD):
                    nc.tensor.matmul(ps[:],
                                     lhsT=xT_h[:, k, j * P:(j + 1) * P],
                                     rhs=lmh_sb[:, k, :],
                                     start=(k == 0), stop=(k == KD - 1))
                lg = lnp.tile([P, VOCAB], F32, name="lgt", tag="ln_f32",
                              bufs=3)
                nc.scalar.copy(out=lg[:], in_=ps[:])
                nc.sync.dma_start(logits_d.ap()[j * P:(j + 1) * P, :], lg[:])

        for _pool in (statp, lnp, arp, yxp, sdp, schp, ropep, csp,
                      dram, persist):
            _pool.release()

    nc.compile()
    return nc


def _host_inputs(idx, embed, encoder, encoder_v, decoder, lm_head):
    """Build the 8 per-core input maps (host-side sharding)."""
    import ml_dtypes
    f8e4 = ml_dtypes.float8_e4m3fn
    f16 = np.float16
    idx = np.asarray(idx).reshape(-1).astype(np.int64)
    embed = np.asarray(embed, np.float32)
    enc = np.asarray(encoder, np.float32)
    encv = np.asarray(encoder_v, np.float32)
    dec = np.asarray(decoder, np.float32)
    lmh = np.asarray(lm_head, np.float32)

    x0 = embed[idx]  # [T, D] gather on host (pure indexing)

    # freqs exactly as the reference computes them (fp32)
    t = np.arange(0, N, dtype=np.float32)
    q = np.floor(t / 2.0) * 2.0
    freqs = (1.0 / ((2.0 ** 16) ** (q / N)) / TWO_PI).astype(np.float32)
    tvec = np.arange(T, dtype=np.float32)

    umask = (np.arange(P)[:, None] < np.arange(P)[None, :]).astype(f16)

    in_maps = []
    for d in range(N_CORES):
        h, half = d // 2, d % 2
        perm = np.concatenate([np.arange(0, NLOC, 2),
                               np.arange(1, NLOC, 2)]) + half * NLOC
        encv128 = (encv[h][:, perm] * 128.0).astype(np.float32)
        encvh8 = encv128.astype(f8e4)
        encvl8 = (encv128 - encvh8.astype(np.float32)).astype(f8e4)
        encvh8 = np.ascontiguousarray(encvh8)
        encvl8 = np.ascontiguousarray(encvl8)
        f_loc = freqs[perm[:NLOC // 2]]
        ph = (tvec[None, :] * f_loc[:, None]).astype(np.float32) % 1.0
        cs = np.concatenate([np.cos(TWO_PI * ph), np.sin(TWO_PI * ph)],
                            axis=1)  # [NLOC//2, 2T]
        in_maps.append({
            "x0": np.ascontiguousarray(x0, f16),
            "encw": np.ascontiguousarray(enc[h][:, perm], f16),
            "encvh": encvh8,
            "encvl": encvl8,
            "decw": np.ascontiguousarray(dec[h * N + perm, :], f16),
            "cs": np.ascontiguousarray(cs, f16),
            "lmh": np.ascontiguousarray(lmh, f16),
            "umask": umask,
        })
    return in_maps


def kernel(idx, embed, encoder, encoder_v, decoder, lm_head,
           _trace=False, _tmpdir=None):
    if "nc" not in _CACHE:
        _CACHE["nc"] = _build_program()
    nc = _CACHE["nc"]
    in_maps = _host_inputs(idx, embed, encoder, encoder_v, decoder, lm_head)
    res = bass_utils.run_bass_kernel_spmd(
        nc, in_maps, core_ids=list(range(N_CORES)),
        trace=_trace, tmpdir=_tmpdir)
    _CACHE["last_results"] = res
    logits = res.results[0]["logits"].astype(np.float32).reshape(B, T, VOCAB)
    return logits


# revision 4
# speedup vs baseline: 1.3463x; 1.0243x over previous
"""Trainium2 Bass kernel for nn_BDH_1726576853700 (sparse_attention), v3.

3-layer sparse-attention net: B=1, T=1024, D=256, NH=4, N=8192, VOCAB=256.

Sharding over 8 NeuronCores: device d -> (head h=d//2, half=d%2) — each device
owns a 4096-wide slice of one head's sparse latent dim, permuted evens-first so
the RoPE pair partner is tile i+16.

v3 design:
  - S = qr^T qr runs in fp8e4 DoubleRow (2 k-tiles per matmul, 0.5 cycles/row).
    qr is quantized to fp8 at scale 32 (folded into the x_sparse relu evac);
    the scale washes out in the downstream LayerNorm.
  - encv projection runs as three fp8 DoubleRow terms at a common scale:
    vh^T yh + vh^T yl + vl^T yh, where vh=fp8(128 encv), vl=fp8(128 encv-vh)
    (host side) and yh=fp8(ykv_ln^T), yl=fp8(ykv_ln^T-yh) (device side,
    extracted during the transpose drain).  Residuals live in fp8 subnormals;
    they carry ~3%% of the magnitude so their quantization noise is ~0.1%%
    of the result.  Validated end-to-end at rel err 2.0e-3.
  - enc/dec stay fp16: every single-fp8 variant of the three projections
    measured over the 2e-2 gate in numpy rehearsal (enc8+x8: 3.8e-2,
    encv8+ykv8: 4.2e-2, dec8+xy8: 4.6e-2); hi/lo for dec needs an xy
    residual extraction that costs more DVE time than the PE it saves.
  - enc/dec weights resident in SBUF (shared by all 3 layers, loaded once).
  - cos/sin tables stream as ONE combined DMA per rope pair ([P, 2T] tile),
    alternating between the SP and Activation HWDGE queues.
  - Elementwise spread across Act/DVE/Pool with tunable splits; LayerNorm
    normalization on Act (Identity with per-partition scale/bias).
  - Engine streams are in-order: Phase C/E LayerNorm pipelines are emitted
    op-major (loop fission) so independent tiles don't serialize behind
    cross-engine latency chains.
  - Phase A emits m-tiles in (0,16,1,17,...) order and Phase B contracts
    k-pairs in (0,8,1,9,...) order so S matmuls chase the rope pipeline.
  - Phase D runs c-outer (two T-halves) so encv matmuls start on the first
    ykvT tiles right after the ykv AllReduce readback begins.

PSUM discipline: every accumulation group owns its bank(s) exclusively
(zero-region = 2KB = one bank).
"""

import math
import sys

for _p in ("/opt/trn_rl_repo",):
    if _p not in sys.path:
        sys.path.insert(0, _p)

import numpy as np

import concourse.bass as bass
import concourse.mybir as mybir
import concourse.tile as tile
from concourse import bacc, bass_utils
from concourse.masks import make_identity

# ---- problem constants (hardcoded per contract) ----
B, T, D, NH, N = 1, 1024, 256, 4, 8192
VOCAB = 256
N_LAYER = 3
EPS = 1e-5
TWO_PI = 2.0 * math.pi
N_CORES = 8
NLOC = N // 2          # latent columns per device: 4096
P = 128
NT = T // P            # 8 t-tiles
KD = D // P            # 2 d-tiles
NM = NLOC // P         # 32 n-tiles per device
NPAIR = NM // 2        # 16 rope pairs
HDT = mybir.dt.float16
F8 = mybir.dt.float8e4
F32 = mybir.dt.float32
DR = mybir.MatmulPerfMode.DoubleRow

XSP_SCALE = 32.0       # x_sparse stored at 32x so qr lands in fp8 normal range
SC_SCALE = 1.0 / 16.0  # S psum -> sc fp16 evacuation scale
YKV_SCALE = 1.0 / 256.0

_CACHE = {}


def _build_program(dbg=False, use_collectives=True, n_layers=N_LAYER,
                   n_pool_rope=0, qr8_split=(10, 4, 18), xsp_dve=0,
                   ysp_dve=8, sc_pool=False, sd_pool=True, ps_s_bufs=4):
    def emit_allreduce(nc, groups, ins, outs):
        if use_collectives:
            nc.gpsimd.collective_compute(
                "AllReduce", mybir.AluOpType.add, replica_groups=groups,
                ins=ins, outs=outs)
        else:
            nc.sync.dma_start(outs[0], ins[0])

    nc = bacc.Bacc("TRN2", target_bir_lowering=False, debug=False,
                   num_devices=N_CORES)
    dbg_tensors = {}
    if dbg:
        for nm, shape, dt in [
            ("dbg_x0ln", [T, D], F32),
            ("dbg_xsp", [NM * P, T], HDT),
            ("dbg_qr", [NM * P, T], F32),
            ("dbg_ykvpre", [T, D], HDT),
            ("dbg_ykvpost", [T, D], HDT),
            ("dbg_ykvT", [D, T], HDT),
            ("dbg_ymlppre", [D, T], HDT),
            ("dbg_ymlppost", [D, T], HDT),
            ("dbg_x1", [T, D], F32),
        ]:
            dbg_tensors[nm] = nc.dram_tensor(nm, shape, dt,
                                             kind="ExternalOutput")

    x0_d = nc.dram_tensor("x0", [T, D], HDT, kind="ExternalInput")
    encw_d = nc.dram_tensor("encw", [D, NLOC], HDT, kind="ExternalInput")
    encvh_d = nc.dram_tensor("encvh", [D, NLOC], F8, kind="ExternalInput")
    encvl_d = nc.dram_tensor("encvl", [D, NLOC], F8, kind="ExternalInput")
    decw_d = nc.dram_tensor("decw", [NLOC, D], HDT, kind="ExternalInput")
    cs_d = nc.dram_tensor("cs", [NLOC // 2, 2 * T], HDT, kind="ExternalInput")
    lmh_d = nc.dram_tensor("lmh", [D, VOCAB], HDT, kind="ExternalInput")
    umask_d = nc.dram_tensor("umask", [P, P], HDT, kind="ExternalInput")
    logits_d = nc.dram_tensor("logits", [T, VOCAB], F32, kind="ExternalOutput")

    PAIR_GROUPS = [[0, 1], [2, 3], [4, 5], [6, 7]]
    ALL_GROUP = [list(range(N_CORES))]

    # interleaved m emission order: pair halves adjacent (0,16,1,17,...)
    M_ORDER = []
    for i in range(NPAIR):
        M_ORDER += [i, i + NPAIR]
    # S contraction k-pair order: (qe pair u) then (qo pair u): 0,8,1,9,...
    K_ORDER = []
    for u in range(NPAIR // 2):
        K_ORDER += [u, u + NPAIR // 2]

    with tile.TileContext(nc) as tc:
        persist = tc.alloc_tile_pool(name="persist", bufs=1)
        dram = tc.alloc_tile_pool(name="dram", bufs=1, space="DRAM")

        # persistent SBUF state
        encw_sb = persist.tile([P, KD, NLOC], HDT)   # 16KB/part
        encvh_sb = persist.tile([P, KD, NLOC], F8)   # 8KB (128*encv hi)
        encvl_sb = persist.tile([P, KD, NLOC], F8)   # 8KB (residual)
        decw_sb = persist.tile([P, NM, D], HDT)      # 16KB
        x_sp = persist.tile([P, NM, T], HDT)         # 64KB, stored *XSP_SCALE
        qr8 = persist.tile([P, NM, T], F8)           # 32KB
        x_f32 = persist.tile([P, NT, D], F32)        # 8KB residual stream
        x_h = persist.tile([P, NT, D], HDT)          # 4KB
        xT_h = persist.tile([P, KD, T], HDT)         # 4KB
        yh8_sb = persist.tile([P, KD, T], F8)        # 2KB ykv_ln^T hi
        yl1_sb = persist.tile([P, KD, T], F8)        # 2KB ykv_ln^T residual
        lmh_sb = persist.tile([P, KD, VOCAB], HDT)
        umask_sb = persist.tile([P, P], HDT)
        ident = persist.tile([P, P], HDT)
        eps_sb = persist.tile([P, 1], F32)

        nc.vector.memset(eps_sb[:], float(EPS))
        nc.sync.dma_start(umask_sb[:], umask_d.ap())
        make_identity(nc, ident[:])

        # streaming / working pools
        csp = tc.alloc_tile_pool(name="csp", bufs=2)
        ropep = tc.alloc_tile_pool(name="ropep", bufs=2)
        schp = tc.alloc_tile_pool(name="schp", bufs=2)
        sdp = tc.alloc_tile_pool(name="sdp", bufs=2)
        yxp = tc.alloc_tile_pool(name="yxp", bufs=2)
        arp = tc.alloc_tile_pool(name="arp", bufs=1)
        lnp = tc.alloc_tile_pool(name="lnp", bufs=2)
        statp = tc.alloc_tile_pool(name="statp", bufs=8)

        def ln_stats(src_ap):
            """Emit stats chain ops; returns (nmur, rstd) [P,1] tiles."""
            stats = statp.tile([P, 6], F32, name="ln_stats")
            mv = statp.tile([P, 2], F32, name="ln_mv")
            rstd = statp.tile([P, 1], F32, name="ln_rstd")
            nmur = statp.tile([P, 1], F32, name="ln_nmur")
            nc.vector.bn_stats(out=stats[:], in_=src_ap)
            nc.vector.bn_aggr(out=mv[:], in_=stats[:])
            nc.scalar.activation(out=rstd[:], in_=mv[:, 1:2],
                                 func=mybir.ActivationFunctionType.Sqrt,
                                 bias=eps_sb[:])
            nc.vector.reciprocal(out=rstd[:], in_=rstd[:])
            nc.vector.tensor_scalar(out=nmur[:], in0=mv[:, 0:1],
                                    scalar1=rstd[:], scalar2=-1.0,
                                    op0=mybir.AluOpType.mult,
                                    op1=mybir.AluOpType.mult)
            return nmur, rstd

        def ln_apply(src_ap, out_ap, nr):
            nc.scalar.activation(out=out_ap, in_=src_ap,
                                 func=mybir.ActivationFunctionType.Identity,
                                 bias=nr[0][:], scale=nr[1][:])

        def layer_norm(src_ap, out_ap):
            ln_apply(src_ap, out_ap, ln_stats(src_ap))

        # Pool/GPSIMD cannot touch PSUM on real HW; PSUM evacuations must go
        # to DVE or Act.  Alternate between them for the transpose drains.
        _tr_rr = [0]

        def transpose_into(dst_ap, src_ap, pst_pool, eng=None):
            """PE-transpose a [P, P] fp16 SBUF block into dst (via PSUM)."""
            pst = pst_pool.tile([P, P], HDT, name="pst")
            nc.tensor.transpose(pst[:], src_ap, ident[:])
            if eng is None:
                _tr_rr[0] += 1
                if _tr_rr[0] % 2 == 0:
                    nc.vector.tensor_copy(out=dst_ap, in_=pst[:])
                else:
                    nc.scalar.copy(out=dst_ap, in_=pst[:])
            else:
                eng.tensor_copy(out=dst_ap, in_=pst[:])

        def set_x_from(j, pst_pool):
            """Write x_h/xT_h for t-tile j from x_f32."""
            nc.vector.tensor_copy(out=x_h[:, j, :], in_=x_f32[:, j, :])
            for k in range(KD):
                transpose_into(xT_h[:, k, j * P:(j + 1) * P],
                               x_h[:, j, k * P:(k + 1) * P], pst_pool)

        # ---- initial x = ln(embed[idx]) (gather done on host into x0) ----
        with tc.tile_pool(name="ps_init", bufs=2, space="PSUM") as ps_init:
            x0stage = arp.tile([P, NT, D], HDT, name="x0stage",
                               tag="ar_stage")
            nc.sync.dma_start(
                x0stage[:], x0_d.ap().rearrange("(j p) d -> p j d", p=P))
            for j in range(NT):
                layer_norm(x0stage[:, j, :], x_f32[:, j, :])
                set_x_from(j, ps_init)
        # weights load behind the init pipeline (enc is needed first, at A0)
        nc.sync.dma_start(
            encw_sb[:], encw_d.ap().rearrange("(k p) n -> p k n", p=P))
        nc.scalar.dma_start(
            encvh_sb[:], encvh_d.ap().rearrange("(k p) n -> p k n", p=P))
        nc.scalar.dma_start(
            encvl_sb[:], encvl_d.ap().rearrange("(k p) n -> p k n", p=P))
        nc.scalar.dma_start(
            decw_sb[:], decw_d.ap().rearrange("(m p) d -> p m d", p=P))
        for k in range(KD):
            nc.scalar.dma_start(lmh_sb[:, k, :],
                                lmh_d.ap()[k * P:(k + 1) * P, :])
        if dbg:
            nc.sync.dma_start(
                dbg_tensors["dbg_x0ln"].ap().rearrange("(j p) d -> p j d", p=P),
                x_f32[:])

        # ---- layers ----
        for layer in range(n_layers):
            # Phase A: x_sparse^T = relu(enc^T x^T)*XSP_SCALE, rope -> qr8.
            # m emitted interleaved so rope pair i fires after its 2 evacs.
            qr8_engs = ([nc.scalar] * qr8_split[0] + [nc.vector] * qr8_split[1]
                        + [nc.gpsimd] * qr8_split[2])
            qr8_engs = [qr8_engs[(7 * z) % len(qr8_engs)]
                        for z in range(len(qr8_engs))]
            with tc.tile_pool(name=f"psA_{layer}", bufs=2,
                              space="PSUM") as psA:
                for mi, m in enumerate(M_ORDER):
                    ps = psA.tile([P, T], F32, name="psA")
                    for c in range(2):
                        for k in range(KD):
                            nc.tensor.matmul(
                                ps[:, c * 512:(c + 1) * 512],
                                lhsT=encw_sb[:, k, m * P:(m + 1) * P],
                                rhs=xT_h[:, k, c * 512:(c + 1) * 512],
                                start=(k == 0), stop=(k == KD - 1))
                    if (mi * xsp_dve) % NM < xsp_dve:
                        # fused relu+scale on DVE: (ps max 0) * XSP_SCALE
                        nc.vector.tensor_scalar(
                            out=x_sp[:, m, :], in0=ps[:],
                            scalar1=0.0, scalar2=float(XSP_SCALE),
                            op0=mybir.AluOpType.max,
                            op1=mybir.AluOpType.mult)
                    else:
                        nc.scalar.activation(
                            out=x_sp[:, m, :], in_=ps[:],
                            func=mybir.ActivationFunctionType.Relu,
                            scale=float(XSP_SCALE))
                    if mi % 2 == 1:
                        i = m - NPAIR  # pair index just completed
                        cst = csp.tile([P, 2, T], HDT, name="cst")
                        dma_eng = nc.sync if i % 2 == 0 else nc.scalar
                        dma_eng.dma_start(
                            cst[:], cs_d.ap().rearrange(
                                "n (two t) -> n two t",
                                two=2)[i * P:(i + 1) * P, :, :])
                        xe = x_sp[:, i, :]
                        xo = x_sp[:, i + NPAIR, :]
                        ctt, stt = cst[:, 0, :], cst[:, 1, :]
                        engs = [nc.vector] * 6
                        for t in range(n_pool_rope):
                            engs[5 - t] = nc.gpsimd
                        t1 = ropep.tile([P, T], HDT, name="rope_t1",
                                        tag="rope_t", bufs=4)
                        t2 = ropep.tile([P, T], HDT, name="rope_t2",
                                        tag="rope_t", bufs=4)
                        qe = ropep.tile([P, T], HDT, name="rope_qe",
                                        tag="rope_q")
                        engs[0].tensor_mul(t1[:], xe, ctt)
                        engs[1].tensor_mul(t2[:], xo, stt)
                        engs[2].tensor_sub(qe[:], t1[:], t2[:])
                        t3 = ropep.tile([P, T], HDT, name="rope_t3",
                                        tag="rope_t", bufs=4)
                        t4 = ropep.tile([P, T], HDT, name="rope_t4",
                                        tag="rope_t", bufs=4)
                        qo = ropep.tile([P, T], HDT, name="rope_qo",
                                        tag="rope_q")
                        engs[3].tensor_mul(t3[:], xo, ctt)
                        engs[4].tensor_mul(t4[:], xe, stt)
                        engs[5].tensor_add(qo[:], t3[:], t4[:])
                        for src, dst_m, e in (
                                (qe, i, qr8_engs[2 * i]),
                                (qo, i + NPAIR, qr8_engs[2 * i + 1])):
                            if e is nc.scalar:
                                e.copy(out=qr8[:, dst_m, :], in_=src[:])
                            else:
                                e.tensor_copy(out=qr8[:, dst_m, :],
                                              in_=src[:])

            if dbg and layer == 0:
                nc.sync.dma_start(
                    dbg_tensors["dbg_xsp"].ap().rearrange(
                        "(m p) t -> p m t", p=P), x_sp[:])
                for m in range(NM):
                    qd = lnp.tile([P, T], F32, name="qr_dbg", tag="qr_dbg")
                    nc.vector.tensor_copy(out=qd[:], in_=qr8[:, m, :])
                    nc.sync.dma_start(
                        dbg_tensors["dbg_qr"].ap().rearrange(
                            "(m p) t -> p m t", p=P)[:, m, :], qd[:])

            # Phase B: S partial (fp8 DoubleRow) + causal mask + ykv partial.
            # One PSUM pool pair across both c passes: S chunks of the second
            # half start while the rope still streams (4 rotating S banks,
            # ykv banks handed from c=0 to c=1 by tag rotation).
            ykv_pre = arp.tile([P, NT, D], HDT, name="ykv_pre",
                               tag="ar_stage")
            with tc.tile_pool(name=f"psS_{layer}", bufs=ps_s_bufs,
                              space="PSUM") as psS, \
                 tc.tile_pool(name=f"psY_{layer}", bufs=1,
                              space="PSUM") as psY:
                for c in range(2):
                    ykv_ps = [psY.tile([P, D], F32, name=f"ykv_ps{j}",
                                       tag=f"ykv_ps{j % 4}")
                              for j in range(4 * c, 4 * c + 4)]
                    for i in range(4 * c + 4):
                        # causal tiling: only columns t >= i*P are needed
                        base = max(c * 512, i * P)
                        width = (c + 1) * 512 - base
                        ps = psS.tile([P, 512], F32, name="psS")
                        for ku, u in enumerate(K_ORDER):
                            nc.tensor.matmul(
                                ps[:, :width],
                                lhsT=qr8[:, 2 * u:2 * u + 2,
                                         i * P:(i + 1) * P],
                                rhs=qr8[:, 2 * u:2 * u + 2,
                                        base:base + width],
                                start=(ku == 0), stop=(ku == NPAIR - 1),
                                perf_mode=DR)
                        sc = schp.tile([P, 512], HDT, name="schunk")
                        if sc_pool:
                            nc.gpsimd.tensor_scalar_mul(
                                sc[:, :width], ps[:, :width], float(SC_SCALE))
                        else:
                            nc.scalar.mul(out=sc[:, :width],
                                          in_=ps[:, :width],
                                          mul=float(SC_SCALE))
                        sd = None
                        if c == i // 4:
                            dcol = i * P - base
                            sd = sdp.tile([P, P], HDT, name="sdiag")
                            (nc.gpsimd if sd_pool else nc.vector).tensor_mul(
                                sd[:], sc[:, dcol:dcol + P], umask_sb[:])
                        for j in range(max(4 * c, i), 4 * c + 4):
                            lhsT = sd[:] if j == i else \
                                sc[:, j * P - base:(j + 1) * P - base]
                            nc.tensor.matmul(
                                ykv_ps[j - 4 * c][:], lhsT=lhsT,
                                rhs=x_h[:, i, :],
                                start=(i == 0), stop=(i == j))
                    for j in range(4 * c, 4 * c + 4):
                        nc.scalar.mul(out=ykv_pre[:, j, :],
                                      in_=ykv_ps[j - 4 * c][:],
                                      mul=float(YKV_SCALE))

            if dbg and layer == 0:
                nc.sync.dma_start(
                    dbg_tensors["dbg_ykvpre"].ap().rearrange(
                        "(j p) d -> p j d", p=P), ykv_pre[:])

            # Phase C: pair AllReduce of ykv, layernorm, transpose.
            # Chunked staging DMAs + op-major (fissioned) LN pipeline.
            ar_in = dram.tile([T, D], HDT, name=f"arin_{layer}",
                              tag=f"arin_{layer}")
            ar_out = dram.tile([T, D], HDT, name=f"arout_{layer}",
                               tag=f"arout_{layer}")
            arin_p = ar_in.rearrange("(j p) d -> p j d", p=P)
            for jc in range(4):
                nc.sync.dma_start(arin_p[:, 2 * jc:2 * jc + 2, :],
                                  ykv_pre[:, 2 * jc:2 * jc + 2, :])
            emit_allreduce(nc, PAIR_GROUPS, [ar_in.opt()], [ar_out.opt()])
            ykv_post = arp.tile([P, NT, D], HDT, name="ykv_post",
                                tag="ar_stage")
            arout_p = ar_out.rearrange("(j p) d -> p j d", p=P)
            for jc in range(4):
                nc.sync.dma_start(ykv_post[:, 2 * jc:2 * jc + 2, :],
                                  arout_p[:, 2 * jc:2 * jc + 2, :])
            with tc.tile_pool(name=f"psT_{layer}", bufs=4,
                              space="PSUM") as psT:
                for jh in range(2):
                    jr = list(range(4 * jh, 4 * jh + 4))
                    nrs = [ln_stats(ykv_post[:, j, :]) for j in jr]
                    yls = []
                    for idx, j in enumerate(jr):
                        yl = lnp.tile([P, D], HDT, name="ykv_ln",
                                      tag="ln_f16", bufs=4)
                        ln_apply(ykv_post[:, j, :], yl[:], nrs[idx])
                        yls.append(yl)
                    for idx, j in enumerate(jr):
                        for k in range(KD):
                            pst = psT.tile([P, P], HDT, name="pst")
                            nc.tensor.transpose(
                                pst[:], yls[idx][:, k * P:(k + 1) * P],
                                ident[:])
                            dst = slice(j * P, (j + 1) * P)
                            nc.scalar.copy(out=yh8_sb[:, k, dst],
                                           in_=pst[:])
                            nc.vector.tensor_sub(yl1_sb[:, k, dst],
                                                 pst[:], yh8_sb[:, k, dst])

            if dbg and layer == 0:
                nc.sync.dma_start(
                    dbg_tensors["dbg_ykvpost"].ap().rearrange(
                        "(j p) d -> p j d", p=P), ykv_post[:])


            # Phase D: y_sp = relu(encv^T ykv_ln^T); xy = x_sp*y_sp;
            # ymlp^T accumulated with dec tiles as lhsT.  c-outer so the
            # first T-half starts as soon as ykvT columns 0..511 exist.
            ymlpT_pre = arp.tile([P, KD, T], HDT, name="ymlpT_pre",
                                 tag="ar_stage")
            ar2_in = dram.tile([D, T], HDT, name=f"ar2in_{layer}",
                               tag=f"ar2in_{layer}")
            ar2_out = dram.tile([D, T], HDT, name=f"ar2out_{layer}",
                                tag=f"ar2out_{layer}", addr_space="Shared")
            ar2in_p = ar2_in.rearrange("(k p) t -> p k t", p=P)
            with tc.tile_pool(name=f"psD_{layer}", bufs=4,
                              space="PSUM") as psD, \
                 tc.tile_pool(name=f"psM_{layer}", bufs=1,
                              space="PSUM") as psM:
                ymlpT_ps = [psM.tile([P, T], F32, name=f"ymlpT_ps{k}",
                                     tag=f"ymlpT_ps{k}") for k in range(KD)]
                for c in range(2):
                    cs = slice(c * 512, (c + 1) * 512)
                    for m in range(NM):
                        ps = psD.tile([P, 512], F32, name="psD")
                        msl = slice(m * P, (m + 1) * P)
                        terms = ((encvh_sb, yh8_sb), (encvh_sb, yl1_sb),
                                 (encvl_sb, yh8_sb))
                        for ti, (wsb, ysb) in enumerate(terms):
                            nc.tensor.matmul(
                                ps[:], lhsT=wsb[:, 0:2, msl],
                                rhs=ysb[:, 0:2, cs],
                                start=(ti == 0), stop=(ti == 2),
                                perf_mode=DR)
                        ysp = yxp.tile([P, 512], HDT, name="ysp")
                        if (m * ysp_dve) % NM < ysp_dve:
                            # relu + 1/128 unscale fused on DVE
                            nc.vector.tensor_scalar(
                                out=ysp[:], in0=ps[:],
                                scalar1=0.0, scalar2=1.0 / 128.0,
                                op0=mybir.AluOpType.max,
                                op1=mybir.AluOpType.mult)
                        else:
                            nc.scalar.activation(
                                out=ysp[:], in_=ps[:],
                                func=mybir.ActivationFunctionType.Relu,
                                scale=1.0 / 128.0)
                        xy = yxp.tile([P, 512], HDT, name="xy")
                        nc.vector.tensor_mul(xy[:], x_sp[:, m, cs], ysp[:])
                        for k in range(KD):
                            nc.tensor.matmul(
                                ymlpT_ps[k][:, cs],
                                lhsT=decw_sb[:, m, k * P:(k + 1) * P],
                                rhs=xy[:],
                                start=(m == 0), stop=(m == NM - 1))
                    for k in range(KD):
                        nc.scalar.copy(out=ymlpT_pre[:, k, cs],
                                       in_=ymlpT_ps[k][:, cs])
                        # upload this quarter while the next half computes
                        nc.sync.dma_start(ar2in_p[:, k, cs],
                                          ymlpT_pre[:, k, cs])

            if dbg and layer == 0:
                nc.sync.dma_start(
                    dbg_tensors["dbg_ymlppre"].ap().rearrange(
                        "(k p) t -> p k t", p=P), ymlpT_pre[:])

            # Phase E: 8-way AllReduce of ymlp^T; x = ln(x + ln(ymlp)).
            # Fissioned: transposes first, then the two LN chains op-major.
            emit_allreduce(nc, ALL_GROUP, [ar2_in.opt()], [ar2_out.opt()])
            ymlpT_post = arp.tile([P, KD, T], HDT, name="ymlpT_post",
                                  tag="ar_stage")
            ar2out_p = ar2_out.rearrange("(k p) t -> p k t", p=P)
            for kc in range(KD):
                nc.sync.dma_start(ymlpT_post[:, kc, :], ar2out_p[:, kc, :])
            if dbg and layer == 0:
                nc.sync.dma_start(
                    dbg_tensors["dbg_ymlppost"].ap().rearrange(
                        "(k p) t -> p k t", p=P), ymlpT_post[:])
            with tc.tile_pool(name=f"psE_{layer}", bufs=6,
                              space="PSUM") as psE:
                for jh in range(2):
                    jr = list(range(4 * jh, 4 * jh + 4))
                    ymts = {}
                    for j in jr:
                        ymt = lnp.tile([P, D], HDT, name="ymt",
                                       tag="ln_f16", bufs=4)
                        for k in range(KD):
                            transpose_into(
                                ymt[:, k * P:(k + 1) * P],
                                ymlpT_post[:, k, j * P:(j + 1) * P], psE)
                        ymts[j] = ymt
                    nrs = {j: ln_stats(ymts[j][:]) for j in jr}
                    us = {}
                    for j in jr:
                        u = lnp.tile([P, D], F32, name="u_ln",
                                     tag="ln_f32", bufs=3)
                        ln_apply(ymts[j][:], u[:], nrs[j])
                        us[j] = u
                    xns = {}
                    for j in jr:
                        xn = lnp.tile([P, D], F32, name="xn",
                                      tag="ln_f32x", bufs=3)
                        nc.vector.tensor_add(xn[:], x_f32[:, j, :], us[j][:])
                        xns[j] = xn
                    nrs2 = {j: ln_stats(xns[j][:]) for j in jr}
                    for j in jr:
                        ln_apply(xns[j][:], x_f32[:, j, :], nrs2[j])
                    for j in jr:
                        set_x_from(j, psE)
            if dbg and layer == 0:
                nc.sync.dma_start(
                    dbg_tensors["dbg_x1"].ap().rearrange(
                        "(j p) d -> p j d", p=P), x_f32[:])

        # ---- logits = x @ lm_head ----
        with tc.tile_pool(name="psL", bufs=2, space="PSUM") as psL:
            for j in range(NT):
                ps = psL.tile([P, VOCAB], F32, name="psLt")
                for k in range(KD):
                    nc.tensor.matmul(ps[:],
                                     lhsT=xT_h[:, k, j * P:(j + 1) * P],
                                     rhs=lmh_sb[:, k, :],
                                     start=(k == 0), stop=(k == KD - 1))
                lg = lnp.tile([P, VOCAB], F32, name="lgt", tag="ln_f32",
                              bufs=3)
                nc.scalar.copy(out=lg[:], in_=ps[:])
                nc.sync.dma_start(logits_d.ap()[j * P:(j + 1) * P, :], lg[:])

        for _pool in (statp, lnp, arp, yxp, sdp, schp, ropep, csp,
                      dram, persist):
            _pool.release()

    nc.compile()
    return nc


def _host_inputs(idx, embed, encoder, encoder_v, decoder, lm_head):
    """Build the 8 per-core input maps (host-side sharding)."""
    import ml_dtypes
    f8e4 = ml_dtypes.float8_e4m3fn
    f16 = np.float16
    idx = np.asarray(idx).reshape(-1).astype(np.int64)
    embed = np.asarray(embed, np.float32)
    enc = np.asarray(encoder, np.float32)
    encv = np.asarray(encoder_v, np.float32)
    dec = np.asarray(decoder, np.float32)
    lmh = np.asarray(lm_head, np.float32)

    x0 = embed[idx]  # [T, D] gather on host (pure indexing)

    # freqs exactly as the reference computes them (fp32)
    t = np.arange(0, N, dtype=np.float32)
    q = np.floor(t / 2.0) * 2.0
    freqs = (1.0 / ((2.0 ** 16) ** (q / N)) / TWO_PI).astype(np.float32)
    tvec = np.arange(T, dtype=np.float32)

    umask = (np.arange(P)[:, None] < np.arange(P)[None, :]).astype(f16)

    in_maps = []
    for d in range(N_CORES):
        h, half = d // 2, d % 2
        perm = np.concatenate([np.arange(0, NLOC, 2),
                               np.arange(1, NLOC, 2)]) + half * NLOC
        encv128 = (encv[h][:, perm] * 128.0).astype(np.float32)
        encvh8 = encv128.astype(f8e4)
        encvl8 = (encv128 - encvh8.astype(np.float32)).astype(f8e4)
        encvh8 = np.ascontiguousarray(encvh8)
        encvl8 = np.ascontiguousarray(encvl8)
        f_loc = freqs[perm[:NLOC // 2]]
        ph = (tvec[None, :] * f_loc[:, None]).astype(np.float32) % 1.0
        cs = np.concatenate([np.cos(TWO_PI * ph), np.sin(TWO_PI * ph)],
                            axis=1)  # [NLOC//2, 2T]
        in_maps.append({
            "x0": np.ascontiguousarray(x0, f16),
            "encw": np.ascontiguousarray(enc[h][:, perm], f16),
            "encvh": encvh8,
            "encvl": encvl8,
            "decw": np.ascontiguousarray(dec[h * N + perm, :], f16),
            "cs": np.ascontiguousarray(cs, f16),
            "lmh": np.ascontiguousarray(lmh, f16),
            "umask": umask,
        })
    return in_maps


def kernel(idx, embed, encoder, encoder_v, decoder, lm_head,
           _trace=False, _tmpdir=None):
    if "nc" not in _CACHE:
        _CACHE["nc"] = _build_program()
    nc = _CACHE["nc"]
    in_maps = _host_inputs(idx, embed, encoder, encoder_v, decoder, lm_head)
    res = bass_utils.run_bass_kernel_spmd(
        nc, in_maps, core_ids=list(range(N_CORES)),
        trace=_trace, tmpdir=_tmpdir)
    _CACHE["last_results"] = res
    logits = res.results[0]["logits"].astype(np.float32).reshape(B, T, VOCAB)
    return logits


# revision 5
# speedup vs baseline: 1.3553x; 1.0067x over previous
"""Trainium2 Bass kernel for nn_BDH_1726576853700 (sparse_attention), v3.

3-layer sparse-attention net: B=1, T=1024, D=256, NH=4, N=8192, VOCAB=256.

Sharding over 8 NeuronCores: device d -> (head h=d//2, half=d%2) — each device
owns a 4096-wide slice of one head's sparse latent dim, permuted evens-first so
the RoPE pair partner is tile i+16.

v3 design:
  - S = qr^T qr runs in fp8e4 DoubleRow (2 k-tiles per matmul, 0.5 cycles/row).
    qr is quantized to fp8 at scale 32 (folded into the x_sparse relu evac);
    the scale washes out in the downstream LayerNorm.
  - encv projection runs as three fp8 DoubleRow terms at a common scale:
    vh^T yh + vh^T yl + vl^T yh, where vh=fp8(128 encv), vl=fp8(128 encv-vh)
    (host side) and yh=fp8(ykv_ln^T), yl=fp8(ykv_ln^T-yh) (device side,
    extracted during the transpose drain).  Residuals live in fp8 subnormals;
    they carry ~3%% of the magnitude so their quantization noise is ~0.1%%
    of the result.  Validated end-to-end at rel err 2.0e-3.
  - enc/dec stay fp16: every single-fp8 variant of the three projections
    measured over the 2e-2 gate in numpy rehearsal (enc8+x8: 3.8e-2,
    encv8+ykv8: 4.2e-2, dec8+xy8: 4.6e-2); hi/lo for dec needs an xy
    residual extraction that costs more DVE time than the PE it saves.
  - enc/dec weights resident in SBUF (shared by all 3 layers, loaded once).
  - cos/sin tables stream as ONE combined DMA per rope pair ([P, 2T] tile),
    alternating between the SP and Activation HWDGE queues.
  - Elementwise spread across Act/DVE/Pool with tunable splits; LayerNorm
    normalization on Act (Identity with per-partition scale/bias).
  - Engine streams are in-order: Phase C/E LayerNorm pipelines are emitted
    op-major (loop fission) so independent tiles don't serialize behind
    cross-engine latency chains.
  - Phase A emits m-tiles in (0,16,1,17,...) order and Phase B contracts
    k-pairs in (0,8,1,9,...) order so S matmuls chase the rope pipeline.
  - Phase D runs c-outer (two T-halves) so encv matmuls start on the first
    ykvT tiles right after the ykv AllReduce readback begins.

PSUM discipline: every accumulation group owns its bank(s) exclusively
(zero-region = 2KB = one bank).
"""

import math
import sys

for _p in ("/opt/trn_rl_repo",):
    if _p not in sys.path:
        sys.path.insert(0, _p)

import numpy as np

import concourse.bass as bass
import concourse.mybir as mybir
import concourse.tile as tile
from concourse import bacc, bass_utils
from concourse.masks import make_identity

# ---- problem constants (hardcoded per contract) ----
B, T, D, NH, N = 1, 1024, 256, 4, 8192
VOCAB = 256
N_LAYER = 3
EPS = 1e-5
TWO_PI = 2.0 * math.pi
N_CORES = 8
NLOC = N // 2          # latent columns per device: 4096
P = 128
NT = T // P            # 8 t-tiles
KD = D // P            # 2 d-tiles
NM = NLOC // P         # 32 n-tiles per device
NPAIR = NM // 2        # 16 rope pairs
HDT = mybir.dt.float16
F8 = mybir.dt.float8e4
F32 = mybir.dt.float32
DR = mybir.MatmulPerfMode.DoubleRow

XSP_SCALE = 32.0       # x_sparse stored at 32x so qr lands in fp8 normal range
SC_SCALE = 1.0 / 16.0  # S psum -> sc fp16 evacuation scale
YKV_SCALE = 1.0 / 256.0

_CACHE = {}


def _build_program(dbg=False, use_collectives=True, n_layers=N_LAYER,
                   n_pool_rope=0, qr8_split=(10, 4, 18), xsp_dve=0,
                   ysp_dve=8, sc_pool=False, sd_pool=True, ps_s_bufs=4):
    def emit_allreduce(nc, groups, ins, outs):
        if use_collectives:
            nc.gpsimd.collective_compute(
                "AllReduce", mybir.AluOpType.add, replica_groups=groups,
                ins=ins, outs=outs)
        else:
            nc.sync.dma_start(outs[0], ins[0])

    nc = bacc.Bacc("TRN2", target_bir_lowering=False, debug=False,
                   num_devices=N_CORES)
    dbg_tensors = {}
    if dbg:
        for nm, shape, dt in [
            ("dbg_x0ln", [T, D], F32),
            ("dbg_xsp", [NM * P, T], HDT),
            ("dbg_qr", [NM * P, T], F32),
            ("dbg_ykvpre", [T, D], HDT),
            ("dbg_ykvpost", [T, D], HDT),
            ("dbg_ykvT", [D, T], HDT),
            ("dbg_ymlppre", [D, T], HDT),
            ("dbg_ymlppost", [D, T], HDT),
            ("dbg_x1", [T, D], F32),
        ]:
            dbg_tensors[nm] = nc.dram_tensor(nm, shape, dt,
                                             kind="ExternalOutput")

    x0_d = nc.dram_tensor("x0", [T, D], HDT, kind="ExternalInput")
    encw_d = nc.dram_tensor("encw", [D, NLOC], HDT, kind="ExternalInput")
    encvh_d = nc.dram_tensor("encvh", [D, NLOC], F8, kind="ExternalInput")
    encvl_d = nc.dram_tensor("encvl", [D, NLOC], F8, kind="ExternalInput")
    decw_d = nc.dram_tensor("decw", [NLOC, D], HDT, kind="ExternalInput")
    cs_d = nc.dram_tensor("cs", [NLOC // 2, 2 * T], HDT, kind="ExternalInput")
    lmh_d = nc.dram_tensor("lmh", [D, VOCAB], HDT, kind="ExternalInput")
    umask_d = nc.dram_tensor("umask", [P, P], HDT, kind="ExternalInput")
    logits_d = nc.dram_tensor("logits", [T, VOCAB], F32, kind="ExternalOutput")

    PAIR_GROUPS = [[0, 1], [2, 3], [4, 5], [6, 7]]
    ALL_GROUP = [list(range(N_CORES))]

    # interleaved m emission order: pair halves adjacent (0,16,1,17,...)
    M_ORDER = []
    for i in range(NPAIR):
        M_ORDER += [i, i + NPAIR]
    # S contraction k-pair order: (qe pair u) then (qo pair u): 0,8,1,9,...
    K_ORDER = []
    for u in range(NPAIR // 2):
        K_ORDER += [u, u + NPAIR // 2]

    with tile.TileContext(nc) as tc:
        persist = tc.alloc_tile_pool(name="persist", bufs=1)
        dram = tc.alloc_tile_pool(name="dram", bufs=1, space="DRAM")

        # persistent SBUF state
        encw_sb = persist.tile([P, KD, NLOC], HDT)   # 16KB/part
        encvh_sb = persist.tile([P, KD, NLOC], F8)   # 8KB (128*encv hi)
        encvl_sb = persist.tile([P, KD, NLOC], F8)   # 8KB (residual)
        decw_sb = persist.tile([P, NM, D], HDT)      # 16KB
        x_sp = persist.tile([P, NM, T], HDT)         # 64KB, stored *XSP_SCALE
        qr8 = persist.tile([P, NM, T], F8)           # 32KB
        x_f32 = persist.tile([P, NT, D], F32)        # 8KB residual stream
        x_h = persist.tile([P, NT, D], HDT)          # 4KB
        xT_h = persist.tile([P, KD, T], HDT)         # 4KB
        yh8_sb = persist.tile([P, KD, T], F8)        # 2KB ykv_ln^T hi
        yl1_sb = persist.tile([P, KD, T], F8)        # 2KB ykv_ln^T residual
        lmh_sb = persist.tile([P, KD, VOCAB], HDT)
        umask_sb = persist.tile([P, P], HDT)
        ident = persist.tile([P, P], HDT)
        eps_sb = persist.tile([P, 1], F32)

        nc.vector.memset(eps_sb[:], float(EPS))
        nc.sync.dma_start(umask_sb[:], umask_d.ap())
        make_identity(nc, ident[:])

        # streaming / working pools
        csp = tc.alloc_tile_pool(name="csp", bufs=2)
        ropep = tc.alloc_tile_pool(name="ropep", bufs=2)
        schp = tc.alloc_tile_pool(name="schp", bufs=2)
        sdp = tc.alloc_tile_pool(name="sdp", bufs=2)
        yxp = tc.alloc_tile_pool(name="yxp", bufs=2)
        arp = tc.alloc_tile_pool(name="arp", bufs=1)
        lnp = tc.alloc_tile_pool(name="lnp", bufs=2)
        statp = tc.alloc_tile_pool(name="statp", bufs=8)

        def ln_stats(src_ap):
            """Emit stats chain ops; returns (nmur, rstd) [P,1] tiles."""
            stats = statp.tile([P, 6], F32, name="ln_stats")
            mv = statp.tile([P, 2], F32, name="ln_mv")
            rstd = statp.tile([P, 1], F32, name="ln_rstd")
            nmur = statp.tile([P, 1], F32, name="ln_nmur")
            nc.vector.bn_stats(out=stats[:], in_=src_ap)
            nc.vector.bn_aggr(out=mv[:], in_=stats[:])
            nc.scalar.activation(out=rstd[:], in_=mv[:, 1:2],
                                 func=mybir.ActivationFunctionType.Sqrt,
                                 bias=eps_sb[:])
            nc.vector.reciprocal(out=rstd[:], in_=rstd[:])
            nc.vector.tensor_scalar(out=nmur[:], in0=mv[:, 0:1],
                                    scalar1=rstd[:], scalar2=-1.0,
                                    op0=mybir.AluOpType.mult,
                                    op1=mybir.AluOpType.mult)
            return nmur, rstd

        def ln_apply(src_ap, out_ap, nr):
            nc.scalar.activation(out=out_ap, in_=src_ap,
                                 func=mybir.ActivationFunctionType.Identity,
                                 bias=nr[0][:], scale=nr[1][:])

        def layer_norm(src_ap, out_ap):
            ln_apply(src_ap, out_ap, ln_stats(src_ap))

        # Pool/GPSIMD cannot touch PSUM on real HW; PSUM evacuations must go
        # to DVE or Act.  Alternate between them for the transpose drains.
        _tr_rr = [0]

        def transpose_into(dst_ap, src_ap, pst_pool, eng=None):
            """PE-transpose a [P, P] fp16 SBUF block into dst (via PSUM)."""
            pst = pst_pool.tile([P, P], HDT, name="pst")
            nc.tensor.transpose(pst[:], src_ap, ident[:])
            if eng is None:
                _tr_rr[0] += 1
                if _tr_rr[0] % 2 == 0:
                    nc.vector.tensor_copy(out=dst_ap, in_=pst[:])
                else:
                    nc.scalar.copy(out=dst_ap, in_=pst[:])
            else:
                eng.tensor_copy(out=dst_ap, in_=pst[:])

        def set_x_from(j, pst_pool):
            """Write x_h/xT_h for t-tile j from x_f32."""
            nc.vector.tensor_copy(out=x_h[:, j, :], in_=x_f32[:, j, :])
            for k in range(KD):
                transpose_into(xT_h[:, k, j * P:(j + 1) * P],
                               x_h[:, j, k * P:(k + 1) * P], pst_pool)

        # ---- initial x = ln(embed[idx]) (gather done on host into x0) ----
        with tc.tile_pool(name="ps_init", bufs=2, space="PSUM") as ps_init:
            x0stage = arp.tile([P, NT, D], HDT, name="x0stage",
                               tag="ar_stage")
            nc.sync.dma_start(
                x0stage[:], x0_d.ap().rearrange("(j p) d -> p j d", p=P))
            for j in range(NT):
                layer_norm(x0stage[:, j, :], x_f32[:, j, :])
                set_x_from(j, ps_init)
        # weights load behind the init pipeline (enc is needed first, at A0)
        nc.sync.dma_start(
            encw_sb[:], encw_d.ap().rearrange("(k p) n -> p k n", p=P))
        nc.scalar.dma_start(
            encvh_sb[:], encvh_d.ap().rearrange("(k p) n -> p k n", p=P))
        nc.scalar.dma_start(
            encvl_sb[:], encvl_d.ap().rearrange("(k p) n -> p k n", p=P))
        nc.scalar.dma_start(
            decw_sb[:], decw_d.ap().rearrange("(m p) d -> p m d", p=P))
        for k in range(KD):
            nc.scalar.dma_start(lmh_sb[:, k, :],
                                lmh_d.ap()[k * P:(k + 1) * P, :])
        if dbg:
            nc.sync.dma_start(
                dbg_tensors["dbg_x0ln"].ap().rearrange("(j p) d -> p j d", p=P),
                x_f32[:])

        # ---- layers ----
        for layer in range(n_layers):
            # Phase A: x_sparse^T = relu(enc^T x^T)*XSP_SCALE, rope -> qr8.
            # m emitted interleaved so rope pair i fires after its 2 evacs.
            qr8_engs = ([nc.scalar] * qr8_split[0] + [nc.vector] * qr8_split[1]
                        + [nc.gpsimd] * qr8_split[2])
            qr8_engs = [qr8_engs[(7 * z) % len(qr8_engs)]
                        for z in range(len(qr8_engs))]
            with tc.tile_pool(name=f"psA_{layer}", bufs=2,
                              space="PSUM") as psA:
                for mi, m in enumerate(M_ORDER):
                    ps = psA.tile([P, T], F32, name="psA")
                    for c in range(2):
                        for k in range(KD):
                            nc.tensor.matmul(
                                ps[:, c * 512:(c + 1) * 512],
                                lhsT=encw_sb[:, k, m * P:(m + 1) * P],
                                rhs=xT_h[:, k, c * 512:(c + 1) * 512],
                                start=(k == 0), stop=(k == KD - 1))
                    if (mi * xsp_dve) % NM < xsp_dve:
                        # fused relu+scale on DVE: (ps max 0) * XSP_SCALE
                        nc.vector.tensor_scalar(
                            out=x_sp[:, m, :], in0=ps[:],
                            scalar1=0.0, scalar2=float(XSP_SCALE),
                            op0=mybir.AluOpType.max,
                            op1=mybir.AluOpType.mult)
                    else:
                        nc.scalar.activation(
                            out=x_sp[:, m, :], in_=ps[:],
                            func=mybir.ActivationFunctionType.Relu,
                            scale=float(XSP_SCALE))
                    if mi % 2 == 1:
                        i = m - NPAIR  # pair index just completed
                        cst = csp.tile([P, 2, T], HDT, name="cst")
                        dma_eng = nc.sync if i % 2 == 0 else nc.scalar
                        dma_eng.dma_start(
                            cst[:], cs_d.ap().rearrange(
                                "n (two t) -> n two t",
                                two=2)[i * P:(i + 1) * P, :, :])
                        xe = x_sp[:, i, :]
                        xo = x_sp[:, i + NPAIR, :]
                        ctt, stt = cst[:, 0, :], cst[:, 1, :]
                        engs = [nc.vector] * 6
                        for t in range(n_pool_rope):
                            engs[5 - t] = nc.gpsimd
                        t1 = ropep.tile([P, T], HDT, name="rope_t1",
                                        tag="rope_t", bufs=4)
                        t2 = ropep.tile([P, T], HDT, name="rope_t2",
                                        tag="rope_t", bufs=4)
                        qe = ropep.tile([P, T], HDT, name="rope_qe",
                                        tag="rope_q")
                        engs[0].tensor_mul(t1[:], xe, ctt)
                        engs[1].tensor_mul(t2[:], xo, stt)
                        engs[2].tensor_sub(qe[:], t1[:], t2[:])
                        t3 = ropep.tile([P, T], HDT, name="rope_t3",
                                        tag="rope_t", bufs=4)
                        t4 = ropep.tile([P, T], HDT, name="rope_t4",
                                        tag="rope_t", bufs=4)
                        qo = ropep.tile([P, T], HDT, name="rope_qo",
                                        tag="rope_q")
                        engs[3].tensor_mul(t3[:], xo, ctt)
                        engs[4].tensor_mul(t4[:], xe, stt)
                        engs[5].tensor_add(qo[:], t3[:], t4[:])
                        for src, dst_m, e in (
                                (qe, i, qr8_engs[2 * i]),
                                (qo, i + NPAIR, qr8_engs[2 * i + 1])):
                            if e is nc.scalar:
                                e.copy(out=qr8[:, dst_m, :], in_=src[:])
                            else:
                                e.tensor_copy(out=qr8[:, dst_m, :],
                                              in_=src[:])

            if dbg and layer == 0:
                nc.sync.dma_start(
                    dbg_tensors["dbg_xsp"].ap().rearrange(
                        "(m p) t -> p m t", p=P), x_sp[:])
                for m in range(NM):
                    qd = lnp.tile([P, T], F32, name="qr_dbg", tag="qr_dbg")
                    nc.vector.tensor_copy(out=qd[:], in_=qr8[:, m, :])
                    nc.sync.dma_start(
                        dbg_tensors["dbg_qr"].ap().rearrange(
                            "(m p) t -> p m t", p=P)[:, m, :], qd[:])

            # Phase B: S partial (fp8 DoubleRow) + causal mask + ykv partial.
            # One PSUM pool pair across both c passes: S chunks of the second
            # half start while the rope still streams (4 rotating S banks,
            # ykv banks handed from c=0 to c=1 by tag rotation).
            ykv_pre = arp.tile([P, NT, D], HDT, name="ykv_pre",
                               tag="ar_stage")
            with tc.tile_pool(name=f"psS_{layer}", bufs=ps_s_bufs,
                              space="PSUM") as psS, \
                 tc.tile_pool(name=f"psY_{layer}", bufs=1,
                              space="PSUM") as psY:
                for c in range(2):
                    ykv_ps = [psY.tile([P, D], F32, name=f"ykv_ps{j}",
                                       tag=f"ykv_ps{j % 4}")
                              for j in range(4 * c, 4 * c + 4)]
                    for i in range(4 * c + 4):
                        # causal tiling: only columns t >= i*P are needed
                        base = max(c * 512, i * P)
                        width = (c + 1) * 512 - base
                        ps = psS.tile([P, 512], F32, name="psS")
                        for ku, u in enumerate(K_ORDER):
                            nc.tensor.matmul(
                                ps[:, :width],
                                lhsT=qr8[:, 2 * u:2 * u + 2,
                                         i * P:(i + 1) * P],
                                rhs=qr8[:, 2 * u:2 * u + 2,
                                        base:base + width],
                                start=(ku == 0), stop=(ku == NPAIR - 1),
                                perf_mode=DR)
                        sc = schp.tile([P, 512], HDT, name="schunk")
                        if sc_pool:
                            nc.gpsimd.tensor_scalar_mul(
                                sc[:, :width], ps[:, :width], float(SC_SCALE))
                        else:
                            nc.scalar.mul(out=sc[:, :width],
                                          in_=ps[:, :width],
                                          mul=float(SC_SCALE))
                        sd = None
                        if c == i // 4:
                            dcol = i * P - base
                            sd = sdp.tile([P, P], HDT, name="sdiag")
                            (nc.gpsimd if sd_pool else nc.vector).tensor_mul(
                                sd[:], sc[:, dcol:dcol + P], umask_sb[:])
                        for j in range(max(4 * c, i), 4 * c + 4):
                            lhsT = sd[:] if j == i else \
                                sc[:, j * P - base:(j + 1) * P - base]
                            nc.tensor.matmul(
                                ykv_ps[j - 4 * c][:], lhsT=lhsT,
                                rhs=x_h[:, i, :],
                                start=(i == 0), stop=(i == j))
                        if i >= 4 * c:
                            # group j=i just hit its stop: evacuate now so
                            # its bank recycles (3-bank psY rotation)
                            nc.scalar.mul(out=ykv_pre[:, i, :],
                                          in_=ykv_ps[i - 4 * c][:],
                                          mul=float(YKV_SCALE))

            if dbg and layer == 0:
                nc.sync.dma_start(
                    dbg_tensors["dbg_ykvpre"].ap().rearrange(
                        "(j p) d -> p j d", p=P), ykv_pre[:])

            # Phase C: pair AllReduce of ykv, layernorm, transpose.
            # Chunked staging DMAs + op-major (fissioned) LN pipeline.
            ar_in = dram.tile([T, D], HDT, name=f"arin_{layer}",
                              tag=f"arin_{layer}")
            ar_out = dram.tile([T, D], HDT, name=f"arout_{layer}",
                               tag=f"arout_{layer}")
            arin_p = ar_in.rearrange("(j p) d -> p j d", p=P)
            for jc in range(4):
                nc.sync.dma_start(arin_p[:, 2 * jc:2 * jc + 2, :],
                                  ykv_pre[:, 2 * jc:2 * jc + 2, :])
            emit_allreduce(nc, PAIR_GROUPS, [ar_in.opt()], [ar_out.opt()])
            ykv_post = arp.tile([P, NT, D], HDT, name="ykv_post",
                                tag="ar_stage")
            arout_p = ar_out.rearrange("(j p) d -> p j d", p=P)
            for jc in range(4):
                nc.sync.dma_start(ykv_post[:, 2 * jc:2 * jc + 2, :],
                                  arout_p[:, 2 * jc:2 * jc + 2, :])
            with tc.tile_pool(name=f"psT_{layer}", bufs=4,
                              space="PSUM") as psT:
                for jh in range(2):
                    jr = list(range(4 * jh, 4 * jh + 4))
                    nrs = [ln_stats(ykv_post[:, j, :]) for j in jr]
                    yls = []
                    for idx, j in enumerate(jr):
                        yl = lnp.tile([P, D], HDT, name="ykv_ln",
                                      tag="ln_f16", bufs=4)
                        ln_apply(ykv_post[:, j, :], yl[:], nrs[idx])
                        yls.append(yl)
                    for idx, j in enumerate(jr):
                        for k in range(KD):
                            pst = psT.tile([P, P], HDT, name="pst")
                            nc.tensor.transpose(
                                pst[:], yls[idx][:, k * P:(k + 1) * P],
                                ident[:])
                            dst = slice(j * P, (j + 1) * P)
                            nc.scalar.copy(out=yh8_sb[:, k, dst],
                                           in_=pst[:])
                            nc.vector.tensor_sub(yl1_sb[:, k, dst],
                                                 pst[:], yh8_sb[:, k, dst])

            if dbg and layer == 0:
                nc.sync.dma_start(
                    dbg_tensors["dbg_ykvpost"].ap().rearrange(
                        "(j p) d -> p j d", p=P), ykv_post[:])


            # Phase D: y_sp = relu(encv^T ykv_ln^T); xy = x_sp*y_sp;
            # ymlp^T accumulated with dec tiles as lhsT.  c-outer so the
            # first T-half starts as soon as ykvT columns 0..511 exist.
            ymlpT_pre = arp.tile([P, KD, T], HDT, name="ymlpT_pre",
                                 tag="ar_stage")
            ar2_in = dram.tile([D, T], HDT, name=f"ar2in_{layer}",
                               tag=f"ar2in_{layer}")
            ar2_out = dram.tile([D, T], HDT, name=f"ar2out_{layer}",
                                tag=f"ar2out_{layer}", addr_space="Shared")
            ar2in_p = ar2_in.rearrange("(k p) t -> p k t", p=P)
            with tc.tile_pool(name=f"psD_{layer}", bufs=4,
                              space="PSUM") as psD, \
                 tc.tile_pool(name=f"psM_{layer}", bufs=1,
                              space="PSUM") as psM:
                ymlpT_ps = [psM.tile([P, T], F32, name=f"ymlpT_ps{k}",
                                     tag=f"ymlpT_ps{k}") for k in range(KD)]
                for c in range(2):
                    cs = slice(c * 512, (c + 1) * 512)
                    for m in range(NM):
                        ps = psD.tile([P, 512], F32, name="psD")
                        msl = slice(m * P, (m + 1) * P)
                        terms = ((encvh_sb, yh8_sb), (encvh_sb, yl1_sb),
                                 (encvl_sb, yh8_sb))
                        for ti, (wsb, ysb) in enumerate(terms):
                            nc.tensor.matmul(
                                ps[:], lhsT=wsb[:, 0:2, msl],
                                rhs=ysb[:, 0:2, cs],
                                start=(ti == 0), stop=(ti == 2),
                                perf_mode=DR)
                        ysp = yxp.tile([P, 512], HDT, name="ysp")
                        if (m * ysp_dve) % NM < ysp_dve:
                            # relu + 1/128 unscale fused on DVE
                            nc.vector.tensor_scalar(
                                out=ysp[:], in0=ps[:],
                                scalar1=0.0, scalar2=1.0 / 128.0,
                                op0=mybir.AluOpType.max,
                                op1=mybir.AluOpType.mult)
                        else:
                            nc.scalar.activation(
                                out=ysp[:], in_=ps[:],
                                func=mybir.ActivationFunctionType.Relu,
                                scale=1.0 / 128.0)
                        xy = yxp.tile([P, 512], HDT, name="xy")
                        nc.vector.tensor_mul(xy[:], x_sp[:, m, cs], ysp[:])
                        for k in range(KD):
                            nc.tensor.matmul(
                                ymlpT_ps[k][:, cs],
                                lhsT=decw_sb[:, m, k * P:(k + 1) * P],
                                rhs=xy[:],
                                start=(m == 0), stop=(m == NM - 1))
                    for k in range(KD):
                        nc.scalar.copy(out=ymlpT_pre[:, k, cs],
                                       in_=ymlpT_ps[k][:, cs])
                        # upload this quarter while the next half computes
                        nc.sync.dma_start(ar2in_p[:, k, cs],
                                          ymlpT_pre[:, k, cs])

            if dbg and layer == 0:
                nc.sync.dma_start(
                    dbg_tensors["dbg_ymlppre"].ap().rearrange(
                        "(k p) t -> p k t", p=P), ymlpT_pre[:])

            # Phase E: 8-way AllReduce of ymlp^T; x = ln(x + ln(ymlp)).
            # Fissioned: transposes first, then the two LN chains op-major.
            emit_allreduce(nc, ALL_GROUP, [ar2_in.opt()], [ar2_out.opt()])
            ymlpT_post = arp.tile([P, KD, T], HDT, name="ymlpT_post",
                                  tag="ar_stage")
            ar2out_p = ar2_out.rearrange("(k p) t -> p k t", p=P)
            for kc in range(KD):
                nc.sync.dma_start(ymlpT_post[:, kc, :], ar2out_p[:, kc, :])
            if dbg and layer == 0:
                nc.sync.dma_start(
                    dbg_tensors["dbg_ymlppost"].ap().rearrange(
                        "(k p) t -> p k t", p=P), ymlpT_post[:])
            with tc.tile_pool(name=f"psE_{layer}", bufs=6,
                              space="PSUM") as psE:
                for jh in range(2):
                    jr = list(range(4 * jh, 4 * jh + 4))
                    ymts = {}
                    for j in jr:
                        ymt = lnp.tile([P, D], HDT, name="ymt",
                                       tag="ln_f16", bufs=4)
                        for k in range(KD):
                            transpose_into(
                                ymt[:, k * P:(k + 1) * P],
                                ymlpT_post[:, k, j * P:(j + 1) * P], psE)
                        ymts[j] = ymt
                    nrs = {j: ln_stats(ymts[j][:]) for j in jr}
                    us = {}
                    for j in jr:
                        u = lnp.tile([P, D], F32, name="u_ln",
                                     tag="ln_f32", bufs=3)
                        ln_apply(ymts[j][:], u[:], nrs[j])
                        us[j] = u
                    xns = {}
                    for j in jr:
                        xn = lnp.tile([P, D], F32, name="xn",
                                      tag="ln_f32x", bufs=3)
                        nc.vector.tensor_add(xn[:], x_f32[:, j, :], us[j][:])
                        xns[j] = xn
                    nrs2 = {j: ln_stats(xns[j][:]) for j in jr}
                    for j in jr:
                        ln_apply(xns[j][:], x_f32[:, j, :], nrs2[j])
                    for j in jr:
                        set_x_from(j, psE)
                        if layer == n_layers - 1:
                            with tc.tile_pool(name=f"psL_{layer}_{j}",
                                              bufs=1, space="PSUM") as psL:
                                ps = psL.tile([P, VOCAB], F32, name="psLt")
                                for k in range(KD):
                                    nc.tensor.matmul(
                                        ps[:],
                                        lhsT=xT_h[:, k, j * P:(j + 1) * P],
                                        rhs=lmh_sb[:, k, :],
                                        start=(k == 0), stop=(k == KD - 1))
                                lg = lnp.tile([P, VOCAB], F32, name="lgt",
                                              tag="ln_f32", bufs=3)
                                nc.scalar.copy(out=lg[:], in_=ps[:])
                                nc.sync.dma_start(
                                    logits_d.ap()[j * P:(j + 1) * P, :],
                                    lg[:])
            if dbg and layer == 0:
                nc.sync.dma_start(
                    dbg_tensors["dbg_x1"].ap().rearrange(
                        "(j p) d -> p j d", p=P), x_f32[:])

        for _pool in (statp, lnp, arp, yxp, sdp, schp, ropep, csp,
                      dram, persist):
            _pool.release()

    nc.compile()
    return nc


def _host_inputs(idx, embed, encoder, encoder_v, decoder, lm_head):
    """Build the 8 per-core input maps (host-side sharding)."""
    import ml_dtypes
    f8e4 = ml_dtypes.float8_e4m3fn
    f16 = np.float16
    idx = np.asarray(idx).reshape(-1).astype(np.int64)
    embed = np.asarray(embed, np.float32)
    enc = np.asarray(encoder, np.float32)
    encv = np.asarray(encoder_v, np.float32)
    dec = np.asarray(decoder, np.float32)
    lmh = np.asarray(lm_head, np.float32)

    x0 = embed[idx]  # [T, D] gather on host (pure indexing)

    # freqs exactly as the reference computes them (fp32)
    t = np.arange(0, N, dtype=np.float32)
    q = np.floor(t / 2.0) * 2.0
    freqs = (1.0 / ((2.0 ** 16) ** (q / N)) / TWO_PI).astype(np.float32)
    tvec = np.arange(T, dtype=np.float32)

    umask = (np.arange(P)[:, None] < np.arange(P)[None, :]).astype(f16)

    in_maps = []
    for d in range(N_CORES):
        h, half = d // 2, d % 2
        perm = np.concatenate([np.arange(0, NLOC, 2),
                               np.arange(1, NLOC, 2)]) + half * NLOC
        encv128 = (encv[h][:, perm] * 128.0).astype(np.float32)
        encvh8 = encv128.astype(f8e4)
        encvl8 = (encv128 - encvh8.astype(np.float32)).astype(f8e4)
        encvh8 = np.ascontiguousarray(encvh8)
        encvl8 = np.ascontiguousarray(encvl8)
        f_loc = freqs[perm[:NLOC // 2]]
        ph = (tvec[None, :] * f_loc[:, None]).astype(np.float32) % 1.0
        cs = np.concatenate([np.cos(TWO_PI * ph), np.sin(TWO_PI * ph)],
                            axis=1)  # [NLOC//2, 2T]
        in_maps.append({
            "x0": np.ascontiguousarray(x0, f16),
            "encw": np.ascontiguousarray(enc[h][:, perm], f16),
            "encvh": encvh8,
            "encvl": encvl8,
            "decw": np.ascontiguousarray(dec[h * N + perm, :], f16),
            "cs": np.ascontiguousarray(cs, f16),
            "lmh": np.ascontiguousarray(lmh, f16),
            "umask": umask,
        })
    return in_maps


def kernel(idx, embed, encoder, encoder_v, decoder, lm_head,
           _trace=False, _tmpdir=None):
    if "nc" not in _CACHE:
        _CACHE["nc"] = _build_program()
    nc = _CACHE["nc"]
    in_maps = _host_inputs(idx, embed, encoder, encoder_v, decoder, lm_head)
    res = bass_utils.run_bass_kernel_spmd(
        nc, in_maps, core_ids=list(range(N_CORES)),
        trace=_trace, tmpdir=_tmpdir)
    _CACHE["last_results"] = res
    logits = res.results[0]["logits"].astype(np.float32).reshape(B, T, VOCAB)
    return logits


# revision 7
# speedup vs baseline: 1.3804x; 1.0186x over previous
"""Trainium2 Bass kernel for nn_BDH_1726576853700 (sparse_attention), v3.

3-layer sparse-attention net: B=1, T=1024, D=256, NH=4, N=8192, VOCAB=256.

Sharding over 8 NeuronCores: device d -> (head h=d//2, half=d%2) — each device
owns a 4096-wide slice of one head's sparse latent dim, permuted evens-first so
the RoPE pair partner is tile i+16.

v3 design:
  - S = qr^T qr runs in fp8e4 DoubleRow (2 k-tiles per matmul, 0.5 cycles/row).
    qr is quantized to fp8 at scale 32 (folded into the x_sparse relu evac);
    the scale washes out in the downstream LayerNorm.
  - encv projection runs as three fp8 DoubleRow terms at a common scale:
    vh^T yh + vh^T yl + vl^T yh, where vh=fp8(128 encv), vl=fp8(128 encv-vh)
    (host side) and yh=fp8(ykv_ln^T), yl=fp8(ykv_ln^T-yh) (device side,
    extracted during the transpose drain).  Residuals live in fp8 subnormals;
    they carry ~3%% of the magnitude so their quantization noise is ~0.1%%
    of the result.  Validated end-to-end at rel err 2.0e-3.
  - enc/dec stay fp16: every single-fp8 variant of the three projections
    measured over the 2e-2 gate in numpy rehearsal (enc8+x8: 3.8e-2,
    encv8+ykv8: 4.2e-2, dec8+xy8: 4.6e-2); hi/lo for dec needs an xy
    residual extraction that costs more DVE time than the PE it saves.
  - enc/dec weights resident in SBUF (shared by all 3 layers, loaded once).
  - cos/sin tables stream as ONE combined DMA per rope pair ([P, 2T] tile),
    alternating between the SP and Activation HWDGE queues.
  - Elementwise spread across Act/DVE/Pool with tunable splits; LayerNorm
    normalization on Act (Identity with per-partition scale/bias).
  - Engine streams are in-order: Phase C/E LayerNorm pipelines are emitted
    op-major (loop fission) so independent tiles don't serialize behind
    cross-engine latency chains.
  - Phase A emits m-tiles in (0,16,1,17,...) order and Phase B contracts
    k-pairs in (0,8,1,9,...) order so S matmuls chase the rope pipeline.
  - Phase D runs c-outer (two T-halves) so encv matmuls start on the first
    ykvT tiles right after the ykv AllReduce readback begins.

PSUM discipline: every accumulation group owns its bank(s) exclusively
(zero-region = 2KB = one bank).
"""

import math
import sys

for _p in ("/opt/trn_rl_repo",):
    if _p not in sys.path:
        sys.path.insert(0, _p)

import numpy as np

import concourse.bass as bass
import concourse.mybir as mybir
import concourse.tile as tile
from concourse import bacc, bass_utils
from concourse.masks import make_identity

# ---- problem constants (hardcoded per contract) ----
B, T, D, NH, N = 1, 1024, 256, 4, 8192
VOCAB = 256
N_LAYER = 3
EPS = 1e-5
TWO_PI = 2.0 * math.pi
N_CORES = 8
NLOC = N // 2          # latent columns per device: 4096
P = 128
NT = T // P            # 8 t-tiles
KD = D // P            # 2 d-tiles
NM = NLOC // P         # 32 n-tiles per device
NPAIR = NM // 2        # 16 rope pairs
HDT = mybir.dt.float16
F8 = mybir.dt.float8e4
F32 = mybir.dt.float32
DR = mybir.MatmulPerfMode.DoubleRow

XSP_SCALE = 32.0       # x_sparse stored at 32x so qr lands in fp8 normal range
SC_SCALE = 1.0 / 16.0  # S psum -> sc fp16 evacuation scale
YKV_SCALE = 1.0 / 256.0

_CACHE = {}


def _build_program(dbg=False, use_collectives=True, n_layers=N_LAYER,
                   n_pool_rope=0, qr8_split=(10, 0, 22), xsp_dve=0,
                   ysp_dve=8, sc_pool=False, sd_pool=True, ps_s_bufs=4):
    def emit_allreduce(nc, groups, ins, outs):
        if use_collectives:
            nc.gpsimd.collective_compute(
                "AllReduce", mybir.AluOpType.add, replica_groups=groups,
                ins=ins, outs=outs)
        else:
            nc.sync.dma_start(outs[0], ins[0])

    nc = bacc.Bacc("TRN2", target_bir_lowering=False, debug=False,
                   num_devices=N_CORES)
    dbg_tensors = {}
    if dbg:
        for nm, shape, dt in [
            ("dbg_x0ln", [T, D], F32),
            ("dbg_xsp", [NM * P, T], HDT),
            ("dbg_qr", [NM * P, T], F32),
            ("dbg_ykvpre", [T, D], HDT),
            ("dbg_ykvpost", [T, D], HDT),
            ("dbg_ykvT", [D, T], HDT),
            ("dbg_ymlppre", [D, T], HDT),
            ("dbg_ymlppost", [D, T], HDT),
            ("dbg_x1", [T, D], F32),
        ]:
            dbg_tensors[nm] = nc.dram_tensor(nm, shape, dt,
                                             kind="ExternalOutput")

    x0_d = nc.dram_tensor("x0", [T, D], HDT, kind="ExternalInput")
    encw_d = nc.dram_tensor("encw", [D, NLOC], HDT, kind="ExternalInput")
    encvh_d = nc.dram_tensor("encvh", [D, NLOC], F8, kind="ExternalInput")
    encvl_d = nc.dram_tensor("encvl", [D, NLOC], F8, kind="ExternalInput")
    decw_d = nc.dram_tensor("decw", [NLOC, D], HDT, kind="ExternalInput")
    cs_d = nc.dram_tensor("cs", [NLOC // 2, 2 * T], HDT, kind="ExternalInput")
    lmh_d = nc.dram_tensor("lmh", [D, VOCAB], HDT, kind="ExternalInput")
    umask_d = nc.dram_tensor("umask", [P, P], HDT, kind="ExternalInput")
    logits_d = nc.dram_tensor("logits", [T, VOCAB], F32, kind="ExternalOutput")

    PAIR_GROUPS = [[0, 1], [2, 3], [4, 5], [6, 7]]
    ALL_GROUP = [list(range(N_CORES))]

    # interleaved m emission order: pair halves adjacent (0,16,1,17,...)
    M_ORDER = []
    for i in range(NPAIR):
        M_ORDER += [i, i + NPAIR]
    # S contraction k-pair order: (qe pair u) then (qo pair u): 0,8,1,9,...
    K_ORDER = []
    for u in range(NPAIR // 2):
        K_ORDER += [u, u + NPAIR // 2]

    with tile.TileContext(nc) as tc:
        persist = tc.alloc_tile_pool(name="persist", bufs=1)
        dram = tc.alloc_tile_pool(name="dram", bufs=1, space="DRAM")

        # persistent SBUF state
        encw_sb = persist.tile([P, KD, NLOC], HDT)   # 16KB/part
        encvh_sb = persist.tile([P, KD, NLOC], F8)   # 8KB (128*encv hi)
        encvl_sb = persist.tile([P, KD, NLOC], F8)   # 8KB (residual)
        decw_sb = persist.tile([P, NM, D], HDT)      # 16KB
        x_sp = persist.tile([P, NM, T], HDT)         # 64KB, stored *XSP_SCALE
        qr8 = persist.tile([P, NM, T], F8)           # 32KB
        x_f32 = persist.tile([P, NT, D], F32)        # 8KB residual stream
        x_h = persist.tile([P, NT, D], HDT)          # 4KB
        xT_h = persist.tile([P, KD, T], HDT)         # 4KB
        yh8_sb = persist.tile([P, KD, T], F8)        # 2KB ykv_ln^T hi
        yl1_sb = persist.tile([P, KD, T], F8)        # 2KB ykv_ln^T residual
        lmh_sb = persist.tile([P, KD, VOCAB], HDT)
        umask_sb = persist.tile([P, P], HDT)
        ident = persist.tile([P, P], HDT)
        eps_sb = persist.tile([P, 1], F32)

        nc.vector.memset(eps_sb[:], float(EPS))
        nc.sync.dma_start(umask_sb[:], umask_d.ap())
        make_identity(nc, ident[:])

        # streaming / working pools
        csp = tc.alloc_tile_pool(name="csp", bufs=2)
        ropep = tc.alloc_tile_pool(name="ropep", bufs=2)
        schp = tc.alloc_tile_pool(name="schp", bufs=2)
        sdp = tc.alloc_tile_pool(name="sdp", bufs=2)
        yxp = tc.alloc_tile_pool(name="yxp", bufs=2)
        arp = tc.alloc_tile_pool(name="arp", bufs=1)
        lnp = tc.alloc_tile_pool(name="lnp", bufs=2)
        statp = tc.alloc_tile_pool(name="statp", bufs=8)

        def ln_stats(src_ap):
            """Emit stats chain ops; returns (nmur, rstd) [P,1] tiles."""
            stats = statp.tile([P, 6], F32, name="ln_stats")
            mv = statp.tile([P, 2], F32, name="ln_mv")
            rstd = statp.tile([P, 1], F32, name="ln_rstd")
            nmur = statp.tile([P, 1], F32, name="ln_nmur")
            nc.vector.bn_stats(out=stats[:], in_=src_ap)
            nc.vector.bn_aggr(out=mv[:], in_=stats[:])
            nc.scalar.activation(out=rstd[:], in_=mv[:, 1:2],
                                 func=mybir.ActivationFunctionType.Sqrt,
                                 bias=eps_sb[:])
            nc.vector.reciprocal(out=rstd[:], in_=rstd[:])
            nc.vector.tensor_scalar(out=nmur[:], in0=mv[:, 0:1],
                                    scalar1=rstd[:], scalar2=-1.0,
                                    op0=mybir.AluOpType.mult,
                                    op1=mybir.AluOpType.mult)
            return nmur, rstd

        def ln_apply(src_ap, out_ap, nr):
            nc.scalar.activation(out=out_ap, in_=src_ap,
                                 func=mybir.ActivationFunctionType.Identity,
                                 bias=nr[0][:], scale=nr[1][:])

        def layer_norm(src_ap, out_ap):
            ln_apply(src_ap, out_ap, ln_stats(src_ap))

        # Pool/GPSIMD cannot touch PSUM on real HW; PSUM evacuations must go
        # to DVE or Act.  Alternate between them for the transpose drains.
        _tr_rr = [0]

        def transpose_into(dst_ap, src_ap, pst_pool, eng=None):
            """PE-transpose a [P, P] fp16 SBUF block into dst (via PSUM)."""
            pst = pst_pool.tile([P, P], HDT, name="pst")
            nc.tensor.transpose(pst[:], src_ap, ident[:])
            if eng is None:
                _tr_rr[0] += 1
                if _tr_rr[0] % 2 == 0:
                    nc.vector.tensor_copy(out=dst_ap, in_=pst[:])
                else:
                    nc.scalar.copy(out=dst_ap, in_=pst[:])
            else:
                eng.tensor_copy(out=dst_ap, in_=pst[:])

        def set_x_from(j, pst_pool):
            """Write x_h/xT_h for t-tile j from x_f32."""
            nc.vector.tensor_copy(out=x_h[:, j, :], in_=x_f32[:, j, :])
            for k in range(KD):
                transpose_into(xT_h[:, k, j * P:(j + 1) * P],
                               x_h[:, j, k * P:(k + 1) * P], pst_pool)

        # ---- initial x = ln(embed[idx]) (gather done on host into x0) ----
        with tc.tile_pool(name="ps_init", bufs=2, space="PSUM") as ps_init:
            x0stage = arp.tile([P, NT, D], HDT, name="x0stage",
                               tag="ar_stage")
            nc.sync.dma_start(
                x0stage[:], x0_d.ap().rearrange("(j p) d -> p j d", p=P))
            for j in range(NT):
                layer_norm(x0stage[:, j, :], x_f32[:, j, :])
                set_x_from(j, ps_init)
        # weights load behind the init pipeline (enc is needed first, at A0)
        nc.sync.dma_start(
            encw_sb[:], encw_d.ap().rearrange("(k p) n -> p k n", p=P))
        nc.scalar.dma_start(
            encvh_sb[:], encvh_d.ap().rearrange("(k p) n -> p k n", p=P))
        nc.scalar.dma_start(
            encvl_sb[:], encvl_d.ap().rearrange("(k p) n -> p k n", p=P))
        nc.scalar.dma_start(
            decw_sb[:], decw_d.ap().rearrange("(m p) d -> p m d", p=P))
        for k in range(KD):
            nc.scalar.dma_start(lmh_sb[:, k, :],
                                lmh_d.ap()[k * P:(k + 1) * P, :])
        if dbg:
            nc.sync.dma_start(
                dbg_tensors["dbg_x0ln"].ap().rearrange("(j p) d -> p j d", p=P),
                x_f32[:])

        # ---- layers ----
        for layer in range(n_layers):
            # Phase A: x_sparse^T = relu(enc^T x^T)*XSP_SCALE, rope -> qr8.
            # m emitted interleaved so rope pair i fires after its 2 evacs.
            qr8_engs = ([nc.scalar] * qr8_split[0] + [nc.vector] * qr8_split[1]
                        + [nc.gpsimd] * qr8_split[2])
            qr8_engs = [qr8_engs[(7 * z) % len(qr8_engs)]
                        for z in range(len(qr8_engs))]
            with tc.tile_pool(name=f"psA_{layer}", bufs=2,
                              space="PSUM") as psA:
                for mi, m in enumerate(M_ORDER):
                    ps = psA.tile([P, T], F32, name="psA")
                    for c in range(2):
                        for k in range(KD):
                            nc.tensor.matmul(
                                ps[:, c * 512:(c + 1) * 512],
                                lhsT=encw_sb[:, k, m * P:(m + 1) * P],
                                rhs=xT_h[:, k, c * 512:(c + 1) * 512],
                                start=(k == 0), stop=(k == KD - 1))
                    if (mi * xsp_dve) % NM < xsp_dve:
                        # fused relu+scale on DVE: (ps max 0) * XSP_SCALE
                        nc.vector.tensor_scalar(
                            out=x_sp[:, m, :], in0=ps[:],
                            scalar1=0.0, scalar2=float(XSP_SCALE),
                            op0=mybir.AluOpType.max,
                            op1=mybir.AluOpType.mult)
                    else:
                        nc.scalar.activation(
                            out=x_sp[:, m, :], in_=ps[:],
                            func=mybir.ActivationFunctionType.Relu,
                            scale=float(XSP_SCALE))
                    if mi % 2 == 1:
                        i = m - NPAIR  # pair index just completed
                        cst = csp.tile([P, 2, T], HDT, name="cst")
                        dma_eng = nc.sync if i % 2 == 0 else nc.scalar
                        dma_eng.dma_start(
                            cst[:], cs_d.ap().rearrange(
                                "n (two t) -> n two t",
                                two=2)[i * P:(i + 1) * P, :, :])
                        xe = x_sp[:, i, :]
                        xo = x_sp[:, i + NPAIR, :]
                        ctt, stt = cst[:, 0, :], cst[:, 1, :]
                        engs = [nc.vector] * 6
                        for t in range(n_pool_rope):
                            engs[5 - t] = nc.gpsimd
                        t1 = ropep.tile([P, T], HDT, name="rope_t1",
                                        tag="rope_t", bufs=4)
                        t2 = ropep.tile([P, T], HDT, name="rope_t2",
                                        tag="rope_t", bufs=4)
                        qe = ropep.tile([P, T], HDT, name="rope_qe",
                                        tag="rope_q")
                        engs[0].tensor_mul(t1[:], xe, ctt)
                        engs[1].tensor_mul(t2[:], xo, stt)
                        engs[2].tensor_sub(qe[:], t1[:], t2[:])
                        t3 = ropep.tile([P, T], HDT, name="rope_t3",
                                        tag="rope_t", bufs=4)
                        t4 = ropep.tile([P, T], HDT, name="rope_t4",
                                        tag="rope_t", bufs=4)
                        qo = ropep.tile([P, T], HDT, name="rope_qo",
                                        tag="rope_q")
                        engs[3].tensor_mul(t3[:], xo, ctt)
                        engs[4].tensor_mul(t4[:], xe, stt)
                        engs[5].tensor_add(qo[:], t3[:], t4[:])
                        for src, dst_m, e in (
                                (qe, i, qr8_engs[2 * i]),
                                (qo, i + NPAIR, qr8_engs[2 * i + 1])):
                            if e is nc.scalar:
                                e.copy(out=qr8[:, dst_m, :], in_=src[:])
                            else:
                                e.tensor_copy(out=qr8[:, dst_m, :],
                                              in_=src[:])

            if dbg and layer == 0:
                nc.sync.dma_start(
                    dbg_tensors["dbg_xsp"].ap().rearrange(
                        "(m p) t -> p m t", p=P), x_sp[:])
                for m in range(NM):
                    qd = lnp.tile([P, T], F32, name="qr_dbg", tag="qr_dbg")
                    nc.vector.tensor_copy(out=qd[:], in_=qr8[:, m, :])
                    nc.sync.dma_start(
                        dbg_tensors["dbg_qr"].ap().rearrange(
                            "(m p) t -> p m t", p=P)[:, m, :], qd[:])

            # Phase B: S partial (fp8 DoubleRow) + causal mask + ykv partial.
            # One PSUM pool pair across both c passes: S chunks of the second
            # half start while the rope still streams (4 rotating S banks,
            # ykv banks handed from c=0 to c=1 by tag rotation).
            ykv_pre = arp.tile([P, NT, D], HDT, name="ykv_pre",
                               tag="ar_stage")
            with tc.tile_pool(name=f"psS_{layer}", bufs=ps_s_bufs,
                              space="PSUM") as psS, \
                 tc.tile_pool(name=f"psY_{layer}", bufs=1,
                              space="PSUM") as psY:
                # c=1 first: its 8 S chunks (the bulk) chase the rope for the
                # whole window; the narrow c=0 chunks drain in the tail.
                for c in (1, 0):
                    ykv_ps = [psY.tile([P, D], F32, name=f"ykv_ps{j}",
                                       tag=f"ykv_ps{j % 4}")
                              for j in range(4 * c, 4 * c + 4)]
                    for i in range(4 * c + 4):
                        # causal tiling: only columns t >= i*P are needed
                        base = max(c * 512, i * P)
                        width = (c + 1) * 512 - base
                        ps = psS.tile([P, 512], F32, name="psS")
                        for ku, u in enumerate(K_ORDER):
                            nc.tensor.matmul(
                                ps[:, :width],
                                lhsT=qr8[:, 2 * u:2 * u + 2,
                                         i * P:(i + 1) * P],
                                rhs=qr8[:, 2 * u:2 * u + 2,
                                        base:base + width],
                                start=(ku == 0), stop=(ku == NPAIR - 1),
                                perf_mode=DR)
                        sc = schp.tile([P, 512], HDT, name="schunk")
                        if sc_pool:
                            nc.gpsimd.tensor_scalar_mul(
                                sc[:, :width], ps[:, :width], float(SC_SCALE))
                        else:
                            nc.scalar.mul(out=sc[:, :width],
                                          in_=ps[:, :width],
                                          mul=float(SC_SCALE))
                        sd = None
                        if c == i // 4:
                            dcol = i * P - base
                            sd = sdp.tile([P, P], HDT, name="sdiag")
                            (nc.gpsimd if sd_pool else nc.vector).tensor_mul(
                                sd[:], sc[:, dcol:dcol + P], umask_sb[:])
                        for j in range(max(4 * c, i), 4 * c + 4):
                            lhsT = sd[:] if j == i else \
                                sc[:, j * P - base:(j + 1) * P - base]
                            nc.tensor.matmul(
                                ykv_ps[j - 4 * c][:], lhsT=lhsT,
                                rhs=x_h[:, i, :],
                                start=(i == 0), stop=(i == j))
                        if i >= 4 * c:
                            # group j=i just hit its stop: evacuate now so
                            # its bank recycles (3-bank psY rotation)
                            nc.scalar.mul(out=ykv_pre[:, i, :],
                                          in_=ykv_ps[i - 4 * c][:],
                                          mul=float(YKV_SCALE))

            if dbg and layer == 0:
                nc.sync.dma_start(
                    dbg_tensors["dbg_ykvpre"].ap().rearrange(
                        "(j p) d -> p j d", p=P), ykv_pre[:])

            # Phase C: pair AllReduce of ykv, layernorm, transpose.
            # Chunked staging DMAs + op-major (fissioned) LN pipeline.
            ar_in = dram.tile([T, D], HDT, name=f"arin_{layer}",
                              tag=f"arin_{layer}")
            ar_out = dram.tile([T, D], HDT, name=f"arout_{layer}",
                               tag=f"arout_{layer}")
            arin_p = ar_in.rearrange("(j p) d -> p j d", p=P)
            for jc in (2, 3, 0, 1):  # c=1 halves evac first now
                nc.sync.dma_start(arin_p[:, 2 * jc:2 * jc + 2, :],
                                  ykv_pre[:, 2 * jc:2 * jc + 2, :])
            emit_allreduce(nc, PAIR_GROUPS, [ar_in.opt()], [ar_out.opt()])
            ykv_post = arp.tile([P, NT, D], HDT, name="ykv_post",
                                tag="ar_stage")
            arout_p = ar_out.rearrange("(j p) d -> p j d", p=P)
            for jc in range(4):
                nc.sync.dma_start(ykv_post[:, 2 * jc:2 * jc + 2, :],
                                  arout_p[:, 2 * jc:2 * jc + 2, :])
            with tc.tile_pool(name=f"psT_{layer}", bufs=4,
                              space="PSUM") as psT:
                for jh in range(2):
                    jr = list(range(4 * jh, 4 * jh + 4))
                    nrs = [ln_stats(ykv_post[:, j, :]) for j in jr]
                    yls = []
                    for idx, j in enumerate(jr):
                        yl = lnp.tile([P, D], HDT, name="ykv_ln",
                                      tag="ln_f16", bufs=4)
                        ln_apply(ykv_post[:, j, :], yl[:], nrs[idx])
                        yls.append(yl)
                    for idx, j in enumerate(jr):
                        for k in range(KD):
                            pst = psT.tile([P, P], HDT, name="pst")
                            nc.tensor.transpose(
                                pst[:], yls[idx][:, k * P:(k + 1) * P],
                                ident[:])
                            dst = slice(j * P, (j + 1) * P)
                            nc.scalar.copy(out=yh8_sb[:, k, dst],
                                           in_=pst[:])
                            nc.vector.tensor_sub(yl1_sb[:, k, dst],
                                                 pst[:], yh8_sb[:, k, dst])

            if dbg and layer == 0:
                nc.sync.dma_start(
                    dbg_tensors["dbg_ykvpost"].ap().rearrange(
                        "(j p) d -> p j d", p=P), ykv_post[:])


            # Phase D: y_sp = relu(encv^T ykv_ln^T); xy = x_sp*y_sp;
            # ymlp^T accumulated with dec tiles as lhsT.  c-outer so the
            # first T-half starts as soon as ykvT columns 0..511 exist.
            ymlpT_pre = arp.tile([P, KD, T], HDT, name="ymlpT_pre",
                                 tag="ar_stage")
            ar2_in = dram.tile([D, T], HDT, name=f"ar2in_{layer}",
                               tag=f"ar2in_{layer}")
            ar2_out = dram.tile([D, T], HDT, name=f"ar2out_{layer}",
                                tag=f"ar2out_{layer}", addr_space="Shared")
            ar2in_p = ar2_in.rearrange("(k p) t -> p k t", p=P)
            with tc.tile_pool(name=f"psD_{layer}", bufs=4,
                              space="PSUM") as psD, \
                 tc.tile_pool(name=f"psM_{layer}", bufs=1,
                              space="PSUM") as psM:
                ymlpT_ps = [psM.tile([P, T], F32, name=f"ymlpT_ps{k}",
                                     tag=f"ymlpT_ps{k}") for k in range(KD)]
                for c in range(2):
                    cs = slice(c * 512, (c + 1) * 512)
                    for m in range(NM):
                        ps = psD.tile([P, 512], F32, name="psD")
                        msl = slice(m * P, (m + 1) * P)
                        terms = ((encvh_sb, yh8_sb), (encvh_sb, yl1_sb),
                                 (encvl_sb, yh8_sb))
                        for ti, (wsb, ysb) in enumerate(terms):
                            nc.tensor.matmul(
                                ps[:], lhsT=wsb[:, 0:2, msl],
                                rhs=ysb[:, 0:2, cs],
                                start=(ti == 0), stop=(ti == 2),
                                perf_mode=DR)
                        ysp = yxp.tile([P, 512], HDT, name="ysp")
                        if (m * ysp_dve) % NM < ysp_dve:
                            # relu + 1/128 unscale fused on DVE
                            nc.vector.tensor_scalar(
                                out=ysp[:], in0=ps[:],
                                scalar1=0.0, scalar2=1.0 / 128.0,
                                op0=mybir.AluOpType.max,
                                op1=mybir.AluOpType.mult)
                        else:
                            nc.scalar.activation(
                                out=ysp[:], in_=ps[:],
                                func=mybir.ActivationFunctionType.Relu,
                                scale=1.0 / 128.0)
                        xy = yxp.tile([P, 512], HDT, name="xy")
                        nc.vector.tensor_mul(xy[:], x_sp[:, m, cs], ysp[:])
                        for k in range(KD):
                            nc.tensor.matmul(
                                ymlpT_ps[k][:, cs],
                                lhsT=decw_sb[:, m, k * P:(k + 1) * P],
                                rhs=xy[:],
                                start=(m == 0), stop=(m == NM - 1))
                    for k in range(KD):
                        nc.scalar.copy(out=ymlpT_pre[:, k, cs],
                                       in_=ymlpT_ps[k][:, cs])
                        # upload this quarter while the next half computes
                        nc.sync.dma_start(ar2in_p[:, k, cs],
                                          ymlpT_pre[:, k, cs])

            if dbg and layer == 0:
                nc.sync.dma_start(
                    dbg_tensors["dbg_ymlppre"].ap().rearrange(
                        "(k p) t -> p k t", p=P), ymlpT_pre[:])

            # Phase E: 8-way AllReduce of ymlp^T; x = ln(x + ln(ymlp)).
            # Fissioned: transposes first, then the two LN chains op-major.
            emit_allreduce(nc, ALL_GROUP, [ar2_in.opt()], [ar2_out.opt()])
            ymlpT_post = arp.tile([P, KD, T], HDT, name="ymlpT_post",
                                  tag="ar_stage")
            ar2out_p = ar2_out.rearrange("(k p) t -> p k t", p=P)
            for kc in range(KD):
                nc.sync.dma_start(ymlpT_post[:, kc, :], ar2out_p[:, kc, :])
            if dbg and layer == 0:
                nc.sync.dma_start(
                    dbg_tensors["dbg_ymlppost"].ap().rearrange(
                        "(k p) t -> p k t", p=P), ymlpT_post[:])
            with tc.tile_pool(name=f"psE_{layer}", bufs=6,
                              space="PSUM") as psE:
                for jh in range(2):
                    jr = list(range(4 * jh, 4 * jh + 4))
                    ymts = {}
                    for j in jr:
                        ymt = lnp.tile([P, D], HDT, name="ymt",
                                       tag="ln_f16", bufs=4)
                        for k in range(KD):
                            transpose_into(
                                ymt[:, k * P:(k + 1) * P],
                                ymlpT_post[:, k, j * P:(j + 1) * P], psE)
                        ymts[j] = ymt
                    nrs = {j: ln_stats(ymts[j][:]) for j in jr}
                    us = {}
                    for j in jr:
                        u = lnp.tile([P, D], F32, name="u_ln",
                                     tag="ln_f32", bufs=3)
                        ln_apply(ymts[j][:], u[:], nrs[j])
                        us[j] = u
                    xns = {}
                    for j in jr:
                        xn = lnp.tile([P, D], F32, name="xn",
                                      tag="ln_f32x", bufs=3)
                        nc.vector.tensor_add(xn[:], x_f32[:, j, :], us[j][:])
                        xns[j] = xn
                    nrs2 = {j: ln_stats(xns[j][:]) for j in jr}
                    for j in jr:
                        ln_apply(xns[j][:], x_f32[:, j, :], nrs2[j])
                    for j in jr:
                        set_x_from(j, psE)
                        if layer == n_layers - 1:
                            with tc.tile_pool(name=f"psL_{layer}_{j}",
                                              bufs=1, space="PSUM") as psL:
                                ps = psL.tile([P, VOCAB], F32, name="psLt")
                                for k in range(KD):
                                    nc.tensor.matmul(
                                        ps[:],
                                        lhsT=xT_h[:, k, j * P:(j + 1) * P],
                                        rhs=lmh_sb[:, k, :],
                                        start=(k == 0), stop=(k == KD - 1))
                                lg = lnp.tile([P, VOCAB], F32, name="lgt",
                                              tag="ln_f32", bufs=3)
                                nc.scalar.copy(out=lg[:], in_=ps[:])
                                nc.sync.dma_start(
                                    logits_d.ap()[j * P:(j + 1) * P, :],
                                    lg[:])
            if dbg and layer == 0:
                nc.sync.dma_start(
                    dbg_tensors["dbg_x1"].ap().rearrange(
                        "(j p) d -> p j d", p=P), x_f32[:])

        for _pool in (statp, lnp, arp, yxp, sdp, schp, ropep, csp,
                      dram, persist):
            _pool.release()

    nc.compile()
    return nc


def _host_inputs(idx, embed, encoder, encoder_v, decoder, lm_head):
    """Build the 8 per-core input maps (host-side sharding)."""
    import ml_dtypes
    f8e4 = ml_dtypes.float8_e4m3fn
    f16 = np.float16
    idx = np.asarray(idx).reshape(-1).astype(np.int64)
    embed = np.asarray(embed, np.float32)
    enc = np.asarray(encoder, np.float32)
    encv = np.asarray(encoder_v, np.float32)
    dec = np.asarray(decoder, np.float32)
    lmh = np.asarray(lm_head, np.float32)

    x0 = embed[idx]  # [T, D] gather on host (pure indexing)

    # freqs exactly as the reference computes them (fp32)
    t = np.arange(0, N, dtype=np.float32)
    q = np.floor(t / 2.0) * 2.0
    freqs = (1.0 / ((2.0 ** 16) ** (q / N)) / TWO_PI).astype(np.float32)
    tvec = np.arange(T, dtype=np.float32)

    umask = (np.arange(P)[:, None] < np.arange(P)[None, :]).astype(f16)

    in_maps = []
    for d in range(N_CORES):
        h, half = d // 2, d % 2
        perm = np.concatenate([np.arange(0, NLOC, 2),
                               np.arange(1, NLOC, 2)]) + half * NLOC
        encv128 = (encv[h][:, perm] * 128.0).astype(np.float32)
        encvh8 = encv128.astype(f8e4)
        encvl8 = (encv128 - encvh8.astype(np.float32)).astype(f8e4)
        encvh8 = np.ascontiguousarray(encvh8)
        encvl8 = np.ascontiguousarray(encvl8)
        f_loc = freqs[perm[:NLOC // 2]]
        ph = (tvec[None, :] * f_loc[:, None]).astype(np.float32) % 1.0
        cs = np.concatenate([np.cos(TWO_PI * ph), np.sin(TWO_PI * ph)],
                            axis=1)  # [NLOC//2, 2T]
        in_maps.append({
            "x0": np.ascontiguousarray(x0, f16),
            "encw": np.ascontiguousarray(enc[h][:, perm], f16),
            "encvh": encvh8,
            "encvl": encvl8,
            "decw": np.ascontiguousarray(dec[h * N + perm, :], f16),
            "cs": np.ascontiguousarray(cs, f16),
            "lmh": np.ascontiguousarray(lmh, f16),
            "umask": umask,
        })
    return in_maps


def kernel(idx, embed, encoder, encoder_v, decoder, lm_head,
           _trace=False, _tmpdir=None):
    if "nc" not in _CACHE:
        _CACHE["nc"] = _build_program()
    nc = _CACHE["nc"]
    in_maps = _host_inputs(idx, embed, encoder, encoder_v, decoder, lm_head)
    res = bass_utils.run_bass_kernel_spmd(
        nc, in_maps, core_ids=list(range(N_CORES)),
        trace=_trace, tmpdir=_tmpdir)
    _CACHE["last_results"] = res
    logits = res.results[0]["logits"].astype(np.float32).reshape(B, T, VOCAB)
    return logits


# revision 8
# speedup vs baseline: 1.3899x; 1.0068x over previous
"""Trainium2 Bass kernel for nn_BDH_1726576853700 (sparse_attention), v3.

3-layer sparse-attention net: B=1, T=1024, D=256, NH=4, N=8192, VOCAB=256.

Sharding over 8 NeuronCores: device d -> (head h=d//2, half=d%2) — each device
owns a 4096-wide slice of one head's sparse latent dim, permuted evens-first so
the RoPE pair partner is tile i+16.

v3 design:
  - S = qr^T qr runs in fp8e4 DoubleRow (2 k-tiles per matmul, 0.5 cycles/row).
    qr is quantized to fp8 at scale 32 (folded into the x_sparse relu evac);
    the scale washes out in the downstream LayerNorm.
  - encv projection runs as three fp8 DoubleRow terms at a common scale:
    vh^T yh + vh^T yl + vl^T yh, where vh=fp8(128 encv), vl=fp8(128 encv-vh)
    (host side) and yh=fp8(ykv_ln^T), yl=fp8(ykv_ln^T-yh) (device side,
    extracted during the transpose drain).  Residuals live in fp8 subnormals;
    they carry ~3%% of the magnitude so their quantization noise is ~0.1%%
    of the result.  Validated end-to-end at rel err 2.0e-3.
  - enc/dec stay fp16: every single-fp8 variant of the three projections
    measured over the 2e-2 gate in numpy rehearsal (enc8+x8: 3.8e-2,
    encv8+ykv8: 4.2e-2, dec8+xy8: 4.6e-2); hi/lo for dec needs an xy
    residual extraction that costs more DVE time than the PE it saves.
  - enc/dec weights resident in SBUF (shared by all 3 layers, loaded once).
  - cos/sin tables stream as ONE combined DMA per rope pair ([P, 2T] tile),
    alternating between the SP and Activation HWDGE queues.
  - Elementwise spread across Act/DVE/Pool with tunable splits; LayerNorm
    normalization on Act (Identity with per-partition scale/bias).
  - Engine streams are in-order: Phase C/E LayerNorm pipelines are emitted
    op-major (loop fission) so independent tiles don't serialize behind
    cross-engine latency chains.
  - Phase A emits m-tiles in (0,16,1,17,...) order and Phase B contracts
    k-pairs in (0,8,1,9,...) order so S matmuls chase the rope pipeline.
  - Phase D runs c-outer (two T-halves) so encv matmuls start on the first
    ykvT tiles right after the ykv AllReduce readback begins.

PSUM discipline: every accumulation group owns its bank(s) exclusively
(zero-region = 2KB = one bank).
"""

import math
import sys

for _p in ("/opt/trn_rl_repo",):
    if _p not in sys.path:
        sys.path.insert(0, _p)

import numpy as np

import concourse.bass as bass
import concourse.mybir as mybir
import concourse.tile as tile
from concourse import bacc, bass_utils
from concourse.masks import make_identity

# ---- problem constants (hardcoded per contract) ----
B, T, D, NH, N = 1, 1024, 256, 4, 8192
VOCAB = 256
N_LAYER = 3
EPS = 1e-5
TWO_PI = 2.0 * math.pi
N_CORES = 8
NLOC = N // 2          # latent columns per device: 4096
P = 128
NT = T // P            # 8 t-tiles
KD = D // P            # 2 d-tiles
NM = NLOC // P         # 32 n-tiles per device
NPAIR = NM // 2        # 16 rope pairs
HDT = mybir.dt.float16
F8 = mybir.dt.float8e4
F32 = mybir.dt.float32
DR = mybir.MatmulPerfMode.DoubleRow

XSP_SCALE = 32.0       # x_sparse stored at 32x so qr lands in fp8 normal range
SC_SCALE = 1.0 / 16.0  # S psum -> sc fp16 evacuation scale
YKV_SCALE = 1.0 / 256.0

_CACHE = {}


def _build_program(dbg=False, use_collectives=True, n_layers=N_LAYER,
                   n_pool_rope=0, qr8_split=(10, 0, 22), xsp_dve=0,
                   ysp_dve=8, sc_pool=False, sd_pool=False, ps_s_bufs=4):
    def emit_allreduce(nc, groups, ins, outs):
        if use_collectives:
            nc.gpsimd.collective_compute(
                "AllReduce", mybir.AluOpType.add, replica_groups=groups,
                ins=ins, outs=outs)
        else:
            nc.sync.dma_start(outs[0], ins[0])

    nc = bacc.Bacc("TRN2", target_bir_lowering=False, debug=False,
                   num_devices=N_CORES)
    dbg_tensors = {}
    if dbg:
        for nm, shape, dt in [
            ("dbg_x0ln", [T, D], F32),
            ("dbg_xsp", [NM * P, T], HDT),
            ("dbg_qr", [NM * P, T], F32),
            ("dbg_ykvpre", [T, D], HDT),
            ("dbg_ykvpost", [T, D], HDT),
            ("dbg_ykvT", [D, T], HDT),
            ("dbg_ymlppre", [D, T], HDT),
            ("dbg_ymlppost", [D, T], HDT),
            ("dbg_x1", [T, D], F32),
        ]:
            dbg_tensors[nm] = nc.dram_tensor(nm, shape, dt,
                                             kind="ExternalOutput")

    x0_d = nc.dram_tensor("x0", [T, D], HDT, kind="ExternalInput")
    encw_d = nc.dram_tensor("encw", [D, NLOC], HDT, kind="ExternalInput")
    encvh_d = nc.dram_tensor("encvh", [D, NLOC], F8, kind="ExternalInput")
    encvl_d = nc.dram_tensor("encvl", [D, NLOC], F8, kind="ExternalInput")
    decw_d = nc.dram_tensor("decw", [NLOC, D], HDT, kind="ExternalInput")
    cs_d = nc.dram_tensor("cs", [NLOC // 2, 2 * T], HDT, kind="ExternalInput")
    lmh_d = nc.dram_tensor("lmh", [D, VOCAB], HDT, kind="ExternalInput")
    umask_d = nc.dram_tensor("umask", [P, P], HDT, kind="ExternalInput")
    logits_d = nc.dram_tensor("logits", [T, VOCAB], F32, kind="ExternalOutput")

    PAIR_GROUPS = [[0, 1], [2, 3], [4, 5], [6, 7]]
    ALL_GROUP = [list(range(N_CORES))]

    # interleaved m emission order: pair halves adjacent (0,16,1,17,...)
    M_ORDER = []
    for i in range(NPAIR):
        M_ORDER += [i, i + NPAIR]
    # S contraction k-pair order: (qe pair u) then (qo pair u): 0,8,1,9,...
    K_ORDER = []
    for u in range(NPAIR // 2):
        K_ORDER += [u, u + NPAIR // 2]

    with tile.TileContext(nc) as tc:
        persist = tc.alloc_tile_pool(name="persist", bufs=1)
        dram = tc.alloc_tile_pool(name="dram", bufs=1, space="DRAM")

        # persistent SBUF state
        encw_sb = persist.tile([P, KD, NLOC], HDT)   # 16KB/part
        encvh_sb = persist.tile([P, KD, NLOC], F8)   # 8KB (128*encv hi)
        encvl_sb = persist.tile([P, KD, NLOC], F8)   # 8KB (residual)
        decw_sb = persist.tile([P, NM, D], HDT)      # 16KB
        x_sp = persist.tile([P, NM, T], HDT)         # 64KB, stored *XSP_SCALE
        qr8 = persist.tile([P, NM, T], F8)           # 32KB
        x_f32 = persist.tile([P, NT, D], F32)        # 8KB residual stream
        x_h = persist.tile([P, NT, D], HDT)          # 4KB
        xT_h = persist.tile([P, KD, T], HDT)         # 4KB
        yh8_sb = persist.tile([P, KD, T], F8)        # 2KB ykv_ln^T hi
        yl1_sb = persist.tile([P, KD, T], F8)        # 2KB ykv_ln^T residual
        lmh_sb = persist.tile([P, KD, VOCAB], HDT)
        umask_sb = persist.tile([P, P], HDT)
        ident = persist.tile([P, P], HDT)
        eps_sb = persist.tile([P, 1], F32)

        nc.vector.memset(eps_sb[:], float(EPS))
        nc.sync.dma_start(umask_sb[:], umask_d.ap())
        make_identity(nc, ident[:])

        # streaming / working pools
        csp = tc.alloc_tile_pool(name="csp", bufs=2)
        ropep = tc.alloc_tile_pool(name="ropep", bufs=2)
        schp = tc.alloc_tile_pool(name="schp", bufs=2)
        sdp = tc.alloc_tile_pool(name="sdp", bufs=2)
        yxp = tc.alloc_tile_pool(name="yxp", bufs=2)
        arp = tc.alloc_tile_pool(name="arp", bufs=1)
        lnp = tc.alloc_tile_pool(name="lnp", bufs=2)
        statp = tc.alloc_tile_pool(name="statp", bufs=8)

        def ln_stats(src_ap):
            """Emit stats chain ops; returns (nmur, rstd) [P,1] tiles."""
            stats = statp.tile([P, 6], F32, name="ln_stats")
            mv = statp.tile([P, 2], F32, name="ln_mv")
            rstd = statp.tile([P, 1], F32, name="ln_rstd")
            nmur = statp.tile([P, 1], F32, name="ln_nmur")
            nc.vector.bn_stats(out=stats[:], in_=src_ap)
            nc.vector.bn_aggr(out=mv[:], in_=stats[:])
            nc.scalar.activation(out=rstd[:], in_=mv[:, 1:2],
                                 func=mybir.ActivationFunctionType.Sqrt,
                                 bias=eps_sb[:])
            nc.vector.reciprocal(out=rstd[:], in_=rstd[:])
            nc.vector.tensor_scalar(out=nmur[:], in0=mv[:, 0:1],
                                    scalar1=rstd[:], scalar2=-1.0,
                                    op0=mybir.AluOpType.mult,
                                    op1=mybir.AluOpType.mult)
            return nmur, rstd

        def ln_apply(src_ap, out_ap, nr):
            nc.scalar.activation(out=out_ap, in_=src_ap,
                                 func=mybir.ActivationFunctionType.Identity,
                                 bias=nr[0][:], scale=nr[1][:])

        def layer_norm(src_ap, out_ap):
            ln_apply(src_ap, out_ap, ln_stats(src_ap))

        # Pool/GPSIMD cannot touch PSUM on real HW; PSUM evacuations must go
        # to DVE or Act.  Alternate between them for the transpose drains.
        _tr_rr = [0]

        def transpose_into(dst_ap, src_ap, pst_pool, eng=None):
            """PE-transpose a [P, P] fp16 SBUF block into dst (via PSUM)."""
            pst = pst_pool.tile([P, P], HDT, name="pst")
            nc.tensor.transpose(pst[:], src_ap, ident[:])
            if eng is None:
                _tr_rr[0] += 1
                if _tr_rr[0] % 2 == 0:
                    nc.vector.tensor_copy(out=dst_ap, in_=pst[:])
                else:
                    nc.scalar.copy(out=dst_ap, in_=pst[:])
            else:
                eng.tensor_copy(out=dst_ap, in_=pst[:])

        def set_x_from(j, pst_pool):
            """Write x_h/xT_h for t-tile j from x_f32."""
            nc.vector.tensor_copy(out=x_h[:, j, :], in_=x_f32[:, j, :])
            for k in range(KD):
                transpose_into(xT_h[:, k, j * P:(j + 1) * P],
                               x_h[:, j, k * P:(k + 1) * P], pst_pool)

        # ---- initial x = ln(embed[idx]) (gather done on host into x0) ----
        with tc.tile_pool(name="ps_init", bufs=2, space="PSUM") as ps_init:
            x0stage = arp.tile([P, NT, D], HDT, name="x0stage",
                               tag="ar_stage")
            nc.sync.dma_start(
                x0stage[:], x0_d.ap().rearrange("(j p) d -> p j d", p=P))
            for j in range(NT):
                layer_norm(x0stage[:, j, :], x_f32[:, j, :])
                set_x_from(j, ps_init)
        # weights load behind the init pipeline (enc is needed first, at A0)
        nc.sync.dma_start(
            encw_sb[:], encw_d.ap().rearrange("(k p) n -> p k n", p=P))
        nc.scalar.dma_start(
            encvh_sb[:], encvh_d.ap().rearrange("(k p) n -> p k n", p=P))
        nc.scalar.dma_start(
            encvl_sb[:], encvl_d.ap().rearrange("(k p) n -> p k n", p=P))
        nc.scalar.dma_start(
            decw_sb[:], decw_d.ap().rearrange("(m p) d -> p m d", p=P))
        for k in range(KD):
            nc.scalar.dma_start(lmh_sb[:, k, :],
                                lmh_d.ap()[k * P:(k + 1) * P, :])
        if dbg:
            nc.sync.dma_start(
                dbg_tensors["dbg_x0ln"].ap().rearrange("(j p) d -> p j d", p=P),
                x_f32[:])

        # ---- layers ----
        for layer in range(n_layers):
            # Phase A: x_sparse^T = relu(enc^T x^T)*XSP_SCALE, rope -> qr8.
            # m emitted interleaved so rope pair i fires after its 2 evacs.
            qr8_engs = ([nc.scalar] * qr8_split[0] + [nc.vector] * qr8_split[1]
                        + [nc.gpsimd] * qr8_split[2])
            qr8_engs = [qr8_engs[(7 * z) % len(qr8_engs)]
                        for z in range(len(qr8_engs))]
            with tc.tile_pool(name=f"psA_{layer}", bufs=2,
                              space="PSUM") as psA:
                for mi, m in enumerate(M_ORDER):
                    ps = psA.tile([P, T], F32, name="psA")
                    for c in range(2):
                        for k in range(KD):
                            nc.tensor.matmul(
                                ps[:, c * 512:(c + 1) * 512],
                                lhsT=encw_sb[:, k, m * P:(m + 1) * P],
                                rhs=xT_h[:, k, c * 512:(c + 1) * 512],
                                start=(k == 0), stop=(k == KD - 1))
                    if (mi * xsp_dve) % NM < xsp_dve:
                        # fused relu+scale on DVE: (ps max 0) * XSP_SCALE
                        nc.vector.tensor_scalar(
                            out=x_sp[:, m, :], in0=ps[:],
                            scalar1=0.0, scalar2=float(XSP_SCALE),
                            op0=mybir.AluOpType.max,
                            op1=mybir.AluOpType.mult)
                    else:
                        nc.scalar.activation(
                            out=x_sp[:, m, :], in_=ps[:],
                            func=mybir.ActivationFunctionType.Relu,
                            scale=float(XSP_SCALE))
                    if mi % 2 == 1:
                        i = m - NPAIR  # pair index just completed
                        cst = csp.tile([P, 2, T], HDT, name="cst")
                        dma_eng = nc.sync if i % 2 == 0 else nc.scalar
                        dma_eng.dma_start(
                            cst[:], cs_d.ap().rearrange(
                                "n (two t) -> n two t",
                                two=2)[i * P:(i + 1) * P, :, :])
                        xe = x_sp[:, i, :]
                        xo = x_sp[:, i + NPAIR, :]
                        ctt, stt = cst[:, 0, :], cst[:, 1, :]
                        engs = [nc.vector] * 6
                        for t in range(n_pool_rope):
                            engs[5 - t] = nc.gpsimd
                        t1 = ropep.tile([P, T], HDT, name="rope_t1",
                                        tag="rope_t", bufs=4)
                        t2 = ropep.tile([P, T], HDT, name="rope_t2",
                                        tag="rope_t", bufs=4)
                        qe = ropep.tile([P, T], HDT, name="rope_qe",
                                        tag="rope_q")
                        engs[0].tensor_mul(t1[:], xe, ctt)
                        engs[1].tensor_mul(t2[:], xo, stt)
                        engs[2].tensor_sub(qe[:], t1[:], t2[:])
                        t3 = ropep.tile([P, T], HDT, name="rope_t3",
                                        tag="rope_t", bufs=4)
                        t4 = ropep.tile([P, T], HDT, name="rope_t4",
                                        tag="rope_t", bufs=4)
                        qo = ropep.tile([P, T], HDT, name="rope_qo",
                                        tag="rope_q")
                        engs[3].tensor_mul(t3[:], xo, ctt)
                        engs[4].tensor_mul(t4[:], xe, stt)
                        engs[5].tensor_add(qo[:], t3[:], t4[:])
                        for src, dst_m, e in (
                                (qe, i, qr8_engs[2 * i]),
                                (qo, i + NPAIR, qr8_engs[2 * i + 1])):
                            if e is nc.scalar:
                                e.copy(out=qr8[:, dst_m, :], in_=src[:])
                            else:
                                e.tensor_copy(out=qr8[:, dst_m, :],
                                              in_=src[:])

            if dbg and layer == 0:
                nc.sync.dma_start(
                    dbg_tensors["dbg_xsp"].ap().rearrange(
                        "(m p) t -> p m t", p=P), x_sp[:])
                for m in range(NM):
                    qd = lnp.tile([P, T], F32, name="qr_dbg", tag="qr_dbg")
                    nc.vector.tensor_copy(out=qd[:], in_=qr8[:, m, :])
                    nc.sync.dma_start(
                        dbg_tensors["dbg_qr"].ap().rearrange(
                            "(m p) t -> p m t", p=P)[:, m, :], qd[:])

            # Phase B: S partial (fp8 DoubleRow) + causal mask + ykv partial.
            # One PSUM pool pair across both c passes: S chunks of the second
            # half start while the rope still streams (4 rotating S banks,
            # ykv banks handed from c=0 to c=1 by tag rotation).
            ykv_pre = arp.tile([P, NT, D], HDT, name="ykv_pre",
                               tag="ar_stage")
            with tc.tile_pool(name=f"psS_{layer}", bufs=ps_s_bufs,
                              space="PSUM") as psS, \
                 tc.tile_pool(name=f"psY_{layer}", bufs=1,
                              space="PSUM") as psY:
                # c=1 first: its 8 S chunks (the bulk) chase the rope for the
                # whole window; the narrow c=0 chunks drain in the tail.
                for c in (1, 0):
                    ykv_ps = [psY.tile([P, D], F32, name=f"ykv_ps{j}",
                                       tag=f"ykv_ps{j % 4}")
                              for j in range(4 * c, 4 * c + 4)]
                    for i in range(4 * c + 4):
                        # causal tiling: only columns t >= i*P are needed
                        base = max(c * 512, i * P)
                        width = (c + 1) * 512 - base
                        ps = psS.tile([P, 512], F32, name="psS")
                        for ku, u in enumerate(K_ORDER):
                            nc.tensor.matmul(
                                ps[:, :width],
                                lhsT=qr8[:, 2 * u:2 * u + 2,
                                         i * P:(i + 1) * P],
                                rhs=qr8[:, 2 * u:2 * u + 2,
                                        base:base + width],
                                start=(ku == 0), stop=(ku == NPAIR - 1),
                                perf_mode=DR)
                        sc = schp.tile([P, 512], HDT, name="schunk")
                        if sc_pool:
                            nc.gpsimd.tensor_scalar_mul(
                                sc[:, :width], ps[:, :width], float(SC_SCALE))
                        else:
                            nc.scalar.mul(out=sc[:, :width],
                                          in_=ps[:, :width],
                                          mul=float(SC_SCALE))
                        sd = None
                        if c == i // 4:
                            dcol = i * P - base
                            sd = sdp.tile([P, P], HDT, name="sdiag")
                            (nc.gpsimd if sd_pool else nc.vector).tensor_mul(
                                sd[:], sc[:, dcol:dcol + P], umask_sb[:])
                        for j in range(max(4 * c, i), 4 * c + 4):
                            lhsT = sd[:] if j == i else \
                                sc[:, j * P - base:(j + 1) * P - base]
                            nc.tensor.matmul(
                                ykv_ps[j - 4 * c][:], lhsT=lhsT,
                                rhs=x_h[:, i, :],
                                start=(i == 0), stop=(i == j))
                        if i >= 4 * c:
                            # group j=i just hit its stop: evacuate now so
                            # its bank recycles (3-bank psY rotation)
                            nc.scalar.mul(out=ykv_pre[:, i, :],
                                          in_=ykv_ps[i - 4 * c][:],
                                          mul=float(YKV_SCALE))

            if dbg and layer == 0:
                nc.sync.dma_start(
                    dbg_tensors["dbg_ykvpre"].ap().rearrange(
                        "(j p) d -> p j d", p=P), ykv_pre[:])

            # Phase C: pair AllReduce of ykv, layernorm, transpose.
            # Chunked staging DMAs + op-major (fissioned) LN pipeline.
            ar_in = dram.tile([T, D], HDT, name=f"arin_{layer}",
                              tag=f"arin_{layer}")
            ar_out = dram.tile([T, D], HDT, name=f"arout_{layer}",
                               tag=f"arout_{layer}")
            arin_p = ar_in.rearrange("(j p) d -> p j d", p=P)
            for jc in (2, 3, 0, 1):  # c=1 halves evac first now
                nc.sync.dma_start(arin_p[:, 2 * jc:2 * jc + 2, :],
                                  ykv_pre[:, 2 * jc:2 * jc + 2, :])
            emit_allreduce(nc, PAIR_GROUPS, [ar_in.opt()], [ar_out.opt()])
            ykv_post = arp.tile([P, NT, D], HDT, name="ykv_post",
                                tag="ar_stage")
            arout_p = ar_out.rearrange("(j p) d -> p j d", p=P)
            for jc in range(4):
                nc.sync.dma_start(ykv_post[:, 2 * jc:2 * jc + 2, :],
                                  arout_p[:, 2 * jc:2 * jc + 2, :])
            with tc.tile_pool(name=f"psT_{layer}", bufs=4,
                              space="PSUM") as psT:
                for jh in range(2):
                    jr = list(range(4 * jh, 4 * jh + 4))
                    nrs = [ln_stats(ykv_post[:, j, :]) for j in jr]
                    yls = []
                    for idx, j in enumerate(jr):
                        yl = lnp.tile([P, D], HDT, name="ykv_ln",
                                      tag="ln_f16", bufs=4)
                        ln_apply(ykv_post[:, j, :], yl[:], nrs[idx])
                        yls.append(yl)
                    for idx, j in enumerate(jr):
                        for k in range(KD):
                            pst = psT.tile([P, P], HDT, name="pst")
                            nc.tensor.transpose(
                                pst[:], yls[idx][:, k * P:(k + 1) * P],
                                ident[:])
                            dst = slice(j * P, (j + 1) * P)
                            nc.scalar.copy(out=yh8_sb[:, k, dst],
                                           in_=pst[:])
                            nc.vector.tensor_sub(yl1_sb[:, k, dst],
                                                 pst[:], yh8_sb[:, k, dst])

            if dbg and layer == 0:
                nc.sync.dma_start(
                    dbg_tensors["dbg_ykvpost"].ap().rearrange(
                        "(j p) d -> p j d", p=P), ykv_post[:])


            # Phase D: y_sp = relu(encv^T ykv_ln^T); xy = x_sp*y_sp;
            # ymlp^T accumulated with dec tiles as lhsT.  c-outer so the
            # first T-half starts as soon as ykvT columns 0..511 exist.
            ymlpT_pre = arp.tile([P, KD, T], HDT, name="ymlpT_pre",
                                 tag="ar_stage")
            ar2_in = dram.tile([D, T], HDT, name=f"ar2in_{layer}",
                               tag=f"ar2in_{layer}")
            ar2_out = dram.tile([D, T], HDT, name=f"ar2out_{layer}",
                                tag=f"ar2out_{layer}", addr_space="Shared")
            ar2in_p = ar2_in.rearrange("(k p) t -> p k t", p=P)
            with tc.tile_pool(name=f"psD_{layer}", bufs=4,
                              space="PSUM") as psD, \
                 tc.tile_pool(name=f"psM_{layer}", bufs=1,
                              space="PSUM") as psM:
                ymlpT_ps = [psM.tile([P, T], F32, name=f"ymlpT_ps{k}",
                                     tag=f"ymlpT_ps{k}") for k in range(KD)]
                for c in range(2):
                    cs = slice(c * 512, (c + 1) * 512)
                    for m in range(NM):
                        ps = psD.tile([P, 512], F32, name="psD")
                        msl = slice(m * P, (m + 1) * P)
                        terms = ((encvh_sb, yh8_sb), (encvh_sb, yl1_sb),
                                 (encvl_sb, yh8_sb))
                        for ti, (wsb, ysb) in enumerate(terms):
                            nc.tensor.matmul(
                                ps[:], lhsT=wsb[:, 0:2, msl],
                                rhs=ysb[:, 0:2, cs],
                                start=(ti == 0), stop=(ti == 2),
                                perf_mode=DR)
                        ysp = yxp.tile([P, 512], HDT, name="ysp")
                        if (m * ysp_dve) % NM < ysp_dve:
                            # relu + 1/128 unscale fused on DVE
                            nc.vector.tensor_scalar(
                                out=ysp[:], in0=ps[:],
                                scalar1=0.0, scalar2=1.0 / 128.0,
                                op0=mybir.AluOpType.max,
                                op1=mybir.AluOpType.mult)
                        else:
                            nc.scalar.activation(
                                out=ysp[:], in_=ps[:],
                                func=mybir.ActivationFunctionType.Relu,
                                scale=1.0 / 128.0)
                        xy = yxp.tile([P, 512], HDT, name="xy")
                        nc.vector.tensor_mul(xy[:], x_sp[:, m, cs], ysp[:])
                        for k in range(KD):
                            nc.tensor.matmul(
                                ymlpT_ps[k][:, cs],
                                lhsT=decw_sb[:, m, k * P:(k + 1) * P],
                                rhs=xy[:],
                                start=(m == 0), stop=(m == NM - 1))
                    for k in range(KD):
                        nc.scalar.copy(out=ymlpT_pre[:, k, cs],
                                       in_=ymlpT_ps[k][:, cs])
                        # upload this quarter while the next half computes
                        nc.sync.dma_start(ar2in_p[:, k, cs],
                                          ymlpT_pre[:, k, cs])

            if dbg and layer == 0:
                nc.sync.dma_start(
                    dbg_tensors["dbg_ymlppre"].ap().rearrange(
                        "(k p) t -> p k t", p=P), ymlpT_pre[:])

            # Phase E: 8-way AllReduce of ymlp^T; x = ln(x + ln(ymlp)).
            # Fissioned: transposes first, then the two LN chains op-major.
            emit_allreduce(nc, ALL_GROUP, [ar2_in.opt()], [ar2_out.opt()])
            ymlpT_post = arp.tile([P, KD, T], HDT, name="ymlpT_post",
                                  tag="ar_stage")
            ar2out_p = ar2_out.rearrange("(k p) t -> p k t", p=P)
            for kc in range(KD):
                nc.sync.dma_start(ymlpT_post[:, kc, :], ar2out_p[:, kc, :])
            if dbg and layer == 0:
                nc.sync.dma_start(
                    dbg_tensors["dbg_ymlppost"].ap().rearrange(
                        "(k p) t -> p k t", p=P), ymlpT_post[:])
            with tc.tile_pool(name=f"psE_{layer}", bufs=6,
                              space="PSUM") as psE:
                for jh in range(2):
                    jr = list(range(4 * jh, 4 * jh + 4))
                    ymts = {}
                    for j in jr:
                        ymt = lnp.tile([P, D], HDT, name="ymt",
                                       tag="ln_f16", bufs=4)
                        for k in range(KD):
                            transpose_into(
                                ymt[:, k * P:(k + 1) * P],
                                ymlpT_post[:, k, j * P:(j + 1) * P], psE)
                        ymts[j] = ymt
                    nrs = {j: ln_stats(ymts[j][:]) for j in jr}
                    us = {}
                    for j in jr:
                        u = lnp.tile([P, D], F32, name="u_ln",
                                     tag="ln_f32", bufs=3)
                        ln_apply(ymts[j][:], u[:], nrs[j])
                        us[j] = u
                    xns = {}
                    for j in jr:
                        xn = lnp.tile([P, D], F32, name="xn",
                                      tag="ln_f32x", bufs=3)
                        nc.vector.tensor_add(xn[:], x_f32[:, j, :], us[j][:])
                        xns[j] = xn
                    nrs2 = {j: ln_stats(xns[j][:]) for j in jr}
                    for j in jr:
                        ln_apply(xns[j][:], x_f32[:, j, :], nrs2[j])
                    for j in jr:
                        set_x_from(j, psE)
                        if layer == n_layers - 1:
                            with tc.tile_pool(name=f"psL_{layer}_{j}",
                                              bufs=1, space="PSUM") as psL:
                                ps = psL.tile([P, VOCAB], F32, name="psLt")
                                for k in range(KD):
                                    nc.tensor.matmul(
                                        ps[:],
                                        lhsT=xT_h[:, k, j * P:(j + 1) * P],
                                        rhs=lmh_sb[:, k, :],
                                        start=(k == 0), stop=(k == KD - 1))
                                lg = lnp.tile([P, VOCAB], F32, name="lgt",
                                              tag="ln_f32", bufs=3)
                                nc.scalar.copy(out=lg[:], in_=ps[:])
                                nc.sync.dma_start(
                                    logits_d.ap()[j * P:(j + 1) * P, :],
                                    lg[:])
            if dbg and layer == 0:
                nc.sync.dma_start(
                    dbg_tensors["dbg_x1"].ap().rearrange(
                        "(j p) d -> p j d", p=P), x_f32[:])

        for _pool in (statp, lnp, arp, yxp, sdp, schp, ropep, csp,
                      dram, persist):
            _pool.release()

    nc.compile()
    return nc


def _host_inputs(idx, embed, encoder, encoder_v, decoder, lm_head):
    """Build the 8 per-core input maps (host-side sharding)."""
    import ml_dtypes
    f8e4 = ml_dtypes.float8_e4m3fn
    f16 = np.float16
    idx = np.asarray(idx).reshape(-1).astype(np.int64)
    embed = np.asarray(embed, np.float32)
    enc = np.asarray(encoder, np.float32)
    encv = np.asarray(encoder_v, np.float32)
    dec = np.asarray(decoder, np.float32)
    lmh = np.asarray(lm_head, np.float32)

    x0 = embed[idx]  # [T, D] gather on host (pure indexing)

    # freqs exactly as the reference computes them (fp32)
    t = np.arange(0, N, dtype=np.float32)
    q = np.floor(t / 2.0) * 2.0
    freqs = (1.0 / ((2.0 ** 16) ** (q / N)) / TWO_PI).astype(np.float32)
    tvec = np.arange(T, dtype=np.float32)

    umask = (np.arange(P)[:, None] < np.arange(P)[None, :]).astype(f16)

    in_maps = []
    for d in range(N_CORES):
        h, half = d // 2, d % 2
        perm = np.concatenate([np.arange(0, NLOC, 2),
                               np.arange(1, NLOC, 2)]) + half * NLOC
        encv128 = (encv[h][:, perm] * 128.0).astype(np.float32)
        encvh8 = encv128.astype(f8e4)
        encvl8 = (encv128 - encvh8.astype(np.float32)).astype(f8e4)
        encvh8 = np.ascontiguousarray(encvh8)
        encvl8 = np.ascontiguousarray(encvl8)
        f_loc = freqs[perm[:NLOC // 2]]
        ph = (tvec[None, :] * f_loc[:, None]).astype(np.float32) % 1.0
        cs = np.concatenate([np.cos(TWO_PI * ph), np.sin(TWO_PI * ph)],
                            axis=1)  # [NLOC//2, 2T]
        in_maps.append({
            "x0": np.ascontiguousarray(x0, f16),
            "encw": np.ascontiguousarray(enc[h][:, perm], f16),
            "encvh": encvh8,
            "encvl": encvl8,
            "decw": np.ascontiguousarray(dec[h * N + perm, :], f16),
            "cs": np.ascontiguousarray(cs, f16),
            "lmh": np.ascontiguousarray(lmh, f16),
            "umask": umask,
        })
    return in_maps


def kernel(idx, embed, encoder, encoder_v, decoder, lm_head,
           _trace=False, _tmpdir=None):
    if "nc" not in _CACHE:
        _CACHE["nc"] = _build_program()
    nc = _CACHE["nc"]
    in_maps = _host_inputs(idx, embed, encoder, encoder_v, decoder, lm_head)
    res = bass_utils.run_bass_kernel_spmd(
        nc, in_maps, core_ids=list(range(N_CORES)),
        trace=_trace, tmpdir=_tmpdir)
    _CACHE["last_results"] = res
    logits = res.results[0]["logits"].astype(np.float32).reshape(B, T, VOCAB)
    return logits
